# revision 1
# baseline (speedup 1.0000x reference)
"""Trainium2 Bass kernel: GQA causal self-attention with ALiBi + QK-RMSNorm.

Model: B=1, S=2048, DM=4096, H=32 q-heads, HKV=8 kv-heads, HD=128.
Sharding: tensor-parallel over heads across 8 cores. Core g computes
q-heads 4g..4g+3 with kv-head g, and a row-parallel partial of the output
projection; the host sums the 8 partials (the unshard for row-parallel Wo).

Layout strategy (per core):
  - x is passed transposed (XT [DM,S]) so every projection matmul contracts
    over DM on the partition axis with no on-device transposes.
  - Q,K are produced transposed ([d, s]); V natural ([s, d]).
  - RMSNorm over d (= partition axis) uses a ones-vector matmul for the
    per-position sum of squares, then a GPSIMD partition_broadcast of 1/rms.
  - Scores are computed transposed: S^T[j,i] (j=key pos on partitions,
    i=query pos on free axis). With q scaled by 1/sqrt(HD) and RMSNormed,
    |s| <= sqrt(128) and the ALiBi bias slope*(j-i) <= 0 after causal
    masking, so exp() cannot overflow and NO row-max pass is needed.
    exp bias: +slope*(j-i0) enters via the ACT per-partition bias operand,
    -slope*(i-i0) via one DVE row add; the causal mask is a precomputed
    [128,128] additive -1e30 triangle applied to diagonal blocks (GPSIMD).
  - P^T tiles feed the PV matmul as rhs with V as lhsT, accumulating O^T
    [d, i] directly in PSUM (no transposes anywhere). A ones-lhsT matmul
    accumulates the softmax denominators as a row, normalized via
    reciprocal + partition_broadcast.
  - Phases are software-pipelined: attention for query block ib runs right
    after projection slice ib, and the (PE-dense) output projection of
    block ib-1 is interleaved into the (dependency-chain-bound) attention
    of block ib to keep the PE fed.
"""

import math

import numpy as np
import ml_dtypes

import concourse.bass as bass
import concourse.bacc as bacc
import concourse.mybir as mybir
import concourse.tile as tile

F32 = mybir.dt.float32
BF16 = mybir.dt.bfloat16
AF = mybir.ActivationFunctionType
ALU = mybir.AluOpType

B, S, DM = 1, 2048, 4096
H, HKV, HD = 32, 8, 128
NC_CORES = 8
HL = H // NC_CORES          # 4 local q heads per core
EPS = 1e-6
NEG = -1.0e30
P = 128

NBF = ml_dtypes.bfloat16


def _alibi_slopes(n_heads: int) -> np.ndarray:
    start = 2 ** (-(2 ** (-(math.log2(n_heads) - 3))))
    return np.array([start * (start**i) for i in range(n_heads)], dtype=np.float32)


def build_module(s: int = S, repeat: int = 1, phases=('proj', 'attn', 'out')):
    """Build the per-core Bass module. `s` parameterized for small tests."""
    assert s % 512 == 0
    nss = s // 512            # 512-wide s slices / query blocks
    njt = s // P              # 128-wide key tiles
    ndm = DM // P             # 32 contraction tiles

    nc = bacc.Bacc(trn_type="TRN2")

    xt_d = nc.dram_tensor("xt", [DM, s], BF16, kind="ExternalInput")
    wq_d = nc.dram_tensor("wq", [DM, HL * HD], BF16, kind="ExternalInput")
    wk_d = nc.dram_tensor("wk", [DM, HD], BF16, kind="ExternalInput")
    wv_d = nc.dram_tensor("wv", [DM, HD], BF16, kind="ExternalInput")
    wo_d = nc.dram_tensor("wo", [HL * HD, DM], BF16, kind="ExternalInput")
    qnw_d = nc.dram_tensor("qnw", [HD, 1], F32, kind="ExternalInput")
    knw_d = nc.dram_tensor("knw", [HD, 1], F32, kind="ExternalInput")
    slp_d = nc.dram_tensor("slp", [P, HL], F32, kind="ExternalInput")
    nslp_d = nc.dram_tensor("nslp", [P, HL], F32, kind="ExternalInput")
    out_d = nc.dram_tensor("out", [s, DM], BF16, kind="ExternalOutput")

    with tile.TileContext(nc) as tc:
        with (
            tc.tile_pool(name="const", bufs=1) as const,
            tc.tile_pool(name="xt", bufs=2) as xt_pool,
            tc.tile_pool(name="big", bufs=1) as big,
            tc.tile_pool(name="sq", bufs=2) as sq_pool,
            tc.tile_pool(name="row1", bufs=3) as row1,
            tc.tile_pool(name="inv", bufs=2) as inv_pool,
            tc.tile_pool(name="nrow", bufs=2) as nr_pool,
            tc.tile_pool(name="jcol", bufs=2) as jc_pool,
            tc.tile_pool(name="tmp", bufs=4) as tmp_pool,
            tc.tile_pool(name="pt", bufs=4) as pt_pool,
            tc.tile_pool(name="fsb", bufs=3) as fsb_pool,
            tc.tile_pool(name="ps", bufs=8, space="PSUM") as ps,
        ):
            # ---------------- constants ----------------
            wq_sb = const.tile([P, ndm, HL * HD], BF16)
            wq_r = wq_d[:, :].rearrange("(o p) m -> p o m", p=P)
            nc.sync.dma_start(wq_sb[:, 0:ndm // 4, :], wq_r[:, 0:ndm // 4, :])
            wk_sb = const.tile([P, ndm, HD], BF16)
            nc.sync.dma_start(wk_sb, wk_d[:, :].rearrange("(o p) m -> p o m", p=P))
            wv_sb = const.tile([P, ndm, HD], BF16)
            wo_sb = const.tile([P, HL, DM], BF16)
            wo_r = wo_d[:, :].rearrange("(o p) m -> p o m", p=P)
            qnw_sb = const.tile([P, 1], F32)
            knw_sb = const.tile([P, 1], F32)
            slp_sb = const.tile([P, HL], F32)
            nslp_sb = const.tile([P, HL], F32)

            def deferred_const_loads():
                # Emitted after proj(0)'s first xt chunks: everything here is
                # first needed tens of microseconds into the kernel.
                for ch in range(1, 4):
                    o0 = ch * (ndm // 4)
                    nc.sync.dma_start(wq_sb[:, o0:o0 + ndm // 4, :],
                                      wq_r[:, o0:o0 + ndm // 4, :])
                nc.sync.dma_start(
                    wv_sb, wv_d[:, :].rearrange("(o p) m -> p o m", p=P))
                nc.sync.dma_start(qnw_sb, qnw_d[:, :])
                nc.sync.dma_start(knw_sb, knw_d[:, :])
                nc.sync.dma_start(slp_sb, slp_d[:, :])
                nc.sync.dma_start(nslp_sb, nslp_d[:, :])

            ones_f32 = const.tile([P, 1], F32)
            nc.vector.memset(ones_f32, 1.0)
            ones_sb = const.tile([P, 1], mybir.dt.float32r)
            nc.scalar.copy(ones_sb, ones_f32)
            ones_bf = const.tile([P, 1], BF16)
            nc.vector.memset(ones_bf, 1.0)
            eps_sb = const.tile([P, 1], F32)
            nc.vector.memset(eps_sb, EPS)

            # iota_row[p, f] = f ; iota_jcol[p, t] = 128*t + p
            iota_row = const.tile([P, 512], F32)
            nc.gpsimd.iota(iota_row, pattern=[[1, 512]], base=0,
                           channel_multiplier=0,
                           allow_small_or_imprecise_dtypes=True)
            iota_jcol = const.tile([P, njt], F32)
            nc.gpsimd.iota(iota_jcol, pattern=[[P, njt]], base=0,
                           channel_multiplier=1,
                           allow_small_or_imprecise_dtypes=True)

            # maskneg[p, f] = 0 where p <= f else -1e30  (additive causal
            # mask for diagonal 128x128 blocks of S^T)
            maskneg = const.tile([P, P], F32)
            nc.gpsimd.memset(maskneg, 0.0)
            nc.gpsimd.affine_select(
                out=maskneg, in_=maskneg,
                compare_op=ALU.is_ge, fill=NEG,
                base=0, pattern=[[1, P]], channel_multiplier=-1,
            )

            # ---------------- persistent activations ----------------
            # qt/ot hold only 2 query blocks (ring): the pipeline uses
            # qt of block ss right after proj(ss), and outproj consumes
            # ot of block ss-1 during attention of block ss.
            qt_sb = big.tile([P, HL, 2, 512], BF16)  # Q^T ring [d, h, ss%2, i]
            kt_sb = big.tile([P, s], BF16)           # K^T      [d, s]
            v_sb = big.tile([P, njt, HD], BF16)      # V        [s, d]
            ot_sb = big.tile([P, HL, 2, 512], BF16)  # O^T ring [d, h, ib%2, i]

            xt_r = xt_d[:, :].rearrange("(o p) t -> p o t", p=P)
            _loaded_consts = []

            def proj_slice(ss):
                s0 = ss * 512
                xt_t = xt_pool.tile([P, ndm, 512], BF16, name="xt_t")
                for ch in range(4):
                    o0 = ch * (ndm // 4)
                    nc.sync.dma_start(xt_t[:, o0:o0 + ndm // 4, :],
                                      xt_r[:, o0:o0 + ndm // 4, s0:s0 + 512])
                if ss == 0 and not _loaded_consts:
                    _loaded_consts.append(True)
                    deferred_const_loads()

                # Q (4 heads) and K projections: [d, s] (transposed)
                pq = [ps.tile([P, 512], F32, tag="ps", name=f"pq{c}")
                      for c in range(HL)]
                pk = ps.tile([P, 512], F32, tag="ps", name="pk")
                for o in range(ndm):
                    for c in range(HL):
                        nc.tensor.matmul(
                            pq[c], wq_sb[:, o, c * HD:(c + 1) * HD],
                            xt_t[:, o, :], start=(o == 0), stop=(o == ndm - 1))
                    nc.tensor.matmul(
                        pk, wk_sb[:, o, :], xt_t[:, o, :],
                        start=(o == 0), stop=(o == ndm - 1))

                # rmsnorm over d (partitions): ones-matmul sumsq -> rsqrt ->
                # partition_broadcast; fused scale+cast to bf16 on evict.
                for c in range(HL + 1):
                    src = pq[c] if c < HL else pk
                    w_sb = qnw_sb if c < HL else knw_sb
                    sqt = sq_pool.tile([P, 512], mybir.dt.float32r,
                                       tag="sq", name="sqt")
                    nc.scalar.activation(sqt, src, AF.Square)
                    psq = ps.tile([1, 512], F32, tag="ps", name="psq")
                    nc.tensor.matmul(psq, ones_sb, sqt,
                                     start=True, stop=True)
                    rms = row1.tile([1, 512], F32, tag="row1", name="rms")
                    nc.scalar.activation(rms, psq, AF.Sqrt,
                                         bias=eps_sb[:1, :], scale=1.0 / HD)
                    rec = row1.tile([1, 512], F32, tag="row1", name="rec")
                    nc.vector.reciprocal(rec, rms)
                    invb = inv_pool.tile([P, 512], F32, tag="inv", name="invb")
                    nc.gpsimd.partition_broadcast(invb, rec)
                    dst = qt_sb[:, c, ss % 2, :] if c < HL \
                        else kt_sb[:, s0:s0 + 512]
                    nc.vector.scalar_tensor_tensor(
                        out=dst, in0=src, scalar=w_sb, in1=invb,
                        op0=ALU.mult, op1=ALU.mult)

                # V projection: [s, d] natural
                for c in range(4):
                    pv = ps.tile([P, HD], F32, tag="ps", name="pv")
                    for o in range(ndm):
                        nc.tensor.matmul(
                            pv, xt_t[:, o, c * P:(c + 1) * P], wv_sb[:, o, :],
                            start=(o == 0), stop=(o == ndm - 1))
                    nc.scalar.copy(v_sb[:, 4 * ss + c, :], pv)

            def outproj_chunk(ib, mi_list):
                """Output projection for query block ib, m-slices mi_list."""
                for mi in mi_list:
                    m0 = mi * 512
                    for st_i in range(4):
                        s0 = ib * 512 + st_i * P
                        fps = ps.tile([P, 512], F32, tag="ps", name="fps")
                        for c in range(HL):
                            nc.tensor.matmul(
                                fps, ot_sb[:, c, ib % 2, st_i * P:(st_i + 1) * P],
                                wo_sb[:, c, m0:m0 + 512],
                                start=(c == 0), stop=(c == HL - 1))
                        fsb = fsb_pool.tile([P, 512], BF16, tag="fsb",
                                            name="fsb")
                        if (mi + st_i) % 2 == 0:
                            nc.scalar.copy(fsb, fps)
                        else:
                            nc.vector.tensor_copy(fsb, fps)
                        nc.sync.dma_start(out_d[s0:s0 + P, m0:m0 + 512], fsb)

            def attn_setup(ib, h):
                i0 = ib * 512
                negrow = nr_pool.tile([P, 512], F32, tag="nrow", name="negrow")
                nc.gpsimd.tensor_tensor(
                    negrow, iota_row,
                    nslp_sb[:, h:h + 1].to_broadcast([P, 512]), ALU.mult)
                # exp bias column: bias[p, jt] = slope * (128*jt + p - i0)
                jtmp = jc_pool.tile([P, njt], F32, tag="jcol", name="jtmp")
                nc.gpsimd.tensor_scalar_add(jtmp, iota_jcol, float(-i0))
                jcol = jc_pool.tile([P, njt], F32, tag="jcol", name="jcol")
                nc.gpsimd.tensor_tensor(
                    jcol, jtmp,
                    slp_sb[:, h:h + 1].to_broadcast([P, njt]), ALU.mult)
                otp = ps.tile([P, 512], F32, tag="ps", name="otp")
                lps = ps.tile([1, 512], F32, tag="ps", name="lps")
                return negrow, jcol, otp, lps

            def attn_jt(ib, h, jt, negrow, jcol, otp, lps):
                i0 = ib * 512
                jlast = 4 * (ib + 1) - 1
                j0 = jt * P
                c0 = max(0, j0 - i0)
                st = ps.tile([P, 512], F32, tag="ps", name="st")
                nc.tensor.matmul(
                    st[:, c0:], kt_sb[:, j0:j0 + P],
                    qt_sb[:, h, ib % 2, c0:],
                    start=True, stop=True)
                tmp = tmp_pool.tile([P, 512], F32, tag="tmp", name="tmp")
                nc.vector.tensor_tensor(
                    tmp[:, c0:], st[:, c0:], negrow[:, c0:], ALU.add)
                if j0 >= i0:  # diagonal block: additive causal mask
                    nc.gpsimd.tensor_tensor(
                        tmp[:, c0:c0 + P], tmp[:, c0:c0 + P],
                        maskneg, ALU.add)
                pt = pt_pool.tile([P, 512], BF16, tag="pt", name="pt")
                nc.scalar.activation(
                    pt[:, c0:], tmp[:, c0:], AF.Exp,
                    bias=jcol[:, jt:jt + 1], scale=1.0)
                # O^T accumulation: otp[d, i] += sum_j V[j, d] P^T[j, i]
                nc.tensor.matmul(
                    otp[:, c0:], v_sb[:, jt, :], pt[:, c0:],
                    start=(jt == 0), stop=(jt == jlast))
                # denominators: lps[0, i] += sum_j P^T[j, i]
                nc.tensor.matmul(
                    lps[:, c0:], ones_bf, pt[:, c0:],
                    start=(jt == 0), stop=(jt == jlast))

            def attn_finish(ib, h, otp, lps):
                lrow = row1.tile([1, 512], F32, tag="row1", name="lrow")
                nc.scalar.copy(lrow, lps)
                linv = row1.tile([1, 512], F32, tag="row1", name="linv")
                nc.vector.reciprocal(linv, lrow)
                linvb = inv_pool.tile([P, 512], F32, tag="inv", name="linvb")
                nc.gpsimd.partition_broadcast(linvb, linv)
                nc.vector.tensor_tensor(
                    ot_sb[:, h, ib % 2, :], otp, linvb, ALU.mult)

            def attn_head(ib, h):
                negrow, jcol, otp, lps = attn_setup(ib, h)
                for jt in range(4 * (ib + 1)):
                    attn_jt(ib, h, jt, negrow, jcol, otp, lps)
                attn_finish(ib, h, otp, lps)

            def attn_head_pair(ib, h0, h1):
                ctx0 = attn_setup(ib, h0)
                ctx1 = attn_setup(ib, h1)
                for jt in range(4 * (ib + 1)):
                    attn_jt(ib, h0, jt, *ctx0)
                    attn_jt(ib, h1, jt, *ctx1)
                attn_finish(ib, h0, ctx0[2], ctx0[3])
                attn_finish(ib, h1, ctx1[2], ctx1[3])

            for _rep in range(repeat):
                # pipelined: proj(ss) -> attention(ss) with outproj(ss-1)
                # interleaved at head granularity to fill PE bubbles.
                for ss in range(nss):
                    if 'proj' in phases:
                        proj_slice(ss)
                    if 'attn' in phases:
                        if ss == 0:
                            for h in range(HL):
                                attn_head(ss, h)
                                if _rep == 0:
                                    # Wo not needed until block 1: load it
                                    # in attention block 0's PE bubbles.
                                    m0 = h * (DM // 4)
                                    nc.sync.dma_start(
                                        wo_sb[:, :, m0:m0 + DM // 4],
                                        wo_r[:, :, m0:m0 + DM // 4])
                        else:
                            for h in range(HL):
                                attn_head(ss, h)
                                if 'out' in phases:
                                    outproj_chunk(ss - 1, [2 * h, 2 * h + 1])
                if 'out' in phases and 'attn' in phases:
                    outproj_chunk(nss - 1, list(range(8)))

    nc.finalize()
    return nc


def shard_inputs(x, Wq, Wk, Wv, Wo, q_norm_w, k_norm_w, s=S):
    """Host-side shard + layout prep. Returns per-core input maps."""
    slopes = _alibi_slopes(H)
    xt = np.ascontiguousarray(x.reshape(s, DM).T).astype(NBF)
    qnw = (np.asarray(q_norm_w, np.float32) / math.sqrt(HD)).reshape(HD, 1)
    knw = np.asarray(k_norm_w, np.float32).reshape(HD, 1).copy()
    in_maps = []
    for g in range(NC_CORES):
        qs = g * HL * HD
        sl = slopes[g * HL:(g + 1) * HL]
        in_maps.append({
            "xt": xt,
            "wq": np.ascontiguousarray(Wq[qs:qs + HL * HD, :].T).astype(NBF),
            "wk": np.ascontiguousarray(Wk[g * HD:(g + 1) * HD, :].T).astype(NBF),
            "wv": np.ascontiguousarray(Wv[g * HD:(g + 1) * HD, :].T).astype(NBF),
            "wo": np.ascontiguousarray(Wo[:, qs:qs + HL * HD].T).astype(NBF),
            "qnw": qnw,
            "knw": knw,
            "slp": np.ascontiguousarray(
                np.broadcast_to(sl, (P, HL))).astype(np.float32),
            "nslp": np.ascontiguousarray(
                np.broadcast_to(-sl, (P, HL))).astype(np.float32),
        })
    return in_maps


_MODULE_CACHE = {}
LAST_RESULT = None


def _get_module(s=S):
    if s not in _MODULE_CACHE:
        _MODULE_CACHE[s] = build_module(s)
    return _MODULE_CACHE[s]


def kernel(x, Wq, Wk, Wv, Wo, q_norm_w, k_norm_w, **run_kwargs):
    global LAST_RESULT
    from concourse.bass_utils import run_bass_kernel_spmd

    x = np.asarray(x)
    in_maps = shard_inputs(np.asarray(x), np.asarray(Wq), np.asarray(Wk),
                           np.asarray(Wv), np.asarray(Wo),
                           np.asarray(q_norm_w), np.asarray(k_norm_w))
    nc = _get_module(S)
    res = run_bass_kernel_spmd(nc, in_maps, core_ids=list(range(NC_CORES)),
                               **run_kwargs)
    LAST_RESULT = res
    acc = np.zeros((S, DM), np.float32)
    for r in res.results:
        acc += r["out"].astype(np.float32)
    return acc.reshape(B, S, DM)



# revision 61
# speedup vs baseline: 1.2053x; 1.2053x over previous
"""Trainium2 Bass kernel: GQA causal self-attention with ALiBi + QK-RMSNorm.

Model: B=1, S=2048, DM=4096, H=32 q-heads, HKV=8 kv-heads, HD=128.
Sharding: tensor-parallel over heads across 8 cores. Core g computes
q-heads 4g..4g+3 with kv-head g, and a row-parallel partial of the output
projection; the host sums the 8 partials (the unshard for row-parallel Wo).

Layout strategy (per core):
  - x is passed transposed (XT [DM,S]) so every projection matmul contracts
    over DM on the partition axis with no on-device transposes.
  - Q,K are produced transposed ([d, s]); V natural ([s, d]).
  - RMSNorm over d (= partition axis) uses a ones-vector matmul for the
    per-position sum of squares, then a GPSIMD partition_broadcast of 1/rms.
  - Scores are computed transposed: S^T[j,i] (j=key pos on partitions,
    i=query pos on free axis). With q scaled by 1/sqrt(HD) and RMSNormed,
    |s| <= sqrt(128) and the ALiBi bias slope*(j-i) <= 0 after causal
    masking, so exp() cannot overflow and NO row-max pass is needed.
    exp bias: +slope*(j-i0) enters via the ACT per-partition bias operand,
    -slope*(i-i0) via one row add (DVE/Pool alternating); the causal mask is
    a precomputed [128,128] additive -1e30 triangle on diagonal blocks.
  - P^T tiles feed the PV matmul as rhs with V as lhsT, accumulating O^T
    [d, i] directly in PSUM (no transposes anywhere). A ones-lhsT matmul
    accumulates the softmax denominators as a row, normalized via
    reciprocal + partition_broadcast.

Scheduling strategy (v2):
  - xt streams in 8 chunks per 512-slice ([P, 4o, 512]); the startup DMA
    order interleaves wq pieces with xt chunks (first loads split in half)
    so the PE starts ~3us in. wk/wv are host-packed to the SBUF layout so
    their DMA descriptors are 8KB runs (the [DM, HD] layout would give
    256B descriptors, which the DMA does at half throughput).
  - One PE "filler queue" per attention phase: attention for block ss runs
    with proj(ss+1) and outproj(ss-1) matmul steps pulled into the exp-
    latency bubbles of the jt pipeline (st(jt+depth) is emitted depth=2
    iterations ahead, 3 in the filler-poor last phase; otp/lps trail once
    exp(jt) lands). Q-projection filler runs as head PAIRS iterating
    o-major, so each xt chunk is consumed slower than its DMA delivers it
    (chunk-major sweeps outran the serial DMA engine and stalled). Each
    head's first score matmuls are emitted during the previous head's last
    iteration (attn_prologue chaining), and phase(ss+1)'s head-0 prologue
    is emitted under the last 8 filler steps of phase ss.
  - RMSNorm is split: chunk epilogues bank the sum-of-squares row and a
    raw bf16 copy (freeing PSUM); the ACT-table-switching Sqrt runs in one
    batched pocket per phase (2 LoadActFuncSet round trips per phase
    instead of ~10 -- Sqrt and Exp live in different ACT table sets).
  - Per-head ALiBi bias rows/columns are precomputed once (masters), so a
    head costs no setup.
  - Output stores: 4 PSUM->SBUF cast copies (alternating ACT/DVE; GPSIMD
    cannot read PSUM) into one [P,4,512] tile, then ONE merged DMA; the
    final m-slices use per-quad DMAs to shorten the kernel tail.
"""

import math
from collections import deque

import numpy as np
import ml_dtypes

import concourse.bass as bass
import concourse.bacc as bacc
import concourse.mybir as mybir
import concourse.tile as tile

F32 = mybir.dt.float32
BF16 = mybir.dt.bfloat16
AF = mybir.ActivationFunctionType
ALU = mybir.AluOpType

B, S, DM = 1, 2048, 4096
H, HKV, HD = 32, 8, 128
NC_CORES = 8
HL = H // NC_CORES          # 4 local q heads per core
EPS = 1e-6
NEG = -1.0e30
P = 128

NBF = ml_dtypes.bfloat16


def _alibi_slopes(n_heads: int) -> np.ndarray:
    start = 2 ** (-(2 ** (-(math.log2(n_heads) - 3))))
    return np.array([start * (start**i) for i in range(n_heads)], dtype=np.float32)


def build_module(s: int = S, repeat: int = 1, phases=('proj', 'attn', 'out')):
    """Build the per-core Bass module. `s` parameterized for small tests."""
    assert s % 512 == 0
    nss = s // 512            # 512-wide s slices / query blocks
    njt = s // P              # 128-wide key tiles
    ndm = DM // P             # 32 contraction tiles
    nxc = ndm // 4            # 8 xt chunks per slice (4 o's each)
    NJ = 4 * nss              # jcol master width
    RB = 4 * (nss - 1)        # ridx bias: ridx = jt - 4*ss + RB in [0, NJ)

    nc = bacc.Bacc(trn_type="TRN2")

    xt_d = nc.dram_tensor("xt", [DM, s], BF16, kind="ExternalInput")
    wq_d = nc.dram_tensor("wq", [DM, HL * HD], BF16, kind="ExternalInput")
    wk_d = nc.dram_tensor("wk", [P, (DM // P) * HD], BF16, kind="ExternalInput")
    wv_d = nc.dram_tensor("wv", [P, (DM // P) * HD], BF16, kind="ExternalInput")
    wo_d = nc.dram_tensor("wo", [HL * HD, DM], BF16, kind="ExternalInput")
    qnw_d = nc.dram_tensor("qnw", [HD, 1], F32, kind="ExternalInput")
    knw_d = nc.dram_tensor("knw", [HD, 1], F32, kind="ExternalInput")
    slp_d = nc.dram_tensor("slp", [P, HL], F32, kind="ExternalInput")
    nslp_d = nc.dram_tensor("nslp", [P, HL], F32, kind="ExternalInput")
    out_d = nc.dram_tensor("out", [s, DM], BF16, kind="ExternalOutput")

    with tile.TileContext(nc) as tc:
        with (
            tc.tile_pool(name="const", bufs=1) as const,
            tc.tile_pool(name="xtc", bufs=10) as xtc_pool,
            tc.tile_pool(name="big", bufs=1) as big,
            tc.tile_pool(name="sq", bufs=2) as sq_pool,
            tc.tile_pool(name="row1", bufs=3) as row1,
            tc.tile_pool(name="msr", bufs=6) as msr_pool,
            tc.tile_pool(name="raw", bufs=6) as raw_pool,
            tc.tile_pool(name="inv", bufs=2) as inv_pool,
            tc.tile_pool(name="tmp", bufs=4) as tmp_pool,
            tc.tile_pool(name="pt", bufs=4) as pt_pool,
            tc.tile_pool(name="fsb", bufs=2) as fsb_pool,
            tc.tile_pool(name="ps", bufs=8, space="PSUM") as ps,
        ):
            # ---------------- small constants (loads deferred into proj) --
            qnw_sb = const.tile([P, 1], F32)
            knw_sb = const.tile([P, 1], F32)
            slp_sb = const.tile([P, HL], F32)
            nslp_sb = const.tile([P, HL], F32)

            def load_small_consts():
                nc.sync.dma_start(qnw_sb, qnw_d[:, :])
                nc.sync.dma_start(knw_sb, knw_d[:, :])
                nc.sync.dma_start(slp_sb, slp_d[:, :])
                nc.sync.dma_start(nslp_sb, nslp_d[:, :])

            # weights tiles (loads interleaved below / deferred)
            wq_sb = const.tile([P, ndm, HL * HD], BF16)
            wq_r = wq_d[:, :].rearrange("(o p) m -> p o m", p=P)
            wk_sb = const.tile([P, ndm, HD], BF16)
            wv_sb = const.tile([P, ndm, HD], BF16)
            wo_sb = const.tile([P, HL, DM], BF16)
            wo_r = wo_d[:, :].rearrange("(o p) m -> p o m", p=P)

            ones_f32 = const.tile([P, 1], F32)
            nc.vector.memset(ones_f32, 1.0)
            ones_sb = const.tile([P, 1], mybir.dt.float32r)
            nc.scalar.copy(ones_sb, ones_f32)
            ones_bf = const.tile([P, 1], BF16)
            nc.vector.memset(ones_bf, 1.0)
            eps_sb = const.tile([P, 1], F32)
            nc.vector.memset(eps_sb, EPS)

            # per-head ALiBi masters:
            #   negrowM[p, h, f] = -slope_h * f           (query-col row add)
            #   jcolM[p, h, t]   = slope_h * (128*(t-RB) + p)  (exp bias; at
            #       t = jt - 4*ss + RB it equals slope*(128*jt + p - 512*ss))
            iota_row = const.tile([P, 512], F32)
            nc.gpsimd.iota(iota_row, pattern=[[1, 512]], base=0,
                           channel_multiplier=0,
                           allow_small_or_imprecise_dtypes=True)
            iota_j = const.tile([P, NJ], F32)
            nc.gpsimd.iota(iota_j, pattern=[[P, NJ]], base=-RB * P,
                           channel_multiplier=1,
                           allow_small_or_imprecise_dtypes=True)
            negrowM = const.tile([P, HL, 512], F32)
            jcolM = const.tile([P, HL, NJ], F32)

            def build_masters():
                for h in range(HL):
                    nc.gpsimd.tensor_tensor(
                        negrowM[:, h, :], iota_row,
                        nslp_sb[:, h:h + 1].to_broadcast([P, 512]), ALU.mult)
                    nc.gpsimd.tensor_tensor(
                        jcolM[:, h, :], iota_j,
                        slp_sb[:, h:h + 1].to_broadcast([P, NJ]), ALU.mult)

            # maskneg[p, f] = 0 where p <= f else -1e30  (additive causal
            # mask for diagonal 128x128 blocks of S^T)
            maskneg = const.tile([P, P], F32)
            nc.gpsimd.memset(maskneg, 0.0)
            nc.gpsimd.affine_select(
                out=maskneg, in_=maskneg,
                compare_op=ALU.is_ge, fill=NEG,
                base=0, pattern=[[1, P]], channel_multiplier=-1,
            )

            # ---------------- persistent activations ----------------
            qt_sb = big.tile([P, HL, 2, 512], BF16)  # Q^T ring [d, h, ss%2, i]
            kt_sb = big.tile([P, s], BF16)           # K^T      [d, s]
            v_sb = big.tile([P, njt, HD], BF16)      # V        [s, d]
            ot_sb = big.tile([P, HL, 2, 512], BF16)  # O^T ring [d, h, ib%2, i]

            xt_r = xt_d[:, :].rearrange("(o p) t -> p o t", p=P)

            # xt chunk bookkeeping: chunks[(ss, c)] = sbuf tile [P, 4, 512]
            xt_chunks = {}

            def load_xt_chunk(ss, c):
                t = xtc_pool.tile([P, 4, 512], BF16, tag="xtc",
                                  name=f"xt{ss}_{c}")
                nc.sync.dma_start(
                    t, xt_r[:, 4 * c:4 * c + 4, ss * 512:ss * 512 + 512])
                xt_chunks[(ss, c)] = t

            def xt_o(ss, o):
                return xt_chunks[(ss, o // 4)][:, o % 4, :]

            # ---------------- rmsnorm chain (non-PE parts) ----------------
            def rms_chain_pre(src):
                """Square the PSUM tile; returns sq tile (ACT)."""
                sqt = sq_pool.tile([P, 512], mybir.dt.float32r,
                                   tag="sq", name="sqt")
                nc.scalar.activation(sqt, src, AF.Square)
                return sqt

            def rms_chain_post(src, psq, w_sb, dst):
                """After PE computed psq = ones^T @ sqt: finish the norm."""
                rms = row1.tile([1, 512], F32, tag="row1", name="rms")
                nc.scalar.activation(rms, psq, AF.Sqrt,
                                     bias=eps_sb[:1, :], scale=1.0 / HD)
                rec = row1.tile([1, 512], F32, tag="row1", name="rec")
                nc.vector.reciprocal(rec, rms)
                invb = inv_pool.tile([P, 512], F32, tag="inv", name="invb")
                nc.gpsimd.partition_broadcast(invb, rec)
                nc.vector.scalar_tensor_tensor(
                    out=dst, in0=src, scalar=w_sb, in1=invb,
                    op0=ALU.mult, op1=ALU.mult)

            # ---------------- initial projection (slice 0) ----------------
            def proj_initial():
                """proj(0): DMA-paced. Q pass (o-major over 4 heads), K pass,
                V pass; weight quarters interleave with xt chunks."""
                # Q pass
                pq = [ps.tile([P, 512], F32, tag="ps", name=f"pq{c}")
                      for c in range(HL)]
                for o in range(ndm):
                    if o == 0:          # finest first loads: PE starts ~3us
                        nc.sync.dma_start(wq_sb[:, 0:2, :], wq_r[:, 0:2, :])
                        t0 = xtc_pool.tile([P, 4, 512], BF16, tag="xtc",
                                           name="xt0_0")
                        nc.sync.dma_start(t0[:, 0:2, :], xt_r[:, 0:2, 0:512])
                        nc.sync.dma_start(wq_sb[:, 2:4, :], wq_r[:, 2:4, :])
                        nc.sync.dma_start(t0[:, 2:4, :], xt_r[:, 2:4, 0:512])
                        xt_chunks[(0, 0)] = t0
                    elif o == 2:
                        nc.sync.dma_start(wq_sb[:, 4:8, :], wq_r[:, 4:8, :])
                    elif o % 8 == 0:    # wq quarter q = o//8
                        q = o // 8
                        nc.sync.dma_start(
                            wq_sb[:, 8 * q:8 * q + 8, :],
                            wq_r[:, 8 * q:8 * q + 8, :])
                    if o % 4 == 0 and o > 0:
                        load_xt_chunk(0, o // 4)
                    if o == 1:
                        load_small_consts()
                        build_masters()
                    if o == ndm - 8:
                        nc.sync.dma_start(
                            wk_sb, wk_d[:, :].rearrange(
                                "p (o m) -> p o m", o=ndm))
                        nc.sync.dma_start(
                            wv_sb, wv_d[:, :].rearrange(
                                "p (o m) -> p o m", o=ndm))
                    for c in range(HL):
                        nc.tensor.matmul(
                            pq[c], wq_sb[:, o, c * HD:(c + 1) * HD],
                            xt_o(0, o), start=(o == 0), stop=(o == ndm - 1))
                # K pass (Q rmsnorm chains interleave into it)
                pk = ps.tile([P, 512], F32, tag="ps", name="pk")
                sqs = {}
                for o in range(ndm):
                    nc.tensor.matmul(pk, wk_sb[:, o, :], xt_o(0, o),
                                     start=(o == 0), stop=(o == ndm - 1))
                    if o % 8 == 1 and o // 8 < HL:
                        c = o // 8
                        sqs[c] = rms_chain_pre(pq[c])
                    elif o % 8 == 5 and o // 8 < HL:
                        c = o // 8
                        psq = ps.tile([1, 512], F32, tag="ps", name="psq")
                        nc.tensor.matmul(psq, ones_sb, sqs[c],
                                         start=True, stop=True)
                        rms_chain_post(pq[c], psq, qnw_sb,
                                       qt_sb[:, c, 0, :])
                # K rmsnorm chain, hidden under the V pass
                sqk = rms_chain_pre(pk)
                psqk = ps.tile([1, 512], F32, tag="ps", name="psqk")
                nc.tensor.matmul(psqk, ones_sb, sqk, start=True, stop=True)
                rms_chain_post(pk, psqk, knw_sb, kt_sb[:, 0:512])
                # V pass before the prologue: covers the K rms-chain
                # latency so head 0's first score matmul finds kt ready
                for c in range(4):
                    pv = ps.tile([P, HD], F32, tag="ps", name="pv")
                    for o in range(ndm):
                        nc.tensor.matmul(
                            pv, xt_chunks[(0, o // 4)][:, o % 4,
                                                       c * P:(c + 1) * P],
                            wv_sb[:, o, :],
                            start=(o == 0), stop=(o == ndm - 1))
                    nc.scalar.copy(v_sb[:, c, :], pv)
                return attn_prologue(0, 0) if 'attn' in phases else None

            # ---------------- filler step generators ----------------
            # Each filler step is a closure emitting ~1-2 PE matmuls (plus
            # trailing non-PE ops). Steps are pulled into attention bubbles.

            def rms_defer_a(src, pending, w_sb, dst):
                """Chunk epilogue: sum-of-squares row + raw copy to SBUF so
                the PSUM accumulator frees now; the ACT-table-switching sqrt
                runs later in one batched pocket per phase."""
                sqt = rms_chain_pre(src)
                psq = ps.tile([1, 512], F32, tag="ps", name="psq")
                nc.tensor.matmul(psq, ones_sb, sqt, start=True, stop=True)
                msrow = msr_pool.tile([1, 512], F32, tag="msr", name="msr")
                nc.scalar.copy(msrow, psq)
                raw = raw_pool.tile([P, 512], BF16, tag="raw", name="raw")
                cast_copy(raw, src)
                pending.append((msrow, raw, w_sb, dst))

            def rms_defer_b(msrow, raw, w_sb, dst):
                rms = row1.tile([1, 512], F32, tag="row1", name="rms")
                nc.scalar.activation(rms, msrow, AF.Sqrt,
                                     bias=eps_sb[:1, :], scale=1.0 / HD)
                rec = row1.tile([1, 512], F32, tag="row1", name="rec")
                nc.vector.reciprocal(rec, rms)
                invb = inv_pool.tile([P, 512], F32, tag="inv", name="invb")
                nc.gpsimd.partition_broadcast(invb, rec)
                nc.vector.scalar_tensor_tensor(
                    out=dst, in0=raw, scalar=w_sb, in1=invb,
                    op0=ALU.mult, op1=ALU.mult)

            def proj_q_pair_steps(ss, c0, c1, pending):
                """Q projection of heads c0,c1 for slice ss, o-major so a
                given xt chunk is consumed at half the single-head rate
                (stays behind the chunk DMA arrival): 64 steps."""
                state = {}

                def step(o, c):
                    def f():
                        if o == 0:
                            state[c] = ps.tile([P, 512], F32, tag="ps",
                                               name=f"pq{ss}_{c}")
                        nc.tensor.matmul(
                            state[c], wq_sb[:, o, c * HD:(c + 1) * HD],
                            xt_o(ss, o), start=(o == 0), stop=(o == ndm - 1))
                        if o == ndm - 1:
                            rms_defer_a(state[c], pending, qnw_sb,
                                        qt_sb[:, c, ss % 2, :])
                    return f
                steps = []
                for o in range(ndm):
                    steps.append(step(o, c0))
                    steps.append(step(o, c1))
                return steps

            def proj_k_steps(ss, pending):
                state = {}

                def step(o):
                    def f():
                        if o == 0:
                            state['pk'] = ps.tile([P, 512], F32, tag="ps",
                                                  name=f"pk{ss}")
                        nc.tensor.matmul(
                            state['pk'], wk_sb[:, o, :], xt_o(ss, o),
                            start=(o == 0), stop=(o == ndm - 1))
                        if o == ndm - 1:
                            rms_defer_a(state['pk'], pending, knw_sb,
                                        kt_sb[:, ss * 512:ss * 512 + 512])
                    return f
                return [step(o) for o in range(ndm)]

            def proj_v_steps(ss, cs=(0, 1, 2, 3)):
                """V projection: pos-chunks x 4 steps of 8 matmuls."""
                state = {}
                steps = []

                def step(c, g):
                    def f():
                        if g == 0:
                            state[c] = ps.tile([P, HD], F32, tag="ps",
                                               name=f"pv{ss}_{c}")
                        for o in range(8 * g, 8 * g + 8):
                            nc.tensor.matmul(
                                state[c],
                                xt_chunks[(ss, o // 4)][:, o % 4,
                                                        c * P:(c + 1) * P],
                                wv_sb[:, o, :],
                                start=(o == 0), stop=(o == ndm - 1))
                        if g == 3:
                            nc.scalar.copy(v_sb[:, 4 * ss + c, :], state[c])
                    return f
                for c in cs:
                    for g in range(4):
                        steps.append(step(c, g))
                return steps

            _copy_rr = [0]
            _copy_mode = [2]    # 2 = alternate; 3 = 2xACT:1xDVE (DVE-heavy
                                # attention phases)

            def cast_copy(dst, src):
                """PSUM f32 -> SBUF bf16 cast copy over the two PSUM-capable
                non-PE engines (GPSIMD cannot read PSUM)."""
                r = _copy_rr[0] = (_copy_rr[0] + 1) % _copy_mode[0]
                if r != 0:
                    nc.scalar.copy(dst, src)
                else:
                    nc.vector.tensor_copy(dst, src)

            def outproj_steps(ib, mi, split_dma=False):
                """Output projection block ib, m-slice mi: 8 steps of 2
                matmuls; one merged DMA after the 4 quads (or 2 half DMAs
                when split_dma, to shorten the kernel tail)."""
                m0 = mi * 512
                state = {}
                steps = []

                def step(st_i, half):
                    def f():
                        if half == 0:
                            state['fps'] = ps.tile([P, 512], F32, tag="ps",
                                                   name=f"fps{ib}_{mi}")
                            if st_i == 0:
                                state['fsb'] = fsb_pool.tile(
                                    [P, 4, 512], BF16, tag="fsb", name="fsb")
                        for c in (0, 1) if half == 0 else (2, 3):
                            nc.tensor.matmul(
                                state['fps'],
                                ot_sb[:, c, ib % 2, st_i * P:(st_i + 1) * P],
                                wo_sb[:, c, m0:m0 + 512],
                                start=(c == 0), stop=(c == HL - 1))
                        if half == 1:
                            cast_copy(state['fsb'][:, st_i, :], state['fps'])
                            if split_dma:   # per-quad DMA: shortest tail
                                dst = out_d[ib * 512 + st_i * P:
                                            ib * 512 + (st_i + 1) * P,
                                            m0:m0 + 512]
                                nc.sync.dma_start(dst, state['fsb'][:, st_i, :])
                            elif st_i == 3:
                                dst = out_d[ib * 512:ib * 512 + 512,
                                            m0:m0 + 512]
                                nc.sync.dma_start(
                                    dst.rearrange("(st p) m -> p st m", p=P),
                                    state['fsb'])
                    return f
                for st_i in range(4):
                    steps.append(step(st_i, 0))
                    steps.append(step(st_i, 1))
                return steps

            # ---------------- attention ----------------
            def attn_prologue(ss, h):
                """Allocate the head's PSUM accumulators and emit its first
                two score matmuls. Called from the PREVIOUS head's last jt
                iteration so the exp chain of head h warms while the PE is
                still busy, killing the head-boundary bubble."""
                i0 = ss * 512
                jtend = 4 * (ss + 1)
                # last phase has little filler: run a deeper score pipeline
                # (PSUM has room there -- no proj-chunk accumulators live)
                depth = 2 if ss + 1 < nss else 3
                ctx = {'otp': ps.tile([P, 512], F32, tag="ps", name="otp"),
                       'lps': ps.tile([1, 512], F32, tag="ps", name="lps"),
                       'sts': {}, 'depth': depth}

                def emit_st(jt):
                    j0 = jt * P
                    c0 = max(0, j0 - i0)
                    stt = ps.tile([P, 512], F32, tag="ps", name="st")
                    nc.tensor.matmul(
                        stt[:, c0:], kt_sb[:, j0:j0 + P],
                        qt_sb[:, h, ss % 2, c0:], start=True, stop=True)
                    ctx['sts'][jt] = (stt, c0)

                ctx['emit_st'] = emit_st
                for jt in range(min(depth, jtend)):
                    emit_st(jt)
                return ctx

            def attn_head(ss, h, pull, ctx, next_prologue=None):
                """Body of one head's attention; `ctx` from attn_prologue.
                `next_prologue` (if set) is invoked during the last jt
                iteration and its result returned."""
                i0 = ss * 512
                jtend = 4 * (ss + 1)
                jlast = jtend - 1
                otp, lps, sts = ctx['otp'], ctx['lps'], ctx['sts']
                emit_st = ctx['emit_st']
                depth = ctx['depth']
                nctx = None
                for jt in range(jtend):
                    stt, c0 = sts.pop(jt)
                    tmp = tmp_pool.tile([P, 512], F32, tag="tmp", name="tmp")
                    nc.vector.tensor_tensor(
                        tmp[:, c0:], stt[:, c0:], negrowM[:, h, c0:], ALU.add)
                    if jt * P >= i0:  # diagonal block: additive causal mask
                        nc.gpsimd.tensor_tensor(
                            tmp[:, c0:c0 + P], tmp[:, c0:c0 + P],
                            maskneg, ALU.add)
                    pt = pt_pool.tile([P, 512], BF16, tag="pt", name="pt")
                    ridx = jt - 4 * ss + RB
                    nc.scalar.activation(
                        pt[:, c0:], tmp[:, c0:], AF.Exp,
                        bias=jcolM[:, h, ridx:ridx + 1], scale=1.0)
                    if jt + depth < jtend:
                        emit_st(jt + depth)
                    pull(3 if jt < jlast else 1)
                    if jt == jlast and next_prologue is not None:
                        nctx = next_prologue()
                    nc.tensor.matmul(
                        otp[:, c0:], v_sb[:, jt, :], pt[:, c0:],
                        start=(jt == 0), stop=(jt == jlast))
                    nc.tensor.matmul(
                        lps[:, c0:], ones_bf, pt[:, c0:],
                        start=(jt == 0), stop=(jt == jlast))
                # normalize
                lrow = row1.tile([1, 512], F32, tag="row1", name="lrow")
                nc.scalar.copy(lrow, lps)
                linv = row1.tile([1, 512], F32, tag="row1", name="linv")
                nc.vector.reciprocal(linv, lrow)
                linvb = inv_pool.tile([P, 512], F32, tag="inv", name="linvb")
                nc.gpsimd.partition_broadcast(linvb, linv)
                nc.vector.tensor_tensor(
                    ot_sb[:, h, ss % 2, :], otp, linvb, ALU.mult)
                return nctx

            # ---------------- phase schedule ----------------
            def phase(ss, ctx):
                """attn(ss) + filler proj(ss+1) + outproj(ss-1). `ctx` is
                head 0's prologue (emitted by the previous phase); returns
                the next phase's head-0 prologue ctx."""
                _copy_mode[0] = 2
                # prefetch DMAs for the next slice; wo afterwards during
                # phase 0 (xt chunks gate proj(1) filler NOW, wo is not
                # read until outproj(0) in phase 1)
                if ss + 1 < nss:
                    for c in range(nxc):
                        load_xt_chunk(ss + 1, c)
                if ss == 0:
                    for q in range(4):
                        nc.sync.dma_start(
                            wo_sb[:, :, q * (DM // 4):(q + 1) * (DM // 4)],
                            wo_r[:, :, q * (DM // 4):(q + 1) * (DM // 4)])

                fill = deque()
                pending = []
                if ss + 1 < nss and 'proj' in phases:
                    qs = [proj_q_pair_steps(ss + 1, 0, 1, pending),
                          proj_q_pair_steps(ss + 1, 2, 3, pending)]
                    ks = proj_k_steps(ss + 1, pending)
                    vs = proj_v_steps(ss + 1)
                else:
                    qs, ks, vs = [], None, None
                if ss >= 1 and 'out' in phases:
                    os_ = [outproj_steps(ss - 1, mi) for mi in range(8)]
                else:
                    os_ = []
                # interleave: outproj mi-pairs between proj chunks (paced by
                # xt arrival); K early so the rsqrt pocket (right after q3)
                # finishes well before the next phase needs qt/kt, with the
                # remaining outproj + V steps as padding behind it.
                def chain_step(i):
                    def f():
                        rms_defer_b(*pending[i])
                    return f
                order = []
                proj_units = ([qs[0], ks, qs[1],
                               [chain_step(i) for i in range(HL + 1)], vs]
                              if ks is not None else [])
                ou = list(os_)
                # zip: outproj pair, proj unit, outproj pair, proj unit, ...
                pi = 0
                for i in range(max(len(ou), len(proj_units))):
                    if i < len(ou):
                        order.append(ou[i])
                    if pi < len(proj_units):
                        order.append(proj_units[pi])
                        pi += 1
                for lst in order:
                    fill.extend(lst)

                def pull(n):
                    for _ in range(n):
                        if fill:
                            fill.popleft()()

                total = len(fill)
                if ctx is None and 'attn' in phases:
                    ctx = attn_prologue(ss, 0)
                for h in range(HL):
                    if 'attn' in phases:
                        nxt = None
                        if h + 1 < HL:
                            def nxt(hh=h + 1):
                                return attn_prologue(ss, hh)
                        ctx = attn_head(ss, h, pull, ctx, nxt)
                    # drain this head's share of the filler, holding back a
                    # few steps to cover the next phase's exp warm-up
                    hold = 8 if h == HL - 1 else 0
                    target = (total * (h + 1)) // HL - hold
                    while len(fill) > max(0, total - target):
                        fill.popleft()()
                nctx = None
                if ss + 1 < nss and 'attn' in phases:
                    nctx = attn_prologue(ss + 1, 0)
                while fill:
                    fill.popleft()()
                return nctx

            def outproj_final(ib):
                _copy_mode[0] = 2
                for mi in range(8):
                    for st in outproj_steps(ib, mi, split_dma=(mi >= 6)):
                        st()

            for _rep in range(repeat):
                ctx = None
                if 'proj' in phases:
                    ctx = proj_initial()
                for ss in range(nss):
                    ctx = phase(ss, ctx)
                if 'out' in phases and 'attn' in phases:
                    outproj_final(nss - 1)

    nc.finalize()
    return nc


def _prep_kv(w):
    """[HD, DM] weight -> [P, ndm*HD] laid out as [p][o][m] (8KB runs)."""
    ndm = DM // P
    wt = np.ascontiguousarray(w.T)                       # [DM, HD]
    return np.ascontiguousarray(
        wt.reshape(ndm, P, HD).transpose(1, 0, 2).reshape(P, ndm * HD)
    ).astype(NBF)


def shard_inputs(x, Wq, Wk, Wv, Wo, q_norm_w, k_norm_w, s=S):
    """Host-side shard + layout prep. Returns per-core input maps."""
    slopes = _alibi_slopes(H)
    xt = np.ascontiguousarray(x.reshape(s, DM).T).astype(NBF)
    qnw = (np.asarray(q_norm_w, np.float32) / math.sqrt(HD)).reshape(HD, 1)
    knw = np.asarray(k_norm_w, np.float32).reshape(HD, 1).copy()
    in_maps = []
    for g in range(NC_CORES):
        qs = g * HL * HD
        sl = slopes[g * HL:(g + 1) * HL]
        in_maps.append({
            "xt": xt,
            "wq": np.ascontiguousarray(Wq[qs:qs + HL * HD, :].T).astype(NBF),
            "wk": _prep_kv(Wk[g * HD:(g + 1) * HD, :]),
            "wv": _prep_kv(Wv[g * HD:(g + 1) * HD, :]),
            "wo": np.ascontiguousarray(Wo[:, qs:qs + HL * HD].T).astype(NBF),
            "qnw": qnw,
            "knw": knw,
            "slp": np.ascontiguousarray(
                np.broadcast_to(sl, (P, HL))).astype(np.float32),
            "nslp": np.ascontiguousarray(
                np.broadcast_to(-sl, (P, HL))).astype(np.float32),
        })
    return in_maps


_MODULE_CACHE = {}
LAST_RESULT = None


def _get_module(s=S):
    if s not in _MODULE_CACHE:
        _MODULE_CACHE[s] = build_module(s)
    return _MODULE_CACHE[s]


def kernel(x, Wq, Wk, Wv, Wo, q_norm_w, k_norm_w, **run_kwargs):
    global LAST_RESULT
    from concourse.bass_utils import run_bass_kernel_spmd

    x = np.asarray(x)
    in_maps = shard_inputs(np.asarray(x), np.asarray(Wq), np.asarray(Wk),
                           np.asarray(Wv), np.asarray(Wo),
                           np.asarray(q_norm_w), np.asarray(k_norm_w))
    nc = _get_module(S)
    res = run_bass_kernel_spmd(nc, in_maps, core_ids=list(range(NC_CORES)),
                               **run_kwargs)
    LAST_RESULT = res
    acc = np.zeros((S, DM), np.float32)
    for r in res.results:
        acc += r["out"].astype(np.float32)
    return acc.reshape(B, S, DM)


# revision 70
# speedup vs baseline: 1.2073x; 1.0017x over previous
"""Trainium2 Bass kernel: GQA causal self-attention with ALiBi + QK-RMSNorm.

Model: B=1, S=2048, DM=4096, H=32 q-heads, HKV=8 kv-heads, HD=128.
Sharding: tensor-parallel over heads across 8 cores. Core g computes
q-heads 4g..4g+3 with kv-head g, and a row-parallel partial of the output
projection; the host sums the 8 partials (the unshard for row-parallel Wo).

Layout strategy (per core):
  - x is passed transposed (XT [DM,S]) so every projection matmul contracts
    over DM on the partition axis with no on-device transposes.
  - Q,K are produced transposed ([d, s]); V natural ([s, d]).
  - RMSNorm over d (= partition axis) uses a ones-vector matmul for the
    per-position sum of squares, then a GPSIMD partition_broadcast of 1/rms.
  - Scores are computed transposed: S^T[j,i] (j=key pos on partitions,
    i=query pos on free axis). With q scaled by 1/sqrt(HD) and RMSNormed,
    |s| <= sqrt(128) and the ALiBi bias slope*(j-i) <= 0 after causal
    masking, so exp() cannot overflow and NO row-max pass is needed.
    exp bias: +slope*(j-i0) enters via the ACT per-partition bias operand,
    -slope*(i-i0) via one row add (DVE/Pool alternating); the causal mask is
    a precomputed [128,128] additive -1e30 triangle on diagonal blocks.
  - P^T tiles feed the PV matmul as rhs with V as lhsT, accumulating O^T
    [d, i] directly in PSUM (no transposes anywhere). A ones-lhsT matmul
    accumulates the softmax denominators as a row, normalized via
    reciprocal + partition_broadcast.

Scheduling strategy (v2):
  - xt streams in 8 chunks per 512-slice ([P, 4o, 512]); the startup DMA
    order interleaves wq pieces with xt chunks (first loads split in half)
    so the PE starts ~3us in. wk/wv are host-packed to the SBUF layout so
    their DMA descriptors are 8KB runs (the [DM, HD] layout would give
    256B descriptors, which the DMA does at half throughput).
  - One PE "filler queue" per attention phase: attention for block ss runs
    with proj(ss+1) and outproj(ss-1) matmul steps pulled into the exp-
    latency bubbles of the jt pipeline (st(jt+depth) is emitted depth=2
    iterations ahead, 3 in the filler-poor last phase; otp/lps trail once
    exp(jt) lands). Q-projection filler runs as head PAIRS iterating
    o-major, so each xt chunk is consumed slower than its DMA delivers it
    (chunk-major sweeps outran the serial DMA engine and stalled). Each
    head's first score matmuls are emitted during the previous head's last
    iteration (attn_prologue chaining), and phase(ss+1)'s head-0 prologue
    is emitted under the last 8 filler steps of phase ss.
  - RMSNorm is split: chunk epilogues bank the sum-of-squares row and a
    raw bf16 copy (freeing PSUM); the ACT-table-switching Sqrt runs in one
    batched pocket per phase (2 LoadActFuncSet round trips per phase
    instead of ~10 -- Sqrt and Exp live in different ACT table sets).
  - Per-head ALiBi bias rows/columns are precomputed once (masters), so a
    head costs no setup.
  - Output stores: 4 PSUM->SBUF cast copies (alternating ACT/DVE; GPSIMD
    cannot read PSUM) into one [P,4,512] tile, then ONE merged DMA; the
    final m-slices use per-quad DMAs to shorten the kernel tail.
"""

import math
from collections import deque

import numpy as np
import ml_dtypes

import concourse.bass as bass
import concourse.bacc as bacc
import concourse.mybir as mybir
import concourse.tile as tile

F32 = mybir.dt.float32
BF16 = mybir.dt.bfloat16
AF = mybir.ActivationFunctionType
ALU = mybir.AluOpType

B, S, DM = 1, 2048, 4096
H, HKV, HD = 32, 8, 128
NC_CORES = 8
HL = H // NC_CORES          # 4 local q heads per core
EPS = 1e-6
NEG = -1.0e30
P = 128

NBF = ml_dtypes.bfloat16


def _alibi_slopes(n_heads: int) -> np.ndarray:
    start = 2 ** (-(2 ** (-(math.log2(n_heads) - 3))))
    return np.array([start * (start**i) for i in range(n_heads)], dtype=np.float32)


def build_module(s: int = S, repeat: int = 1, phases=('proj', 'attn', 'out')):
    """Build the per-core Bass module. `s` parameterized for small tests."""
    assert s % 512 == 0
    nss = s // 512            # 512-wide s slices / query blocks
    njt = s // P              # 128-wide key tiles
    ndm = DM // P             # 32 contraction tiles
    nxc = ndm // 4            # 8 xt chunks per slice (4 o's each)
    NJ = 4 * nss              # jcol master width
    RB = 4 * (nss - 1)        # ridx bias: ridx = jt - 4*ss + RB in [0, NJ)

    nc = bacc.Bacc(trn_type="TRN2")

    xt_d = nc.dram_tensor("xt", [DM, s], BF16, kind="ExternalInput")
    wq_d = nc.dram_tensor("wq", [DM, HL * HD], BF16, kind="ExternalInput")
    wk_d = nc.dram_tensor("wk", [P, (DM // P) * HD], BF16, kind="ExternalInput")
    wv_d = nc.dram_tensor("wv", [P, (DM // P) * HD], BF16, kind="ExternalInput")
    wo_d = nc.dram_tensor("wo", [HL * HD, DM], BF16, kind="ExternalInput")
    qnw_d = nc.dram_tensor("qnw", [HD, 1], F32, kind="ExternalInput")
    knw_d = nc.dram_tensor("knw", [HD, 1], F32, kind="ExternalInput")
    slp_d = nc.dram_tensor("slp", [P, HL], F32, kind="ExternalInput")
    nslp_d = nc.dram_tensor("nslp", [P, HL], F32, kind="ExternalInput")
    out_d = nc.dram_tensor("out", [s, DM], BF16, kind="ExternalOutput")

    with tile.TileContext(nc) as tc:
        with (
            tc.tile_pool(name="const", bufs=1) as const,
            tc.tile_pool(name="xtc", bufs=10) as xtc_pool,
            tc.tile_pool(name="big", bufs=1) as big,
            tc.tile_pool(name="sq", bufs=2) as sq_pool,
            tc.tile_pool(name="row1", bufs=3) as row1,
            tc.tile_pool(name="msr", bufs=6) as msr_pool,
            tc.tile_pool(name="raw", bufs=6) as raw_pool,
            tc.tile_pool(name="inv", bufs=2) as inv_pool,
            tc.tile_pool(name="tmp", bufs=4) as tmp_pool,
            tc.tile_pool(name="pt", bufs=4) as pt_pool,
            tc.tile_pool(name="fsb", bufs=2) as fsb_pool,
            tc.tile_pool(name="ps", bufs=8, space="PSUM") as ps,
        ):
            # ---------------- small constants (loads deferred into proj) --
            qnw_sb = const.tile([P, 1], F32)
            knw_sb = const.tile([P, 1], F32)
            slp_sb = const.tile([P, HL], F32)
            nslp_sb = const.tile([P, HL], F32)

            def load_small_consts():
                nc.sync.dma_start(qnw_sb, qnw_d[:, :])
                nc.sync.dma_start(knw_sb, knw_d[:, :])
                nc.sync.dma_start(slp_sb, slp_d[:, :])
                nc.sync.dma_start(nslp_sb, nslp_d[:, :])

            # weights tiles (loads interleaved below / deferred)
            wq_sb = const.tile([P, ndm, HL * HD], BF16)
            wq_r = wq_d[:, :].rearrange("(o p) m -> p o m", p=P)
            wk_sb = const.tile([P, ndm, HD], BF16)
            wv_sb = const.tile([P, ndm, HD], BF16)
            wo_sb = const.tile([P, HL, DM], BF16)
            wo_r = wo_d[:, :].rearrange("(o p) m -> p o m", p=P)

            ones_f32 = const.tile([P, 1], F32)
            nc.vector.memset(ones_f32, 1.0)
            ones_sb = const.tile([P, 1], mybir.dt.float32r)
            nc.scalar.copy(ones_sb, ones_f32)
            ones_bf = const.tile([P, 1], BF16)
            nc.vector.memset(ones_bf, 1.0)
            eps_sb = const.tile([P, 1], F32)
            nc.vector.memset(eps_sb, EPS)

            # per-head ALiBi masters:
            #   negrowM[p, h, f] = -slope_h * f           (query-col row add)
            #   jcolM[p, h, t]   = slope_h * (128*(t-RB) + p)  (exp bias; at
            #       t = jt - 4*ss + RB it equals slope*(128*jt + p - 512*ss))
            iota_row = const.tile([P, 512], F32)
            nc.gpsimd.iota(iota_row, pattern=[[1, 512]], base=0,
                           channel_multiplier=0,
                           allow_small_or_imprecise_dtypes=True)
            iota_j = const.tile([P, NJ], F32)
            nc.gpsimd.iota(iota_j, pattern=[[P, NJ]], base=-RB * P,
                           channel_multiplier=1,
                           allow_small_or_imprecise_dtypes=True)
            negrowM = const.tile([P, HL, 512], F32)
            jcolM = const.tile([P, HL, NJ], F32)

            def build_masters():
                for h in range(HL):
                    nc.gpsimd.tensor_tensor(
                        negrowM[:, h, :], iota_row,
                        nslp_sb[:, h:h + 1].to_broadcast([P, 512]), ALU.mult)
                    nc.gpsimd.tensor_tensor(
                        jcolM[:, h, :], iota_j,
                        slp_sb[:, h:h + 1].to_broadcast([P, NJ]), ALU.mult)

            # maskneg[p, f] = 0 where p <= f else -1e30  (additive causal
            # mask for diagonal 128x128 blocks of S^T)
            maskneg = const.tile([P, P], F32)
            nc.gpsimd.memset(maskneg, 0.0)
            nc.gpsimd.affine_select(
                out=maskneg, in_=maskneg,
                compare_op=ALU.is_ge, fill=NEG,
                base=0, pattern=[[1, P]], channel_multiplier=-1,
            )

            # ---------------- persistent activations ----------------
            qt_sb = big.tile([P, HL, 2, 512], BF16)  # Q^T ring [d, h, ss%2, i]
            kt_sb = big.tile([P, s], BF16)           # K^T      [d, s]
            v_sb = big.tile([P, njt, HD], BF16)      # V        [s, d]
            ot_sb = big.tile([P, HL, 2, 512], BF16)  # O^T ring [d, h, ib%2, i]

            xt_r = xt_d[:, :].rearrange("(o p) t -> p o t", p=P)

            # xt chunk bookkeeping: chunks[(ss, c)] = sbuf tile [P, 4, 512]
            xt_chunks = {}

            def load_xt_chunk(ss, c):
                t = xtc_pool.tile([P, 4, 512], BF16, tag="xtc",
                                  name=f"xt{ss}_{c}")
                nc.sync.dma_start(
                    t, xt_r[:, 4 * c:4 * c + 4, ss * 512:ss * 512 + 512])
                xt_chunks[(ss, c)] = t

            def xt_o(ss, o):
                return xt_chunks[(ss, o // 4)][:, o % 4, :]

            # ---------------- rmsnorm chain (non-PE parts) ----------------
            def rms_chain_pre(src):
                """Square the PSUM tile; returns sq tile (ACT)."""
                sqt = sq_pool.tile([P, 512], mybir.dt.float32r,
                                   tag="sq", name="sqt")
                nc.scalar.activation(sqt, src, AF.Square)
                return sqt

            def rms_chain_post(src, psq, w_sb, dst):
                """After PE computed psq = ones^T @ sqt: finish the norm."""
                rms = row1.tile([1, 512], F32, tag="row1", name="rms")
                nc.scalar.activation(rms, psq, AF.Sqrt,
                                     bias=eps_sb[:1, :], scale=1.0 / HD)
                rec = row1.tile([1, 512], F32, tag="row1", name="rec")
                nc.vector.reciprocal(rec, rms)
                invb = inv_pool.tile([P, 512], F32, tag="inv", name="invb")
                nc.gpsimd.partition_broadcast(invb, rec)
                nc.vector.scalar_tensor_tensor(
                    out=dst, in0=src, scalar=w_sb, in1=invb,
                    op0=ALU.mult, op1=ALU.mult)

            # ---------------- initial projection (slice 0) ----------------
            def proj_initial():
                """proj(0): DMA-paced. Q pass (o-major over 4 heads), K pass,
                V pass; weight quarters interleave with xt chunks."""
                # Q pass
                pq = [ps.tile([P, 512], F32, tag="ps", name=f"pq{c}")
                      for c in range(HL)]
                for o in range(ndm):
                    if o == 0:          # finest first loads: PE starts ~3us
                        nc.sync.dma_start(wq_sb[:, 0:2, :], wq_r[:, 0:2, :])
                        t0 = xtc_pool.tile([P, 4, 512], BF16, tag="xtc",
                                           name="xt0_0")
                        nc.sync.dma_start(t0[:, 0:2, :], xt_r[:, 0:2, 0:512])
                        nc.sync.dma_start(wq_sb[:, 2:4, :], wq_r[:, 2:4, :])
                        nc.sync.dma_start(t0[:, 2:4, :], xt_r[:, 2:4, 0:512])
                        xt_chunks[(0, 0)] = t0
                    elif o == 2:
                        nc.sync.dma_start(wq_sb[:, 4:8, :], wq_r[:, 4:8, :])
                    elif o % 8 == 0:    # wq quarter q = o//8
                        q = o // 8
                        nc.sync.dma_start(
                            wq_sb[:, 8 * q:8 * q + 8, :],
                            wq_r[:, 8 * q:8 * q + 8, :])
                    if o % 4 == 0 and o > 0:
                        load_xt_chunk(0, o // 4)
                    if o == 1:
                        load_small_consts()
                        build_masters()
                    if o == ndm - 8:
                        nc.sync.dma_start(
                            wk_sb, wk_d[:, :].rearrange(
                                "p (o m) -> p o m", o=ndm))
                        nc.sync.dma_start(
                            wv_sb, wv_d[:, :].rearrange(
                                "p (o m) -> p o m", o=ndm))
                    for c in range(HL):
                        nc.tensor.matmul(
                            pq[c], wq_sb[:, o, c * HD:(c + 1) * HD],
                            xt_o(0, o), start=(o == 0), stop=(o == ndm - 1))
                # K pass (Q rmsnorm chains interleave into it)
                pk = ps.tile([P, 512], F32, tag="ps", name="pk")
                sqs = {}
                for o in range(ndm):
                    nc.tensor.matmul(pk, wk_sb[:, o, :], xt_o(0, o),
                                     start=(o == 0), stop=(o == ndm - 1))
                    if o % 8 == 1 and o // 8 < HL:
                        c = o // 8
                        sqs[c] = rms_chain_pre(pq[c])
                    elif o % 8 == 5 and o // 8 < HL:
                        c = o // 8
                        psq = ps.tile([1, 512], F32, tag="ps", name="psq")
                        nc.tensor.matmul(psq, ones_sb, sqs[c],
                                         start=True, stop=True)
                        rms_chain_post(pq[c], psq, qnw_sb,
                                       qt_sb[:, c, 0, :])
                # V chunks interleave around the K rms chain and the
                # attn(0) prologue: V c=0 covers ACT finishing the Q-chain
                # sqrts before the K sum-of-squares matmul; V c=1,2 cover
                # the K chain -> kt latency before st0; V c=3 covers the
                # first exp warm-up.
                def v_chunk(c):
                    pv = ps.tile([P, HD], F32, tag="ps", name="pv")
                    for o in range(ndm):
                        nc.tensor.matmul(
                            pv, xt_chunks[(0, o // 4)][:, o % 4,
                                                       c * P:(c + 1) * P],
                            wv_sb[:, o, :],
                            start=(o == 0), stop=(o == ndm - 1))
                    nc.scalar.copy(v_sb[:, c, :], pv)

                v_chunk(0)
                sqk = rms_chain_pre(pk)
                psqk = ps.tile([1, 512], F32, tag="ps", name="psqk")
                nc.tensor.matmul(psqk, ones_sb, sqk, start=True, stop=True)
                rms_chain_post(pk, psqk, knw_sb, kt_sb[:, 0:512])
                v_chunk(1)
                v_chunk(2)
                ctx0 = attn_prologue(0, 0) if 'attn' in phases else None
                v_chunk(3)
                return ctx0

            # ---------------- filler step generators ----------------
            # Each filler step is a closure emitting ~1-2 PE matmuls (plus
            # trailing non-PE ops). Steps are pulled into attention bubbles.

            def rms_defer_a(src, pending, w_sb, dst):
                """Chunk epilogue: sum-of-squares row + raw copy to SBUF so
                the PSUM accumulator frees now; the ACT-table-switching sqrt
                runs later in one batched pocket per phase."""
                sqt = rms_chain_pre(src)
                psq = ps.tile([1, 512], F32, tag="ps", name="psq")
                nc.tensor.matmul(psq, ones_sb, sqt, start=True, stop=True)
                msrow = msr_pool.tile([1, 512], F32, tag="msr", name="msr")
                nc.scalar.copy(msrow, psq)
                raw = raw_pool.tile([P, 512], BF16, tag="raw", name="raw")
                cast_copy(raw, src)
                pending.append((msrow, raw, w_sb, dst))

            def rms_defer_b(msrow, raw, w_sb, dst):
                rms = row1.tile([1, 512], F32, tag="row1", name="rms")
                nc.scalar.activation(rms, msrow, AF.Sqrt,
                                     bias=eps_sb[:1, :], scale=1.0 / HD)
                rec = row1.tile([1, 512], F32, tag="row1", name="rec")
                nc.vector.reciprocal(rec, rms)
                invb = inv_pool.tile([P, 512], F32, tag="inv", name="invb")
                nc.gpsimd.partition_broadcast(invb, rec)
                nc.vector.scalar_tensor_tensor(
                    out=dst, in0=raw, scalar=w_sb, in1=invb,
                    op0=ALU.mult, op1=ALU.mult)

            def proj_q_pair_steps(ss, c0, c1, pending):
                """Q projection of heads c0,c1 for slice ss, o-major so a
                given xt chunk is consumed at half the single-head rate
                (stays behind the chunk DMA arrival): 64 steps."""
                state = {}

                def step(o, c):
                    def f():
                        if o == 0:
                            state[c] = ps.tile([P, 512], F32, tag="ps",
                                               name=f"pq{ss}_{c}")
                        nc.tensor.matmul(
                            state[c], wq_sb[:, o, c * HD:(c + 1) * HD],
                            xt_o(ss, o), start=(o == 0), stop=(o == ndm - 1))
                        if o == ndm - 1:
                            rms_defer_a(state[c], pending, qnw_sb,
                                        qt_sb[:, c, ss % 2, :])
                    return f
                steps = []
                for o in range(ndm):
                    steps.append(step(o, c0))
                    steps.append(step(o, c1))
                return steps

            def proj_k_steps(ss, pending):
                state = {}

                def step(o):
                    def f():
                        if o == 0:
                            state['pk'] = ps.tile([P, 512], F32, tag="ps",
                                                  name=f"pk{ss}")
                        nc.tensor.matmul(
                            state['pk'], wk_sb[:, o, :], xt_o(ss, o),
                            start=(o == 0), stop=(o == ndm - 1))
                        if o == ndm - 1:
                            rms_defer_a(state['pk'], pending, knw_sb,
                                        kt_sb[:, ss * 512:ss * 512 + 512])
                    return f
                return [step(o) for o in range(ndm)]

            def proj_v_steps(ss, cs=(0, 1, 2, 3)):
                """V projection: pos-chunks x 4 steps of 8 matmuls."""
                state = {}
                steps = []

                def step(c, g):
                    def f():
                        if g == 0:
                            state[c] = ps.tile([P, HD], F32, tag="ps",
                                               name=f"pv{ss}_{c}")
                        for o in range(8 * g, 8 * g + 8):
                            nc.tensor.matmul(
                                state[c],
                                xt_chunks[(ss, o // 4)][:, o % 4,
                                                        c * P:(c + 1) * P],
                                wv_sb[:, o, :],
                                start=(o == 0), stop=(o == ndm - 1))
                        if g == 3:
                            nc.scalar.copy(v_sb[:, 4 * ss + c, :], state[c])
                    return f
                for c in cs:
                    for g in range(4):
                        steps.append(step(c, g))
                return steps

            _copy_rr = [0]
            _copy_mode = [2]    # 2 = alternate; 3 = 2xACT:1xDVE (DVE-heavy
                                # attention phases)

            def cast_copy(dst, src):
                """PSUM f32 -> SBUF bf16 cast copy over the two PSUM-capable
                non-PE engines (GPSIMD cannot read PSUM)."""
                r = _copy_rr[0] = (_copy_rr[0] + 1) % _copy_mode[0]
                if r != 0:
                    nc.scalar.copy(dst, src)
                else:
                    nc.vector.tensor_copy(dst, src)

            def outproj_steps(ib, mi, split_dma=False):
                """Output projection block ib, m-slice mi: 8 steps of 2
                matmuls; one merged DMA after the 4 quads (or 2 half DMAs
                when split_dma, to shorten the kernel tail)."""
                m0 = mi * 512
                state = {}
                steps = []

                def step(st_i, half):
                    def f():
                        if half == 0:
                            state['fps'] = ps.tile([P, 512], F32, tag="ps",
                                                   name=f"fps{ib}_{mi}")
                            if st_i == 0:
                                state['fsb'] = fsb_pool.tile(
                                    [P, 4, 512], BF16, tag="fsb", name="fsb")
                        for c in (0, 1) if half == 0 else (2, 3):
                            nc.tensor.matmul(
                                state['fps'],
                                ot_sb[:, c, ib % 2, st_i * P:(st_i + 1) * P],
                                wo_sb[:, c, m0:m0 + 512],
                                start=(c == 0), stop=(c == HL - 1))
                        if half == 1:
                            cast_copy(state['fsb'][:, st_i, :], state['fps'])
                            if split_dma:   # per-quad DMA: shortest tail
                                dst = out_d[ib * 512 + st_i * P:
                                            ib * 512 + (st_i + 1) * P,
                                            m0:m0 + 512]
                                nc.sync.dma_start(dst, state['fsb'][:, st_i, :])
                            elif st_i == 3:
                                dst = out_d[ib * 512:ib * 512 + 512,
                                            m0:m0 + 512]
                                nc.sync.dma_start(
                                    dst.rearrange("(st p) m -> p st m", p=P),
                                    state['fsb'])
                    return f
                for st_i in range(4):
                    steps.append(step(st_i, 0))
                    steps.append(step(st_i, 1))
                return steps

            # ---------------- attention ----------------
            def attn_prologue(ss, h):
                """Allocate the head's PSUM accumulators and emit its first
                two score matmuls. Called from the PREVIOUS head's last jt
                iteration so the exp chain of head h warms while the PE is
                still busy, killing the head-boundary bubble."""
                i0 = ss * 512
                jtend = 4 * (ss + 1)
                # last phase has little filler: run a deeper score pipeline
                # (PSUM has room there -- no proj-chunk accumulators live)
                depth = 2 if ss + 1 < nss else 3
                ctx = {'otp': ps.tile([P, 512], F32, tag="ps", name="otp"),
                       'lps': ps.tile([1, 512], F32, tag="ps", name="lps"),
                       'sts': {}, 'depth': depth}

                def emit_st(jt):
                    j0 = jt * P
                    c0 = max(0, j0 - i0)
                    stt = ps.tile([P, 512], F32, tag="ps", name="st")
                    nc.tensor.matmul(
                        stt[:, c0:], kt_sb[:, j0:j0 + P],
                        qt_sb[:, h, ss % 2, c0:], start=True, stop=True)
                    ctx['sts'][jt] = (stt, c0)

                ctx['emit_st'] = emit_st
                for jt in range(min(depth, jtend)):
                    emit_st(jt)
                return ctx

            def attn_head(ss, h, pull, ctx, next_prologue=None):
                """Body of one head's attention; `ctx` from attn_prologue.
                `next_prologue` (if set) is invoked during the last jt
                iteration and its result returned."""
                i0 = ss * 512
                jtend = 4 * (ss + 1)
                jlast = jtend - 1
                otp, lps, sts = ctx['otp'], ctx['lps'], ctx['sts']
                emit_st = ctx['emit_st']
                depth = ctx['depth']
                nctx = None
                for jt in range(jtend):
                    stt, c0 = sts.pop(jt)
                    tmp = tmp_pool.tile([P, 512], F32, tag="tmp", name="tmp")
                    nc.vector.tensor_tensor(
                        tmp[:, c0:], stt[:, c0:], negrowM[:, h, c0:], ALU.add)
                    if jt * P >= i0:  # diagonal block: additive causal mask
                        nc.gpsimd.tensor_tensor(
                            tmp[:, c0:c0 + P], tmp[:, c0:c0 + P],
                            maskneg, ALU.add)
                    pt = pt_pool.tile([P, 512], BF16, tag="pt", name="pt")
                    ridx = jt - 4 * ss + RB
                    nc.scalar.activation(
                        pt[:, c0:], tmp[:, c0:], AF.Exp,
                        bias=jcolM[:, h, ridx:ridx + 1], scale=1.0)
                    if jt + depth < jtend:
                        emit_st(jt + depth)
                    pull(3 if jt < jlast else 1)
                    if jt == jlast and next_prologue is not None:
                        nctx = next_prologue()
                    nc.tensor.matmul(
                        otp[:, c0:], v_sb[:, jt, :], pt[:, c0:],
                        start=(jt == 0), stop=(jt == jlast))
                    nc.tensor.matmul(
                        lps[:, c0:], ones_bf, pt[:, c0:],
                        start=(jt == 0), stop=(jt == jlast))
                # normalize
                lrow = row1.tile([1, 512], F32, tag="row1", name="lrow")
                nc.scalar.copy(lrow, lps)
                linv = row1.tile([1, 512], F32, tag="row1", name="linv")
                nc.vector.reciprocal(linv, lrow)
                linvb = inv_pool.tile([P, 512], F32, tag="inv", name="linvb")
                nc.gpsimd.partition_broadcast(linvb, linv)
                nc.vector.tensor_tensor(
                    ot_sb[:, h, ss % 2, :], otp, linvb, ALU.mult)
                return nctx

            # ---------------- phase schedule ----------------
            def phase(ss, ctx):
                """attn(ss) + filler proj(ss+1) + outproj(ss-1). `ctx` is
                head 0's prologue (emitted by the previous phase); returns
                the next phase's head-0 prologue ctx."""
                _copy_mode[0] = 2
                # prefetch DMAs for the next slice; wo afterwards during
                # phase 0 (xt chunks gate proj(1) filler NOW, wo is not
                # read until outproj(0) in phase 1)
                if ss + 1 < nss:
                    for c in range(nxc):
                        load_xt_chunk(ss + 1, c)
                if ss == 0:
                    for q in range(4):
                        nc.sync.dma_start(
                            wo_sb[:, :, q * (DM // 4):(q + 1) * (DM // 4)],
                            wo_r[:, :, q * (DM // 4):(q + 1) * (DM // 4)])

                fill = deque()
                pending = []
                if ss + 1 < nss and 'proj' in phases:
                    qs = [proj_q_pair_steps(ss + 1, 0, 1, pending),
                          proj_q_pair_steps(ss + 1, 2, 3, pending)]
                    ks = proj_k_steps(ss + 1, pending)
                    vs = proj_v_steps(ss + 1)
                else:
                    qs, ks, vs = [], None, None
                if ss >= 1 and 'out' in phases:
                    os_ = [outproj_steps(ss - 1, mi) for mi in range(8)]
                else:
                    os_ = []
                # interleave: outproj mi-pairs between proj chunks (paced by
                # xt arrival); K early so the rsqrt pocket (right after q3)
                # finishes well before the next phase needs qt/kt, with the
                # remaining outproj + V steps as padding behind it.
                def chain_step(i):
                    def f():
                        rms_defer_b(*pending[i])
                    return f
                order = []
                proj_units = ([qs[0], ks, qs[1],
                               [chain_step(i) for i in range(HL + 1)], vs]
                              if ks is not None else [])
                ou = list(os_)
                # zip: outproj pair, proj unit, outproj pair, proj unit, ...
                pi = 0
                for i in range(max(len(ou), len(proj_units))):
                    if i < len(ou):
                        order.append(ou[i])
                    if pi < len(proj_units):
                        order.append(proj_units[pi])
                        pi += 1
                for lst in order:
                    fill.extend(lst)

                def pull(n):
                    for _ in range(n):
                        if fill:
                            fill.popleft()()

                total = len(fill)
                if ctx is None and 'attn' in phases:
                    ctx = attn_prologue(ss, 0)
                for h in range(HL):
                    if 'attn' in phases:
                        nxt = None
                        if h + 1 < HL:
                            def nxt(hh=h + 1):
                                return attn_prologue(ss, hh)
                        ctx = attn_head(ss, h, pull, ctx, nxt)
                    # drain this head's share of the filler, holding back a
                    # few steps to cover the next phase's exp warm-up
                    hold = 14 if h == HL - 1 else 0
                    target = (total * (h + 1)) // HL - hold
                    while len(fill) > max(0, total - target):
                        fill.popleft()()
                nctx = None
                if ss + 1 < nss and 'attn' in phases:
                    nctx = attn_prologue(ss + 1, 0)
                while fill:
                    fill.popleft()()
                return nctx

            def outproj_final(ib):
                _copy_mode[0] = 2
                for mi in range(8):
                    for st in outproj_steps(ib, mi, split_dma=(mi >= 6)):
                        st()

            for _rep in range(repeat):
                ctx = None
                if 'proj' in phases:
                    ctx = proj_initial()
                for ss in range(nss):
                    ctx = phase(ss, ctx)
                if 'out' in phases and 'attn' in phases:
                    outproj_final(nss - 1)

    nc.finalize()
    return nc


def _prep_kv(w):
    """[HD, DM] weight -> [P, ndm*HD] laid out as [p][o][m] (8KB runs)."""
    ndm = DM // P
    wt = np.ascontiguousarray(w.T)                       # [DM, HD]
    return np.ascontiguousarray(
        wt.reshape(ndm, P, HD).transpose(1, 0, 2).reshape(P, ndm * HD)
    ).astype(NBF)


def shard_inputs(x, Wq, Wk, Wv, Wo, q_norm_w, k_norm_w, s=S):
    """Host-side shard + layout prep. Returns per-core input maps."""
    slopes = _alibi_slopes(H)
    xt = np.ascontiguousarray(x.reshape(s, DM).T).astype(NBF)
    qnw = (np.asarray(q_norm_w, np.float32) / math.sqrt(HD)).reshape(HD, 1)
    knw = np.asarray(k_norm_w, np.float32).reshape(HD, 1).copy()
    in_maps = []
    for g in range(NC_CORES):
        qs = g * HL * HD
        sl = slopes[g * HL:(g + 1) * HL]
        in_maps.append({
            "xt": xt,
            "wq": np.ascontiguousarray(Wq[qs:qs + HL * HD, :].T).astype(NBF),
            "wk": _prep_kv(Wk[g * HD:(g + 1) * HD, :]),
            "wv": _prep_kv(Wv[g * HD:(g + 1) * HD, :]),
            "wo": np.ascontiguousarray(Wo[:, qs:qs + HL * HD].T).astype(NBF),
            "qnw": qnw,
            "knw": knw,
            "slp": np.ascontiguousarray(
                np.broadcast_to(sl, (P, HL))).astype(np.float32),
            "nslp": np.ascontiguousarray(
                np.broadcast_to(-sl, (P, HL))).astype(np.float32),
        })
    return in_maps


_MODULE_CACHE = {}
LAST_RESULT = None


def _get_module(s=S):
    if s not in _MODULE_CACHE:
        _MODULE_CACHE[s] = build_module(s)
    return _MODULE_CACHE[s]


def kernel(x, Wq, Wk, Wv, Wo, q_norm_w, k_norm_w, **run_kwargs):
    global LAST_RESULT
    from concourse.bass_utils import run_bass_kernel_spmd

    x = np.asarray(x)
    in_maps = shard_inputs(np.asarray(x), np.asarray(Wq), np.asarray(Wk),
                           np.asarray(Wv), np.asarray(Wo),
                           np.asarray(q_norm_w), np.asarray(k_norm_w))
    nc = _get_module(S)
    res = run_bass_kernel_spmd(nc, in_maps, core_ids=list(range(NC_CORES)),
                               **run_kwargs)
    LAST_RESULT = res
    acc = np.zeros((S, DM), np.float32)
    for r in res.results:
        acc += r["out"].astype(np.float32)
    return acc.reshape(B, S, DM)


# revision 74
# speedup vs baseline: 1.2075x; 1.0002x over previous
"""Trainium2 Bass kernel: GQA causal self-attention with ALiBi + QK-RMSNorm.

Model: B=1, S=2048, DM=4096, H=32 q-heads, HKV=8 kv-heads, HD=128.
Sharding: tensor-parallel over heads across 8 cores. Core g computes
q-heads 4g..4g+3 with kv-head g, and a row-parallel partial of the output
projection; the host sums the 8 partials (the unshard for row-parallel Wo).

Layout strategy (per core):
  - x is passed transposed (XT [DM,S]) so every projection matmul contracts
    over DM on the partition axis with no on-device transposes.
  - Q,K are produced transposed ([d, s]); V natural ([s, d]).
  - RMSNorm over d (= partition axis) uses a ones-vector matmul for the
    per-position sum of squares, then a GPSIMD partition_broadcast of 1/rms.
  - Scores are computed transposed: S^T[j,i] (j=key pos on partitions,
    i=query pos on free axis). With q scaled by 1/sqrt(HD) and RMSNormed,
    |s| <= sqrt(128) and the ALiBi bias slope*(j-i) <= 0 after causal
    masking, so exp() cannot overflow and NO row-max pass is needed.
    exp bias: +slope*(j-i0) enters via the ACT per-partition bias operand,
    -slope*(i-i0) via one row add (DVE/Pool alternating); the causal mask is
    a precomputed [128,128] additive -1e30 triangle on diagonal blocks.
  - P^T tiles feed the PV matmul as rhs with V as lhsT, accumulating O^T
    [d, i] directly in PSUM (no transposes anywhere). A ones-lhsT matmul
    accumulates the softmax denominators as a row, normalized via
    reciprocal + partition_broadcast.

Scheduling strategy (v2):
  - xt streams in 8 chunks per 512-slice ([P, 4o, 512]); the startup DMA
    order interleaves wq pieces with xt chunks (first loads split in half)
    so the PE starts ~3us in. wk/wv are host-packed to the SBUF layout so
    their DMA descriptors are 8KB runs (the [DM, HD] layout would give
    256B descriptors, which the DMA does at half throughput).
  - One PE "filler queue" per attention phase: attention for block ss runs
    with proj(ss+1) and outproj(ss-1) matmul steps pulled into the exp-
    latency bubbles of the jt pipeline (st(jt+depth) is emitted depth=2
    iterations ahead, 3 in the filler-poor last phase; otp/lps trail once
    exp(jt) lands). Q-projection filler runs as head PAIRS iterating
    o-major, so each xt chunk is consumed slower than its DMA delivers it
    (chunk-major sweeps outran the serial DMA engine and stalled). Each
    head's first score matmuls are emitted during the previous head's last
    iteration (attn_prologue chaining), and phase(ss+1)'s head-0 prologue
    is emitted under the last 8 filler steps of phase ss.
  - RMSNorm is split: chunk epilogues bank the sum-of-squares row and a
    raw bf16 copy (freeing PSUM); the ACT-table-switching Sqrt runs in one
    batched pocket per phase (2 LoadActFuncSet round trips per phase
    instead of ~10 -- Sqrt and Exp live in different ACT table sets).
  - Per-head ALiBi bias rows/columns are precomputed once (masters), so a
    head costs no setup.
  - Output stores: 4 PSUM->SBUF cast copies (alternating ACT/DVE; GPSIMD
    cannot read PSUM) into one [P,4,512] tile, then ONE merged DMA; the
    final m-slices use per-quad DMAs to shorten the kernel tail.
"""

import math
from collections import deque

import numpy as np
import ml_dtypes

import concourse.bass as bass
import concourse.bacc as bacc
import concourse.mybir as mybir
import concourse.tile as tile

F32 = mybir.dt.float32
BF16 = mybir.dt.bfloat16
AF = mybir.ActivationFunctionType
ALU = mybir.AluOpType

B, S, DM = 1, 2048, 4096
H, HKV, HD = 32, 8, 128
NC_CORES = 8
HL = H // NC_CORES          # 4 local q heads per core
EPS = 1e-6
NEG = -1.0e30
P = 128

NBF = ml_dtypes.bfloat16


def _alibi_slopes(n_heads: int) -> np.ndarray:
    start = 2 ** (-(2 ** (-(math.log2(n_heads) - 3))))
    return np.array([start * (start**i) for i in range(n_heads)], dtype=np.float32)


def build_module(s: int = S, repeat: int = 1, phases=('proj', 'attn', 'out')):
    """Build the per-core Bass module. `s` parameterized for small tests."""
    assert s % 512 == 0
    nss = s // 512            # 512-wide s slices / query blocks
    njt = s // P              # 128-wide key tiles
    ndm = DM // P             # 32 contraction tiles
    nxc = ndm // 4            # 8 xt chunks per slice (4 o's each)
    NJ = 4 * nss              # jcol master width
    RB = 4 * (nss - 1)        # ridx bias: ridx = jt - 4*ss + RB in [0, NJ)

    nc = bacc.Bacc(trn_type="TRN2")

    xt_d = nc.dram_tensor("xt", [DM, s], BF16, kind="ExternalInput")
    wq_d = nc.dram_tensor("wq", [DM, HL * HD], BF16, kind="ExternalInput")
    wk_d = nc.dram_tensor("wk", [P, (DM // P) * HD], BF16, kind="ExternalInput")
    wv_d = nc.dram_tensor("wv", [P, (DM // P) * HD], BF16, kind="ExternalInput")
    wo_d = nc.dram_tensor("wo", [HL * HD, DM], BF16, kind="ExternalInput")
    qnw_d = nc.dram_tensor("qnw", [HD, 1], F32, kind="ExternalInput")
    knw_d = nc.dram_tensor("knw", [HD, 1], F32, kind="ExternalInput")
    slp_d = nc.dram_tensor("slp", [P, HL], F32, kind="ExternalInput")
    nslp_d = nc.dram_tensor("nslp", [P, HL], F32, kind="ExternalInput")
    out_d = nc.dram_tensor("out", [s, DM], BF16, kind="ExternalOutput")

    with tile.TileContext(nc) as tc:
        with (
            tc.tile_pool(name="const", bufs=1) as const,
            tc.tile_pool(name="xtc", bufs=10) as xtc_pool,
            tc.tile_pool(name="big", bufs=1) as big,
            tc.tile_pool(name="sq", bufs=2) as sq_pool,
            tc.tile_pool(name="row1", bufs=3) as row1,
            tc.tile_pool(name="msr", bufs=6) as msr_pool,
            tc.tile_pool(name="raw", bufs=6) as raw_pool,
            tc.tile_pool(name="inv", bufs=2) as inv_pool,
            tc.tile_pool(name="tmp", bufs=4) as tmp_pool,
            tc.tile_pool(name="pt", bufs=4) as pt_pool,
            tc.tile_pool(name="fsb", bufs=2) as fsb_pool,
            tc.tile_pool(name="ps", bufs=8, space="PSUM") as ps,
        ):
            # ---------------- small constants (loads deferred into proj) --
            qnw_sb = const.tile([P, 1], F32)
            knw_sb = const.tile([P, 1], F32)
            slp_sb = const.tile([P, HL], F32)
            nslp_sb = const.tile([P, HL], F32)

            def load_small_consts():
                nc.sync.dma_start(qnw_sb, qnw_d[:, :])
                nc.sync.dma_start(knw_sb, knw_d[:, :])
                nc.sync.dma_start(slp_sb, slp_d[:, :])
                nc.sync.dma_start(nslp_sb, nslp_d[:, :])

            # weights tiles (loads interleaved below / deferred)
            wq_sb = const.tile([P, ndm, HL * HD], BF16)
            wq_r = wq_d[:, :].rearrange("(o p) m -> p o m", p=P)
            wk_sb = const.tile([P, ndm, HD], BF16)
            wv_sb = const.tile([P, ndm, HD], BF16)
            wo_sb = const.tile([P, HL, DM], BF16)
            wo_r = wo_d[:, :].rearrange("(o p) m -> p o m", p=P)

            ones_f32 = const.tile([P, 1], F32)
            nc.vector.memset(ones_f32, 1.0)
            ones_sb = const.tile([P, 1], mybir.dt.float32r)
            nc.scalar.copy(ones_sb, ones_f32)
            ones_bf = const.tile([P, 1], BF16)
            nc.vector.memset(ones_bf, 1.0)
            eps_sb = const.tile([P, 1], F32)
            nc.vector.memset(eps_sb, EPS)

            # per-head ALiBi masters:
            #   negrowM[p, h, f] = -slope_h * f           (query-col row add)
            #   jcolM[p, h, t]   = slope_h * (128*(t-RB) + p)  (exp bias; at
            #       t = jt - 4*ss + RB it equals slope*(128*jt + p - 512*ss))
            iota_row = const.tile([P, 512], F32)
            nc.gpsimd.iota(iota_row, pattern=[[1, 512]], base=0,
                           channel_multiplier=0,
                           allow_small_or_imprecise_dtypes=True)
            iota_j = const.tile([P, NJ], F32)
            nc.gpsimd.iota(iota_j, pattern=[[P, NJ]], base=-RB * P,
                           channel_multiplier=1,
                           allow_small_or_imprecise_dtypes=True)
            negrowM = const.tile([P, HL, 512], F32)
            jcolM = const.tile([P, HL, NJ], F32)

            def build_masters():
                for h in range(HL):
                    nc.gpsimd.tensor_tensor(
                        negrowM[:, h, :], iota_row,
                        nslp_sb[:, h:h + 1].to_broadcast([P, 512]), ALU.mult)
                    nc.gpsimd.tensor_tensor(
                        jcolM[:, h, :], iota_j,
                        slp_sb[:, h:h + 1].to_broadcast([P, NJ]), ALU.mult)

            # maskneg[p, f] = 0 where p <= f else -1e30  (additive causal
            # mask for diagonal 128x128 blocks of S^T)
            maskneg = const.tile([P, P], F32)
            nc.gpsimd.memset(maskneg, 0.0)
            nc.gpsimd.affine_select(
                out=maskneg, in_=maskneg,
                compare_op=ALU.is_ge, fill=NEG,
                base=0, pattern=[[1, P]], channel_multiplier=-1,
            )

            # ---------------- persistent activations ----------------
            qt_sb = big.tile([P, HL, 2, 512], BF16)  # Q^T ring [d, h, ss%2, i]
            kt_sb = big.tile([P, s], BF16)           # K^T      [d, s]
            v_sb = big.tile([P, njt, HD], BF16)      # V        [s, d]
            ot_sb = big.tile([P, HL, 2, 512], BF16)  # O^T ring [d, h, ib%2, i]

            xt_r = xt_d[:, :].rearrange("(o p) t -> p o t", p=P)

            # xt chunk bookkeeping: chunks[(ss, c)] = sbuf tile [P, 4, 512]
            xt_chunks = {}

            def load_xt_chunk(ss, c):
                t = xtc_pool.tile([P, 4, 512], BF16, tag="xtc",
                                  name=f"xt{ss}_{c}")
                nc.sync.dma_start(
                    t, xt_r[:, 4 * c:4 * c + 4, ss * 512:ss * 512 + 512])
                xt_chunks[(ss, c)] = t

            def xt_o(ss, o):
                return xt_chunks[(ss, o // 4)][:, o % 4, :]

            # ---------------- rmsnorm chain (non-PE parts) ----------------
            def rms_chain_pre(src):
                """Square the PSUM tile; returns sq tile (ACT)."""
                sqt = sq_pool.tile([P, 512], mybir.dt.float32r,
                                   tag="sq", name="sqt")
                nc.scalar.activation(sqt, src, AF.Square)
                return sqt

            def rms_chain_post(src, psq, w_sb, dst):
                """After PE computed psq = ones^T @ sqt: finish the norm."""
                rms = row1.tile([1, 512], F32, tag="row1", name="rms")
                nc.scalar.activation(rms, psq, AF.Sqrt,
                                     bias=eps_sb[:1, :], scale=1.0 / HD)
                rec = row1.tile([1, 512], F32, tag="row1", name="rec")
                nc.vector.reciprocal(rec, rms)
                invb = inv_pool.tile([P, 512], F32, tag="inv", name="invb")
                nc.gpsimd.partition_broadcast(invb, rec)
                nc.vector.scalar_tensor_tensor(
                    out=dst, in0=src, scalar=w_sb, in1=invb,
                    op0=ALU.mult, op1=ALU.mult)

            # ---------------- initial projection (slice 0) ----------------
            def proj_initial():
                """proj(0): DMA-paced. Q pass (o-major over 4 heads), K pass,
                V pass; weight quarters interleave with xt chunks."""
                # Q pass
                pq = [ps.tile([P, 512], F32, tag="ps", name=f"pq{c}")
                      for c in range(HL)]
                for o in range(ndm):
                    if o == 0:          # finest first loads: PE starts ~3us
                        nc.sync.dma_start(wq_sb[:, 0:2, :], wq_r[:, 0:2, :])
                        t0 = xtc_pool.tile([P, 4, 512], BF16, tag="xtc",
                                           name="xt0_0")
                        nc.sync.dma_start(t0[:, 0:2, :], xt_r[:, 0:2, 0:512])
                        nc.sync.dma_start(wq_sb[:, 2:4, :], wq_r[:, 2:4, :])
                        nc.sync.dma_start(t0[:, 2:4, :], xt_r[:, 2:4, 0:512])
                        xt_chunks[(0, 0)] = t0
                    elif o == 2:
                        nc.sync.dma_start(wq_sb[:, 4:8, :], wq_r[:, 4:8, :])
                    elif o % 8 == 0:    # wq quarter q = o//8
                        q = o // 8
                        nc.sync.dma_start(
                            wq_sb[:, 8 * q:8 * q + 8, :],
                            wq_r[:, 8 * q:8 * q + 8, :])
                    if o % 4 == 0 and o > 0:
                        load_xt_chunk(0, o // 4)
                    if o == 1:
                        load_small_consts()
                        build_masters()
                    if o == ndm - 8:
                        nc.sync.dma_start(
                            wk_sb, wk_d[:, :].rearrange(
                                "p (o m) -> p o m", o=ndm))
                        nc.sync.dma_start(
                            wv_sb, wv_d[:, :].rearrange(
                                "p (o m) -> p o m", o=ndm))
                    for c in range(HL):
                        nc.tensor.matmul(
                            pq[c], wq_sb[:, o, c * HD:(c + 1) * HD],
                            xt_o(0, o), start=(o == 0), stop=(o == ndm - 1))
                # K pass (Q rmsnorm chains interleave into it)
                pk = ps.tile([P, 512], F32, tag="ps", name="pk")
                sqs = {}
                for o in range(ndm):
                    nc.tensor.matmul(pk, wk_sb[:, o, :], xt_o(0, o),
                                     start=(o == 0), stop=(o == ndm - 1))
                    if o % 8 == 1 and o // 8 < HL:
                        c = o // 8
                        sqs[c] = rms_chain_pre(pq[c])
                    elif o % 8 == 5 and o // 8 < HL:
                        c = o // 8
                        psq = ps.tile([1, 512], F32, tag="ps", name="psq")
                        nc.tensor.matmul(psq, ones_sb, sqs[c],
                                         start=True, stop=True)
                        rms_chain_post(pq[c], psq, qnw_sb,
                                       qt_sb[:, c, 0, :])
                # V chunks interleave around the K rms chain and the
                # attn(0) prologue: V c=0 covers ACT finishing the Q-chain
                # sqrts before the K sum-of-squares matmul; V c=1,2 cover
                # the K chain -> kt latency before st0; V c=3 covers the
                # first exp warm-up.
                def v_chunk(c):
                    pv = ps.tile([P, HD], F32, tag="ps", name="pv")
                    for o in range(ndm):
                        nc.tensor.matmul(
                            pv, xt_chunks[(0, o // 4)][:, o % 4,
                                                       c * P:(c + 1) * P],
                            wv_sb[:, o, :],
                            start=(o == 0), stop=(o == ndm - 1))
                    nc.scalar.copy(v_sb[:, c, :], pv)

                v_chunk(0)
                sqk = rms_chain_pre(pk)
                psqk = ps.tile([1, 512], F32, tag="ps", name="psqk")
                nc.tensor.matmul(psqk, ones_sb, sqk, start=True, stop=True)
                rms_chain_post(pk, psqk, knw_sb, kt_sb[:, 0:512])
                v_chunk(1)
                v_chunk(2)
                ctx0 = attn_prologue(0, 0) if 'attn' in phases else None
                v_chunk(3)
                return ctx0

            # ---------------- filler step generators ----------------
            # Each filler step is a closure emitting ~1-2 PE matmuls (plus
            # trailing non-PE ops). Steps are pulled into attention bubbles.

            def rms_defer_a(src, pending, w_sb, dst):
                """Chunk epilogue: sum-of-squares row + raw copy to SBUF so
                the PSUM accumulator frees now; the ACT-table-switching sqrt
                runs later in one batched pocket per phase."""
                sqt = rms_chain_pre(src)
                psq = ps.tile([1, 512], F32, tag="ps", name="psq")
                nc.tensor.matmul(psq, ones_sb, sqt, start=True, stop=True)
                msrow = msr_pool.tile([1, 512], F32, tag="msr", name="msr")
                nc.scalar.copy(msrow, psq)
                raw = raw_pool.tile([P, 512], BF16, tag="raw", name="raw")
                cast_copy(raw, src)
                pending.append((msrow, raw, w_sb, dst))

            def rms_defer_b(msrow, raw, w_sb, dst):
                rms = row1.tile([1, 512], F32, tag="row1", name="rms")
                nc.scalar.activation(rms, msrow, AF.Sqrt,
                                     bias=eps_sb[:1, :], scale=1.0 / HD)
                rec = row1.tile([1, 512], F32, tag="row1", name="rec")
                nc.vector.reciprocal(rec, rms)
                invb = inv_pool.tile([P, 512], F32, tag="inv", name="invb")
                nc.gpsimd.partition_broadcast(invb, rec)
                nc.vector.scalar_tensor_tensor(
                    out=dst, in0=raw, scalar=w_sb, in1=invb,
                    op0=ALU.mult, op1=ALU.mult)

            def proj_q_pair_steps(ss, c0, c1, pending):
                """Q projection of heads c0,c1 for slice ss, o-major so a
                given xt chunk is consumed at half the single-head rate
                (stays behind the chunk DMA arrival): 64 steps."""
                state = {}

                def step(o, c):
                    def f():
                        if o == 0:
                            state[c] = ps.tile([P, 512], F32, tag="ps",
                                               name=f"pq{ss}_{c}")
                        nc.tensor.matmul(
                            state[c], wq_sb[:, o, c * HD:(c + 1) * HD],
                            xt_o(ss, o), start=(o == 0), stop=(o == ndm - 1))
                        if o == ndm - 1:
                            rms_defer_a(state[c], pending, qnw_sb,
                                        qt_sb[:, c, ss % 2, :])
                    return f
                steps = []
                for o in range(ndm):
                    steps.append(step(o, c0))
                    steps.append(step(o, c1))
                return steps

            def proj_k_steps(ss, pending):
                state = {}

                def step(o):
                    def f():
                        if o == 0:
                            state['pk'] = ps.tile([P, 512], F32, tag="ps",
                                                  name=f"pk{ss}")
                        nc.tensor.matmul(
                            state['pk'], wk_sb[:, o, :], xt_o(ss, o),
                            start=(o == 0), stop=(o == ndm - 1))
                        if o == ndm - 1:
                            rms_defer_a(state['pk'], pending, knw_sb,
                                        kt_sb[:, ss * 512:ss * 512 + 512])
                    return f
                return [step(o) for o in range(ndm)]

            def proj_v_steps(ss, cs=(0, 1, 2, 3)):
                """V projection: pos-chunks x 4 steps of 8 matmuls."""
                state = {}
                steps = []

                def step(c, g):
                    def f():
                        if g == 0:
                            state[c] = ps.tile([P, HD], F32, tag="ps",
                                               name=f"pv{ss}_{c}")
                        for o in range(8 * g, 8 * g + 8):
                            nc.tensor.matmul(
                                state[c],
                                xt_chunks[(ss, o // 4)][:, o % 4,
                                                        c * P:(c + 1) * P],
                                wv_sb[:, o, :],
                                start=(o == 0), stop=(o == ndm - 1))
                        if g == 3:
                            nc.scalar.copy(v_sb[:, 4 * ss + c, :], state[c])
                    return f
                for c in cs:
                    for g in range(4):
                        steps.append(step(c, g))
                return steps

            _copy_rr = [0]
            _copy_mode = [2]    # 2 = alternate; 3 = 2xACT:1xDVE (DVE-heavy
                                # attention phases)

            def cast_copy(dst, src):
                """PSUM f32 -> SBUF bf16 cast copy over the two PSUM-capable
                non-PE engines (GPSIMD cannot read PSUM)."""
                r = _copy_rr[0] = (_copy_rr[0] + 1) % _copy_mode[0]
                if r != 0:
                    nc.scalar.copy(dst, src)
                else:
                    nc.vector.tensor_copy(dst, src)

            def outproj_steps(ib, mi, split_dma=False):
                """Output projection block ib, m-slice mi: 8 steps of 2
                matmuls; one merged DMA after the 4 quads (or 2 half DMAs
                when split_dma, to shorten the kernel tail)."""
                m0 = mi * 512
                state = {}
                steps = []

                def step(st_i, half):
                    def f():
                        if half == 0:
                            state['fps'] = ps.tile([P, 512], F32, tag="ps",
                                                   name=f"fps{ib}_{mi}")
                            if st_i == 0:
                                state['fsb'] = fsb_pool.tile(
                                    [P, 4, 512], BF16, tag="fsb", name="fsb")
                        for c in (0, 1) if half == 0 else (2, 3):
                            nc.tensor.matmul(
                                state['fps'],
                                ot_sb[:, c, ib % 2, st_i * P:(st_i + 1) * P],
                                wo_sb[:, c, m0:m0 + 512],
                                start=(c == 0), stop=(c == HL - 1))
                        if half == 1:
                            cast_copy(state['fsb'][:, st_i, :], state['fps'])
                            if split_dma:   # per-quad DMA: shortest tail
                                dst = out_d[ib * 512 + st_i * P:
                                            ib * 512 + (st_i + 1) * P,
                                            m0:m0 + 512]
                                nc.sync.dma_start(dst, state['fsb'][:, st_i, :])
                            elif st_i == 3:
                                dst = out_d[ib * 512:ib * 512 + 512,
                                            m0:m0 + 512]
                                nc.sync.dma_start(
                                    dst.rearrange("(st p) m -> p st m", p=P),
                                    state['fsb'])
                    return f
                for st_i in range(4):
                    steps.append(step(st_i, 0))
                    steps.append(step(st_i, 1))
                return steps

            # ---------------- attention ----------------
            def attn_prologue(ss, h):
                """Allocate the head's PSUM accumulators and emit its first
                two score matmuls. Called from the PREVIOUS head's last jt
                iteration so the exp chain of head h warms while the PE is
                still busy, killing the head-boundary bubble."""
                i0 = ss * 512
                jtend = 4 * (ss + 1)
                # last phase has little filler: run a deeper score pipeline
                # (PSUM has room there -- no proj-chunk accumulators live)
                depth = 2 if ss + 1 < nss else 3
                ctx = {'otp': ps.tile([P, 512], F32, tag="ps", name="otp"),
                       'lps': ps.tile([1, 512], F32, tag="ps", name="lps"),
                       'sts': {}, 'depth': depth}

                def emit_st(jt):
                    j0 = jt * P
                    c0 = max(0, j0 - i0)
                    stt = ps.tile([P, 512], F32, tag="ps", name="st")
                    nc.tensor.matmul(
                        stt[:, c0:], kt_sb[:, j0:j0 + P],
                        qt_sb[:, h, ss % 2, c0:], start=True, stop=True)
                    ctx['sts'][jt] = (stt, c0)

                ctx['emit_st'] = emit_st
                for jt in range(min(depth, jtend)):
                    emit_st(jt)
                return ctx

            def attn_head(ss, h, pull, ctx, next_prologue=None):
                """Body of one head's attention; `ctx` from attn_prologue.
                `next_prologue` (if set) is invoked during the last jt
                iteration and its result returned."""
                i0 = ss * 512
                jtend = 4 * (ss + 1)
                jlast = jtend - 1
                otp, lps, sts = ctx['otp'], ctx['lps'], ctx['sts']
                emit_st = ctx['emit_st']
                depth = ctx['depth']
                nctx = None
                for jt in range(jtend):
                    stt, c0 = sts.pop(jt)
                    tmp = tmp_pool.tile([P, 512], F32, tag="tmp", name="tmp")
                    nc.vector.tensor_tensor(
                        tmp[:, c0:], stt[:, c0:], negrowM[:, h, c0:], ALU.add)
                    if jt * P >= i0:  # diagonal block: additive causal mask
                        nc.gpsimd.tensor_tensor(
                            tmp[:, c0:c0 + P], tmp[:, c0:c0 + P],
                            maskneg, ALU.add)
                    pt = pt_pool.tile([P, 512], BF16, tag="pt", name="pt")
                    ridx = jt - 4 * ss + RB
                    nc.scalar.activation(
                        pt[:, c0:], tmp[:, c0:], AF.Exp,
                        bias=jcolM[:, h, ridx:ridx + 1], scale=1.0)
                    if jt + depth < jtend:
                        emit_st(jt + depth)
                    pull(2 if jt < jlast else 1)
                    if jt == jlast and next_prologue is not None:
                        nctx = next_prologue()
                    nc.tensor.matmul(
                        otp[:, c0:], v_sb[:, jt, :], pt[:, c0:],
                        start=(jt == 0), stop=(jt == jlast))
                    nc.tensor.matmul(
                        lps[:, c0:], ones_bf, pt[:, c0:],
                        start=(jt == 0), stop=(jt == jlast))
                # normalize
                lrow = row1.tile([1, 512], F32, tag="row1", name="lrow")
                nc.scalar.copy(lrow, lps)
                linv = row1.tile([1, 512], F32, tag="row1", name="linv")
                nc.vector.reciprocal(linv, lrow)
                linvb = inv_pool.tile([P, 512], F32, tag="inv", name="linvb")
                nc.gpsimd.partition_broadcast(linvb, linv)
                nc.vector.tensor_tensor(
                    ot_sb[:, h, ss % 2, :], otp, linvb, ALU.mult)
                return nctx

            # ---------------- phase schedule ----------------
            def phase(ss, ctx):
                """attn(ss) + filler proj(ss+1) + outproj(ss-1). `ctx` is
                head 0's prologue (emitted by the previous phase); returns
                the next phase's head-0 prologue ctx."""
                _copy_mode[0] = 2
                # prefetch DMAs for the next slice; wo afterwards during
                # phase 0 (xt chunks gate proj(1) filler NOW, wo is not
                # read until outproj(0) in phase 1)
                if ss + 1 < nss:
                    for c in range(nxc):
                        load_xt_chunk(ss + 1, c)
                if ss == 0:
                    for q in range(4):
                        nc.sync.dma_start(
                            wo_sb[:, :, q * (DM // 4):(q + 1) * (DM // 4)],
                            wo_r[:, :, q * (DM // 4):(q + 1) * (DM // 4)])

                fill = deque()
                pending = []
                if ss + 1 < nss and 'proj' in phases:
                    qs = [proj_q_pair_steps(ss + 1, 0, 1, pending),
                          proj_q_pair_steps(ss + 1, 2, 3, pending)]
                    ks = proj_k_steps(ss + 1, pending)
                    vs = proj_v_steps(ss + 1)
                else:
                    qs, ks, vs = [], None, None
                if ss >= 1 and 'out' in phases:
                    os_ = [outproj_steps(ss - 1, mi) for mi in range(8)]
                else:
                    os_ = []
                # interleave: outproj mi-pairs between proj chunks (paced by
                # xt arrival); K early so the rsqrt pocket (right after q3)
                # finishes well before the next phase needs qt/kt, with the
                # remaining outproj + V steps as padding behind it.
                def chain_step(i):
                    def f():
                        rms_defer_b(*pending[i])
                    return f
                order = []
                proj_units = ([qs[0], ks, qs[1],
                               [chain_step(i) for i in range(HL + 1)], vs]
                              if ks is not None else [])
                ou = list(os_)
                # zip: outproj pair, proj unit, outproj pair, proj unit, ...
                pi = 0
                for i in range(max(len(ou), len(proj_units))):
                    if i < len(ou):
                        order.append(ou[i])
                    if pi < len(proj_units):
                        order.append(proj_units[pi])
                        pi += 1
                for lst in order:
                    fill.extend(lst)

                def pull(n):
                    for _ in range(n):
                        if fill:
                            fill.popleft()()

                total = len(fill)
                if ctx is None and 'attn' in phases:
                    ctx = attn_prologue(ss, 0)
                for h in range(HL):
                    if 'attn' in phases:
                        nxt = None
                        if h + 1 < HL:
                            def nxt(hh=h + 1):
                                return attn_prologue(ss, hh)
                        ctx = attn_head(ss, h, pull, ctx, nxt)
                    # drain this head's share of the filler, holding back a
                    # few steps to cover the next phase's exp warm-up
                    hold = 14 if h == HL - 1 else 0
                    target = (total * (h + 1)) // HL - hold
                    while len(fill) > max(0, total - target):
                        fill.popleft()()
                nctx = None
                if ss + 1 < nss and 'attn' in phases:
                    nctx = attn_prologue(ss + 1, 0)
                while fill:
                    fill.popleft()()
                return nctx

            def outproj_final(ib):
                _copy_mode[0] = 2
                for mi in range(8):
                    for st in outproj_steps(ib, mi, split_dma=(mi >= 6)):
                        st()

            for _rep in range(repeat):
                ctx = None
                if 'proj' in phases:
                    ctx = proj_initial()
                for ss in range(nss):
                    ctx = phase(ss, ctx)
                if 'out' in phases and 'attn' in phases:
                    outproj_final(nss - 1)

    nc.finalize()
    return nc


def _prep_kv(w):
    """[HD, DM] weight -> [P, ndm*HD] laid out as [p][o][m] (8KB runs)."""
    ndm = DM // P
    wt = np.ascontiguousarray(w.T)                       # [DM, HD]
    return np.ascontiguousarray(
        wt.reshape(ndm, P, HD).transpose(1, 0, 2).reshape(P, ndm * HD)
    ).astype(NBF)


def shard_inputs(x, Wq, Wk, Wv, Wo, q_norm_w, k_norm_w, s=S):
    """Host-side shard + layout prep. Returns per-core input maps."""
    slopes = _alibi_slopes(H)
    xt = np.ascontiguousarray(x.reshape(s, DM).T).astype(NBF)
    qnw = (np.asarray(q_norm_w, np.float32) / math.sqrt(HD)).reshape(HD, 1)
    knw = np.asarray(k_norm_w, np.float32).reshape(HD, 1).copy()
    in_maps = []
    for g in range(NC_CORES):
        qs = g * HL * HD
        sl = slopes[g * HL:(g + 1) * HL]
        in_maps.append({
            "xt": xt,
            "wq": np.ascontiguousarray(Wq[qs:qs + HL * HD, :].T).astype(NBF),
            "wk": _prep_kv(Wk[g * HD:(g + 1) * HD, :]),
            "wv": _prep_kv(Wv[g * HD:(g + 1) * HD, :]),
            "wo": np.ascontiguousarray(Wo[:, qs:qs + HL * HD].T).astype(NBF),
            "qnw": qnw,
            "knw": knw,
            "slp": np.ascontiguousarray(
                np.broadcast_to(sl, (P, HL))).astype(np.float32),
            "nslp": np.ascontiguousarray(
                np.broadcast_to(-sl, (P, HL))).astype(np.float32),
        })
    return in_maps


_MODULE_CACHE = {}
LAST_RESULT = None


def _get_module(s=S):
    if s not in _MODULE_CACHE:
        _MODULE_CACHE[s] = build_module(s)
    return _MODULE_CACHE[s]


def kernel(x, Wq, Wk, Wv, Wo, q_norm_w, k_norm_w, **run_kwargs):
    global LAST_RESULT
    from concourse.bass_utils import run_bass_kernel_spmd

    x = np.asarray(x)
    in_maps = shard_inputs(np.asarray(x), np.asarray(Wq), np.asarray(Wk),
                           np.asarray(Wv), np.asarray(Wo),
                           np.asarray(q_norm_w), np.asarray(k_norm_w))
    nc = _get_module(S)
    res = run_bass_kernel_spmd(nc, in_maps, core_ids=list(range(NC_CORES)),
                               **run_kwargs)
    LAST_RESULT = res
    acc = np.zeros((S, DM), np.float32)
    for r in res.results:
        acc += r["out"].astype(np.float32)
    return acc.reshape(B, S, DM)


# revision 77
# speedup vs baseline: 1.2127x; 1.0043x over previous
"""Trainium2 Bass kernel: GQA causal self-attention with ALiBi + QK-RMSNorm.

Model: B=1, S=2048, DM=4096, H=32 q-heads, HKV=8 kv-heads, HD=128.
Sharding: tensor-parallel over heads across 8 cores. Core g computes
q-heads 4g..4g+3 with kv-head g, and a row-parallel partial of the output
projection; the host sums the 8 partials (the unshard for row-parallel Wo).

Layout strategy (per core):
  - x is passed transposed (XT [DM,S]) so every projection matmul contracts
    over DM on the partition axis with no on-device transposes.
  - Q,K are produced transposed ([d, s]); V natural ([s, d]).
  - RMSNorm over d (= partition axis) uses a ones-vector matmul for the
    per-position sum of squares, then a GPSIMD partition_broadcast of 1/rms.
  - Scores are computed transposed: S^T[j,i] (j=key pos on partitions,
    i=query pos on free axis). With q scaled by 1/sqrt(HD) and RMSNormed,
    |s| <= sqrt(128) and the ALiBi bias slope*(j-i) <= 0 after causal
    masking, so exp() cannot overflow and NO row-max pass is needed.
    exp bias: +slope*(j-i0) enters via the ACT per-partition bias operand,
    -slope*(i-i0) via one row add (DVE/Pool alternating); the causal mask is
    a precomputed [128,128] additive -1e30 triangle on diagonal blocks.
  - P^T tiles feed the PV matmul as rhs with V as lhsT, accumulating O^T
    [d, i] directly in PSUM (no transposes anywhere). A ones-lhsT matmul
    accumulates the softmax denominators as a row, normalized via
    reciprocal + partition_broadcast.

Scheduling strategy (v2):
  - xt streams in 8 chunks per 512-slice ([P, 4o, 512]); the startup DMA
    order interleaves wq pieces with xt chunks (first loads split in half)
    so the PE starts ~3us in. wk/wv are host-packed to the SBUF layout so
    their DMA descriptors are 8KB runs (the [DM, HD] layout would give
    256B descriptors, which the DMA does at half throughput).
  - One PE "filler queue" per attention phase: attention for block ss runs
    with proj(ss+1) and outproj(ss-1) matmul steps pulled into the exp-
    latency bubbles of the jt pipeline (st(jt+depth) is emitted depth=2
    iterations ahead, 3 in the filler-poor last phase; otp/lps trail once
    exp(jt) lands). Q-projection filler runs as head PAIRS iterating
    o-major, so each xt chunk is consumed slower than its DMA delivers it
    (chunk-major sweeps outran the serial DMA engine and stalled). Each
    head's first score matmuls are emitted during the previous head's last
    iteration (attn_prologue chaining), and phase(ss+1)'s head-0 prologue
    is emitted under the last 8 filler steps of phase ss.
  - RMSNorm is split: chunk epilogues bank the sum-of-squares row and a
    raw bf16 copy (freeing PSUM); the ACT-table-switching Sqrt runs in one
    batched pocket per phase (2 LoadActFuncSet round trips per phase
    instead of ~10 -- Sqrt and Exp live in different ACT table sets).
  - Per-head ALiBi bias rows/columns are precomputed once (masters), so a
    head costs no setup.
  - Output stores: 4 PSUM->SBUF cast copies (alternating ACT/DVE; GPSIMD
    cannot read PSUM) into one [P,4,512] tile, then ONE merged DMA; the
    final m-slices use per-quad DMAs to shorten the kernel tail.
"""

import math
from collections import deque

import numpy as np
import ml_dtypes

import concourse.bass as bass
import concourse.bacc as bacc
import concourse.mybir as mybir
import concourse.tile as tile

F32 = mybir.dt.float32
BF16 = mybir.dt.bfloat16
AF = mybir.ActivationFunctionType
ALU = mybir.AluOpType

B, S, DM = 1, 2048, 4096
H, HKV, HD = 32, 8, 128
NC_CORES = 8
HL = H // NC_CORES          # 4 local q heads per core
EPS = 1e-6
NEG = -1.0e30
P = 128

NBF = ml_dtypes.bfloat16


def _alibi_slopes(n_heads: int) -> np.ndarray:
    start = 2 ** (-(2 ** (-(math.log2(n_heads) - 3))))
    return np.array([start * (start**i) for i in range(n_heads)], dtype=np.float32)


def build_module(s: int = S, repeat: int = 1, phases=('proj', 'attn', 'out')):
    """Build the per-core Bass module. `s` parameterized for small tests."""
    assert s % 512 == 0
    nss = s // 512            # 512-wide s slices / query blocks
    njt = s // P              # 128-wide key tiles
    ndm = DM // P             # 32 contraction tiles
    nxc = ndm // 4            # 8 xt chunks per slice (4 o's each)
    NJ = 4 * nss              # jcol master width
    RB = 4 * (nss - 1)        # ridx bias: ridx = jt - 4*ss + RB in [0, NJ)

    nc = bacc.Bacc(trn_type="TRN2")

    xt_d = nc.dram_tensor("xt", [DM, s], BF16, kind="ExternalInput")
    wq_d = nc.dram_tensor("wq", [DM, HL * HD], BF16, kind="ExternalInput")
    wk_d = nc.dram_tensor("wk", [P, (DM // P) * HD], BF16, kind="ExternalInput")
    wv_d = nc.dram_tensor("wv", [P, (DM // P) * HD], BF16, kind="ExternalInput")
    wo_d = nc.dram_tensor("wo", [HL * HD, DM], BF16, kind="ExternalInput")
    qnw_d = nc.dram_tensor("qnw", [HD, 1], F32, kind="ExternalInput")
    knw_d = nc.dram_tensor("knw", [HD, 1], F32, kind="ExternalInput")
    slp_d = nc.dram_tensor("slp", [P, HL], F32, kind="ExternalInput")
    nslp_d = nc.dram_tensor("nslp", [P, HL], F32, kind="ExternalInput")
    out_d = nc.dram_tensor("out", [s, DM], BF16, kind="ExternalOutput")

    with tile.TileContext(nc) as tc:
        with (
            tc.tile_pool(name="const", bufs=1) as const,
            tc.tile_pool(name="xtc", bufs=10) as xtc_pool,
            tc.tile_pool(name="big", bufs=1) as big,
            tc.tile_pool(name="sq", bufs=2) as sq_pool,
            tc.tile_pool(name="row1", bufs=3) as row1,
            tc.tile_pool(name="msr", bufs=6) as msr_pool,
            tc.tile_pool(name="raw", bufs=6) as raw_pool,
            tc.tile_pool(name="inv", bufs=2) as inv_pool,
            tc.tile_pool(name="tmp", bufs=4) as tmp_pool,
            tc.tile_pool(name="pt", bufs=4) as pt_pool,
            tc.tile_pool(name="fsb", bufs=2) as fsb_pool,
            tc.tile_pool(name="ps", bufs=8, space="PSUM") as ps,
        ):
            # ---------------- small constants (loads deferred into proj) --
            qnw_sb = const.tile([P, 1], F32)
            knw_sb = const.tile([P, 1], F32)
            slp_sb = const.tile([P, HL], F32)
            nslp_sb = const.tile([P, HL], F32)

            def load_small_consts():
                nc.sync.dma_start(qnw_sb, qnw_d[:, :])
                nc.sync.dma_start(knw_sb, knw_d[:, :])
                nc.sync.dma_start(slp_sb, slp_d[:, :])
                nc.sync.dma_start(nslp_sb, nslp_d[:, :])

            # weights tiles (loads interleaved below / deferred)
            wq_sb = const.tile([P, ndm, HL * HD], BF16)
            wq_r = wq_d[:, :].rearrange("(o p) m -> p o m", p=P)
            wk_sb = const.tile([P, ndm, HD], BF16)
            wv_sb = const.tile([P, ndm, HD], BF16)
            wo_sb = const.tile([P, HL, DM], BF16)
            wo_r = wo_d[:, :].rearrange("(o p) m -> p o m", p=P)

            ones_f32 = const.tile([P, 1], F32)
            nc.vector.memset(ones_f32, 1.0)
            ones_sb = const.tile([P, 1], mybir.dt.float32r)
            nc.scalar.copy(ones_sb, ones_f32)
            ones_bf = const.tile([P, 1], BF16)
            # sqrt(1.0) == 1.0; issuing it here pulls the ACT sqrt-table
            # load to t~0 (idle ACT) instead of mid-K-pass where it delays
            # the rmsnorm Squares queued behind it
            nc.scalar.activation(ones_bf, ones_f32, AF.Sqrt)
            eps_sb = const.tile([P, 1], F32)
            nc.vector.memset(eps_sb, EPS)

            # per-head ALiBi masters:
            #   negrowM[p, h, f] = -slope_h * f           (query-col row add)
            #   jcolM[p, h, t]   = slope_h * (128*(t-RB) + p)  (exp bias; at
            #       t = jt - 4*ss + RB it equals slope*(128*jt + p - 512*ss))
            iota_row = const.tile([P, 512], F32)
            nc.gpsimd.iota(iota_row, pattern=[[1, 512]], base=0,
                           channel_multiplier=0,
                           allow_small_or_imprecise_dtypes=True)
            iota_j = const.tile([P, NJ], F32)
            nc.gpsimd.iota(iota_j, pattern=[[P, NJ]], base=-RB * P,
                           channel_multiplier=1,
                           allow_small_or_imprecise_dtypes=True)
            negrowM = const.tile([P, HL, 512], F32)
            jcolM = const.tile([P, HL, NJ], F32)

            def build_masters():
                for h in range(HL):
                    nc.gpsimd.tensor_tensor(
                        negrowM[:, h, :], iota_row,
                        nslp_sb[:, h:h + 1].to_broadcast([P, 512]), ALU.mult)
                    nc.gpsimd.tensor_tensor(
                        jcolM[:, h, :], iota_j,
                        slp_sb[:, h:h + 1].to_broadcast([P, NJ]), ALU.mult)

            # maskneg[p, f] = 0 where p <= f else -1e30  (additive causal
            # mask for diagonal 128x128 blocks of S^T)
            maskneg = const.tile([P, P], F32)
            nc.gpsimd.memset(maskneg, 0.0)
            nc.gpsimd.affine_select(
                out=maskneg, in_=maskneg,
                compare_op=ALU.is_ge, fill=NEG,
                base=0, pattern=[[1, P]], channel_multiplier=-1,
            )

            # ---------------- persistent activations ----------------
            qt_sb = big.tile([P, HL, 2, 512], BF16)  # Q^T ring [d, h, ss%2, i]
            kt_sb = big.tile([P, s], BF16)           # K^T      [d, s]
            v_sb = big.tile([P, njt, HD], BF16)      # V        [s, d]
            ot_sb = big.tile([P, HL, 2, 512], BF16)  # O^T ring [d, h, ib%2, i]

            xt_r = xt_d[:, :].rearrange("(o p) t -> p o t", p=P)

            # xt chunk bookkeeping: chunks[(ss, c)] = sbuf tile [P, 4, 512]
            xt_chunks = {}

            def load_xt_chunk(ss, c):
                t = xtc_pool.tile([P, 4, 512], BF16, tag="xtc",
                                  name=f"xt{ss}_{c}")
                nc.sync.dma_start(
                    t, xt_r[:, 4 * c:4 * c + 4, ss * 512:ss * 512 + 512])
                xt_chunks[(ss, c)] = t

            def xt_o(ss, o):
                return xt_chunks[(ss, o // 4)][:, o % 4, :]

            # ---------------- rmsnorm chain (non-PE parts) ----------------
            def rms_chain_pre(src):
                """Square the PSUM tile; returns sq tile (ACT)."""
                sqt = sq_pool.tile([P, 512], mybir.dt.float32r,
                                   tag="sq", name="sqt")
                nc.scalar.activation(sqt, src, AF.Square)
                return sqt

            def rms_chain_post(src, psq, w_sb, dst):
                """After PE computed psq = ones^T @ sqt: finish the norm."""
                rms = row1.tile([1, 512], F32, tag="row1", name="rms")
                nc.scalar.activation(rms, psq, AF.Sqrt,
                                     bias=eps_sb[:1, :], scale=1.0 / HD)
                rec = row1.tile([1, 512], F32, tag="row1", name="rec")
                nc.vector.reciprocal(rec, rms)
                invb = inv_pool.tile([P, 512], F32, tag="inv", name="invb")
                nc.gpsimd.partition_broadcast(invb, rec)
                nc.vector.scalar_tensor_tensor(
                    out=dst, in0=src, scalar=w_sb, in1=invb,
                    op0=ALU.mult, op1=ALU.mult)

            # ---------------- initial projection (slice 0) ----------------
            def proj_initial():
                """proj(0): DMA-paced. Q pass (o-major over 4 heads), K pass,
                V pass; weight quarters interleave with xt chunks."""
                # Q pass
                pq = [ps.tile([P, 512], F32, tag="ps", name=f"pq{c}")
                      for c in range(HL)]
                for o in range(ndm):
                    if o == 0:          # finest first loads: PE starts ~3us
                        nc.sync.dma_start(wq_sb[:, 0:2, :], wq_r[:, 0:2, :])
                        t0 = xtc_pool.tile([P, 4, 512], BF16, tag="xtc",
                                           name="xt0_0")
                        nc.sync.dma_start(t0[:, 0:2, :], xt_r[:, 0:2, 0:512])
                        nc.sync.dma_start(wq_sb[:, 2:4, :], wq_r[:, 2:4, :])
                        nc.sync.dma_start(t0[:, 2:4, :], xt_r[:, 2:4, 0:512])
                        xt_chunks[(0, 0)] = t0
                    elif o == 2:
                        nc.sync.dma_start(wq_sb[:, 4:8, :], wq_r[:, 4:8, :])
                    elif o % 8 == 0:    # wq quarter q = o//8
                        q = o // 8
                        nc.sync.dma_start(
                            wq_sb[:, 8 * q:8 * q + 8, :],
                            wq_r[:, 8 * q:8 * q + 8, :])
                    if o % 4 == 0 and o > 0:
                        load_xt_chunk(0, o // 4)
                    if o == 1:
                        load_small_consts()
                        build_masters()
                    if o == ndm - 8:
                        nc.sync.dma_start(
                            wk_sb, wk_d[:, :].rearrange(
                                "p (o m) -> p o m", o=ndm))
                        nc.sync.dma_start(
                            wv_sb, wv_d[:, :].rearrange(
                                "p (o m) -> p o m", o=ndm))
                    for c in range(HL):
                        nc.tensor.matmul(
                            pq[c], wq_sb[:, o, c * HD:(c + 1) * HD],
                            xt_o(0, o), start=(o == 0), stop=(o == ndm - 1))
                # K pass (Q rmsnorm chains interleave into it)
                pk = ps.tile([P, 512], F32, tag="ps", name="pk")
                sqs = {}
                for o in range(ndm):
                    nc.tensor.matmul(pk, wk_sb[:, o, :], xt_o(0, o),
                                     start=(o == 0), stop=(o == ndm - 1))
                    if o % 8 == 1 and o // 8 < HL:
                        c = o // 8
                        sqs[c] = rms_chain_pre(pq[c])
                    elif o % 8 == 5 and o // 8 < HL:
                        c = o // 8
                        psq = ps.tile([1, 512], F32, tag="ps", name="psq")
                        nc.tensor.matmul(psq, ones_sb, sqs[c],
                                         start=True, stop=True)
                        rms_chain_post(pq[c], psq, qnw_sb,
                                       qt_sb[:, c, 0, :])
                # V chunks interleave around the K rms chain and the
                # attn(0) prologue: V c=0 covers ACT finishing the Q-chain
                # sqrts before the K sum-of-squares matmul; V c=1,2 cover
                # the K chain -> kt latency before st0; V c=3 covers the
                # first exp warm-up.
                def v_chunk(c):
                    pv = ps.tile([P, HD], F32, tag="ps", name="pv")
                    for o in range(ndm):
                        nc.tensor.matmul(
                            pv, xt_chunks[(0, o // 4)][:, o % 4,
                                                       c * P:(c + 1) * P],
                            wv_sb[:, o, :],
                            start=(o == 0), stop=(o == ndm - 1))
                    nc.scalar.copy(v_sb[:, c, :], pv)

                v_chunk(0)
                sqk = rms_chain_pre(pk)
                psqk = ps.tile([1, 512], F32, tag="ps", name="psqk")
                nc.tensor.matmul(psqk, ones_sb, sqk, start=True, stop=True)
                rms_chain_post(pk, psqk, knw_sb, kt_sb[:, 0:512])
                # throwaway Exp: pulls the exp-table load under the V-pass
                # matmuls instead of into head 0's first softmax chain
                warm = row1.tile([1, 1], F32, tag="row1", name="warm")
                nc.scalar.activation(warm, eps_sb[:1, :1], AF.Exp)
                v_chunk(1)
                v_chunk(2)
                ctx0 = attn_prologue(0, 0) if 'attn' in phases else None
                v_chunk(3)
                return ctx0

            # ---------------- filler step generators ----------------
            # Each filler step is a closure emitting ~1-2 PE matmuls (plus
            # trailing non-PE ops). Steps are pulled into attention bubbles.

            def rms_defer_a(src, pending, w_sb, dst):
                """Chunk epilogue: sum-of-squares row + raw copy to SBUF so
                the PSUM accumulator frees now; the ACT-table-switching sqrt
                runs later in one batched pocket per phase."""
                sqt = rms_chain_pre(src)
                psq = ps.tile([1, 512], F32, tag="ps", name="psq")
                nc.tensor.matmul(psq, ones_sb, sqt, start=True, stop=True)
                msrow = msr_pool.tile([1, 512], F32, tag="msr", name="msr")
                nc.scalar.copy(msrow, psq)
                raw = raw_pool.tile([P, 512], BF16, tag="raw", name="raw")
                cast_copy(raw, src)
                pending.append((msrow, raw, w_sb, dst))

            def rms_defer_b(msrow, raw, w_sb, dst):
                rms = row1.tile([1, 512], F32, tag="row1", name="rms")
                nc.scalar.activation(rms, msrow, AF.Sqrt,
                                     bias=eps_sb[:1, :], scale=1.0 / HD)
                rec = row1.tile([1, 512], F32, tag="row1", name="rec")
                nc.vector.reciprocal(rec, rms)
                invb = inv_pool.tile([P, 512], F32, tag="inv", name="invb")
                nc.gpsimd.partition_broadcast(invb, rec)
                nc.vector.scalar_tensor_tensor(
                    out=dst, in0=raw, scalar=w_sb, in1=invb,
                    op0=ALU.mult, op1=ALU.mult)

            def proj_q_pair_steps(ss, c0, c1, pending):
                """Q projection of heads c0,c1 for slice ss, o-major so a
                given xt chunk is consumed at half the single-head rate
                (stays behind the chunk DMA arrival): 64 steps."""
                state = {}

                def step(o, c):
                    def f():
                        if o == 0:
                            state[c] = ps.tile([P, 512], F32, tag="ps",
                                               name=f"pq{ss}_{c}")
                        nc.tensor.matmul(
                            state[c], wq_sb[:, o, c * HD:(c + 1) * HD],
                            xt_o(ss, o), start=(o == 0), stop=(o == ndm - 1))
                        if o == ndm - 1:
                            rms_defer_a(state[c], pending, qnw_sb,
                                        qt_sb[:, c, ss % 2, :])
                    return f
                steps = []
                for o in range(ndm):
                    steps.append(step(o, c0))
                    steps.append(step(o, c1))
                return steps

            def proj_k_steps(ss, pending):
                state = {}

                def step(o):
                    def f():
                        if o == 0:
                            state['pk'] = ps.tile([P, 512], F32, tag="ps",
                                                  name=f"pk{ss}")
                        nc.tensor.matmul(
                            state['pk'], wk_sb[:, o, :], xt_o(ss, o),
                            start=(o == 0), stop=(o == ndm - 1))
                        if o == ndm - 1:
                            rms_defer_a(state['pk'], pending, knw_sb,
                                        kt_sb[:, ss * 512:ss * 512 + 512])
                    return f
                return [step(o) for o in range(ndm)]

            def proj_v_steps(ss, cs=(0, 1, 2, 3)):
                """V projection: pos-chunks x 4 steps of 8 matmuls."""
                state = {}
                steps = []

                def step(c, g):
                    def f():
                        if g == 0:
                            state[c] = ps.tile([P, HD], F32, tag="ps",
                                               name=f"pv{ss}_{c}")
                        for o in range(8 * g, 8 * g + 8):
                            nc.tensor.matmul(
                                state[c],
                                xt_chunks[(ss, o // 4)][:, o % 4,
                                                        c * P:(c + 1) * P],
                                wv_sb[:, o, :],
                                start=(o == 0), stop=(o == ndm - 1))
                        if g == 3:
                            nc.scalar.copy(v_sb[:, 4 * ss + c, :], state[c])
                    return f
                for c in cs:
                    for g in range(4):
                        steps.append(step(c, g))
                return steps

            _copy_rr = [0]
            _copy_mode = [2]    # 2 = alternate; 3 = 2xACT:1xDVE (DVE-heavy
                                # attention phases)

            def cast_copy(dst, src):
                """PSUM f32 -> SBUF bf16 cast copy over the two PSUM-capable
                non-PE engines (GPSIMD cannot read PSUM)."""
                r = _copy_rr[0] = (_copy_rr[0] + 1) % _copy_mode[0]
                if r != 0:
                    nc.scalar.copy(dst, src)
                else:
                    nc.vector.tensor_copy(dst, src)

            def outproj_steps(ib, mi, split_dma=False):
                """Output projection block ib, m-slice mi: 8 steps of 2
                matmuls; one merged DMA after the 4 quads (or 2 half DMAs
                when split_dma, to shorten the kernel tail)."""
                m0 = mi * 512
                state = {}
                steps = []

                def step(st_i, half):
                    def f():
                        if half == 0:
                            state['fps'] = ps.tile([P, 512], F32, tag="ps",
                                                   name=f"fps{ib}_{mi}")
                            if st_i == 0:
                                state['fsb'] = fsb_pool.tile(
                                    [P, 4, 512], BF16, tag="fsb", name="fsb")
                        for c in (0, 1) if half == 0 else (2, 3):
                            nc.tensor.matmul(
                                state['fps'],
                                ot_sb[:, c, ib % 2, st_i * P:(st_i + 1) * P],
                                wo_sb[:, c, m0:m0 + 512],
                                start=(c == 0), stop=(c == HL - 1))
                        if half == 1:
                            cast_copy(state['fsb'][:, st_i, :], state['fps'])
                            if split_dma:   # per-quad DMA: shortest tail
                                dst = out_d[ib * 512 + st_i * P:
                                            ib * 512 + (st_i + 1) * P,
                                            m0:m0 + 512]
                                nc.sync.dma_start(dst, state['fsb'][:, st_i, :])
                            elif st_i == 3:
                                dst = out_d[ib * 512:ib * 512 + 512,
                                            m0:m0 + 512]
                                nc.sync.dma_start(
                                    dst.rearrange("(st p) m -> p st m", p=P),
                                    state['fsb'])
                    return f
                for st_i in range(4):
                    steps.append(step(st_i, 0))
                    steps.append(step(st_i, 1))
                return steps

            # ---------------- attention ----------------
            def attn_prologue(ss, h):
                """Allocate the head's PSUM accumulators and emit its first
                two score matmuls. Called from the PREVIOUS head's last jt
                iteration so the exp chain of head h warms while the PE is
                still busy, killing the head-boundary bubble."""
                i0 = ss * 512
                jtend = 4 * (ss + 1)
                # last phase has little filler: run a deeper score pipeline
                # (PSUM has room there -- no proj-chunk accumulators live)
                depth = 2 if ss + 1 < nss else 3
                ctx = {'otp': ps.tile([P, 512], F32, tag="ps", name="otp"),
                       'lps': ps.tile([1, 512], F32, tag="ps", name="lps"),
                       'sts': {}, 'depth': depth}

                def emit_st(jt):
                    j0 = jt * P
                    c0 = max(0, j0 - i0)
                    stt = ps.tile([P, 512], F32, tag="ps", name="st")
                    nc.tensor.matmul(
                        stt[:, c0:], kt_sb[:, j0:j0 + P],
                        qt_sb[:, h, ss % 2, c0:], start=True, stop=True)
                    ctx['sts'][jt] = (stt, c0)

                ctx['emit_st'] = emit_st
                for jt in range(min(depth, jtend)):
                    emit_st(jt)
                return ctx

            def attn_head(ss, h, pull, ctx, next_prologue=None):
                """Body of one head's attention; `ctx` from attn_prologue.
                `next_prologue` (if set) is invoked during the last jt
                iteration and its result returned."""
                i0 = ss * 512
                jtend = 4 * (ss + 1)
                jlast = jtend - 1
                otp, lps, sts = ctx['otp'], ctx['lps'], ctx['sts']
                emit_st = ctx['emit_st']
                depth = ctx['depth']
                nctx = None
                for jt in range(jtend):
                    stt, c0 = sts.pop(jt)
                    tmp = tmp_pool.tile([P, 512], F32, tag="tmp", name="tmp")
                    nc.vector.tensor_tensor(
                        tmp[:, c0:], stt[:, c0:], negrowM[:, h, c0:], ALU.add)
                    if jt * P >= i0:  # diagonal block: additive causal mask
                        nc.gpsimd.tensor_tensor(
                            tmp[:, c0:c0 + P], tmp[:, c0:c0 + P],
                            maskneg, ALU.add)
                    pt = pt_pool.tile([P, 512], BF16, tag="pt", name="pt")
                    ridx = jt - 4 * ss + RB
                    nc.scalar.activation(
                        pt[:, c0:], tmp[:, c0:], AF.Exp,
                        bias=jcolM[:, h, ridx:ridx + 1], scale=1.0)
                    if jt + depth < jtend:
                        emit_st(jt + depth)
                    pull(2 if jt < jlast else 1)
                    if jt == jlast and next_prologue is not None:
                        nctx = next_prologue()
                    nc.tensor.matmul(
                        otp[:, c0:], v_sb[:, jt, :], pt[:, c0:],
                        start=(jt == 0), stop=(jt == jlast))
                    nc.tensor.matmul(
                        lps[:, c0:], ones_bf, pt[:, c0:],
                        start=(jt == 0), stop=(jt == jlast))
                # normalize
                lrow = row1.tile([1, 512], F32, tag="row1", name="lrow")
                nc.scalar.copy(lrow, lps)
                linv = row1.tile([1, 512], F32, tag="row1", name="linv")
                nc.vector.reciprocal(linv, lrow)
                linvb = inv_pool.tile([P, 512], F32, tag="inv", name="linvb")
                nc.gpsimd.partition_broadcast(linvb, linv)
                nc.vector.tensor_tensor(
                    ot_sb[:, h, ss % 2, :], otp, linvb, ALU.mult)
                return nctx

            # ---------------- phase schedule ----------------
            def phase(ss, ctx):
                """attn(ss) + filler proj(ss+1) + outproj(ss-1). `ctx` is
                head 0's prologue (emitted by the previous phase); returns
                the next phase's head-0 prologue ctx."""
                _copy_mode[0] = 2
                # prefetch DMAs for the next slice; wo afterwards during
                # phase 0 (xt chunks gate proj(1) filler NOW, wo is not
                # read until outproj(0) in phase 1)
                if ss + 1 < nss:
                    for c in range(nxc):
                        load_xt_chunk(ss + 1, c)
                if ss == 0:
                    for q in range(4):
                        nc.sync.dma_start(
                            wo_sb[:, :, q * (DM // 4):(q + 1) * (DM // 4)],
                            wo_r[:, :, q * (DM // 4):(q + 1) * (DM // 4)])

                fill = deque()
                pending = []
                if ss + 1 < nss and 'proj' in phases:
                    qs = [proj_q_pair_steps(ss + 1, 0, 1, pending),
                          proj_q_pair_steps(ss + 1, 2, 3, pending)]
                    ks = proj_k_steps(ss + 1, pending)
                    vs = proj_v_steps(ss + 1)
                else:
                    qs, ks, vs = [], None, None
                if ss >= 1 and 'out' in phases:
                    os_ = [outproj_steps(ss - 1, mi) for mi in range(8)]
                else:
                    os_ = []
                # interleave: outproj mi-pairs between proj chunks (paced by
                # xt arrival); K early so the rsqrt pocket (right after q3)
                # finishes well before the next phase needs qt/kt, with the
                # remaining outproj + V steps as padding behind it.
                def chain_step(i):
                    def f():
                        rms_defer_b(*pending[i])
                        if i == HL:   # pocket done: re-warm the exp table
                            warm = row1.tile([1, 1], F32, tag="row1",
                                             name="warm")
                            nc.scalar.activation(warm, eps_sb[:1, :1],
                                                 AF.Exp)
                    return f
                order = []
                proj_units = ([qs[0], ks, qs[1],
                               [chain_step(i) for i in range(HL + 1)], vs]
                              if ks is not None else [])
                ou = list(os_)
                # zip: outproj pair, proj unit, outproj pair, proj unit, ...
                pi = 0
                for i in range(max(len(ou), len(proj_units))):
                    if i < len(ou):
                        order.append(ou[i])
                    if pi < len(proj_units):
                        order.append(proj_units[pi])
                        pi += 1
                for lst in order:
                    fill.extend(lst)

                def pull(n):
                    for _ in range(n):
                        if fill:
                            fill.popleft()()

                total = len(fill)
                if ctx is None and 'attn' in phases:
                    ctx = attn_prologue(ss, 0)
                for h in range(HL):
                    if 'attn' in phases:
                        nxt = None
                        if h + 1 < HL:
                            def nxt(hh=h + 1):
                                return attn_prologue(ss, hh)
                        ctx = attn_head(ss, h, pull, ctx, nxt)
                    # drain this head's share of the filler, holding back a
                    # few steps to cover the next phase's exp warm-up
                    hold = 14 if h == HL - 1 else 0
                    target = (total * (h + 1)) // HL - hold
                    while len(fill) > max(0, total - target):
                        fill.popleft()()
                nctx = None
                if ss + 1 < nss and 'attn' in phases:
                    nctx = attn_prologue(ss + 1, 0)
                while fill:
                    fill.popleft()()
                return nctx

            def outproj_final(ib):
                _copy_mode[0] = 2
                for mi in range(8):
                    for st in outproj_steps(ib, mi, split_dma=(mi >= 6)):
                        st()

            for _rep in range(repeat):
                ctx = None
                if 'proj' in phases:
                    ctx = proj_initial()
                for ss in range(nss):
                    ctx = phase(ss, ctx)
                if 'out' in phases and 'attn' in phases:
                    outproj_final(nss - 1)

    nc.finalize()
    return nc


def _prep_kv(w):
    """[HD, DM] weight -> [P, ndm*HD] laid out as [p][o][m] (8KB runs)."""
    ndm = DM // P
    wt = np.ascontiguousarray(w.T)                       # [DM, HD]
    return np.ascontiguousarray(
        wt.reshape(ndm, P, HD).transpose(1, 0, 2).reshape(P, ndm * HD)
    ).astype(NBF)


def shard_inputs(x, Wq, Wk, Wv, Wo, q_norm_w, k_norm_w, s=S):
    """Host-side shard + layout prep. Returns per-core input maps."""
    slopes = _alibi_slopes(H)
    xt = np.ascontiguousarray(x.reshape(s, DM).T).astype(NBF)
    qnw = (np.asarray(q_norm_w, np.float32) / math.sqrt(HD)).reshape(HD, 1)
    knw = np.asarray(k_norm_w, np.float32).reshape(HD, 1).copy()
    in_maps = []
    for g in range(NC_CORES):
        qs = g * HL * HD
        sl = slopes[g * HL:(g + 1) * HL]
        in_maps.append({
            "xt": xt,
            "wq": np.ascontiguousarray(Wq[qs:qs + HL * HD, :].T).astype(NBF),
            "wk": _prep_kv(Wk[g * HD:(g + 1) * HD, :]),
            "wv": _prep_kv(Wv[g * HD:(g + 1) * HD, :]),
            "wo": np.ascontiguousarray(Wo[:, qs:qs + HL * HD].T).astype(NBF),
            "qnw": qnw,
            "knw": knw,
            "slp": np.ascontiguousarray(
                np.broadcast_to(sl, (P, HL))).astype(np.float32),
            "nslp": np.ascontiguousarray(
                np.broadcast_to(-sl, (P, HL))).astype(np.float32),
        })
    return in_maps


_MODULE_CACHE = {}
LAST_RESULT = None


def _get_module(s=S):
    if s not in _MODULE_CACHE:
        _MODULE_CACHE[s] = build_module(s)
    return _MODULE_CACHE[s]


def kernel(x, Wq, Wk, Wv, Wo, q_norm_w, k_norm_w, **run_kwargs):
    global LAST_RESULT
    from concourse.bass_utils import run_bass_kernel_spmd

    x = np.asarray(x)
    in_maps = shard_inputs(np.asarray(x), np.asarray(Wq), np.asarray(Wk),
                           np.asarray(Wv), np.asarray(Wo),
                           np.asarray(q_norm_w), np.asarray(k_norm_w))
    nc = _get_module(S)
    res = run_bass_kernel_spmd(nc, in_maps, core_ids=list(range(NC_CORES)),
                               **run_kwargs)
    LAST_RESULT = res
    acc = np.zeros((S, DM), np.float32)
    for r in res.results:
        acc += r["out"].astype(np.float32)
    return acc.reshape(B, S, DM)


# revision 79
# speedup vs baseline: 1.2346x; 1.0181x over previous
"""Trainium2 Bass kernel: GQA causal self-attention with ALiBi + QK-RMSNorm.

Model: B=1, S=2048, DM=4096, H=32 q-heads, HKV=8 kv-heads, HD=128.
Sharding: tensor-parallel over heads across 8 cores. Core g computes
q-heads 4g..4g+3 with kv-head g, and a row-parallel partial of the output
projection; the host sums the 8 partials (the unshard for row-parallel Wo).

Layout strategy (per core):
  - x is passed transposed (XT [DM,S]) so every projection matmul contracts
    over DM on the partition axis with no on-device transposes.
  - Q,K are produced transposed ([d, s]); V natural ([s, d]).
  - RMSNorm over d (= partition axis) uses a ones-vector matmul for the
    per-position sum of squares, then a GPSIMD partition_broadcast of 1/rms.
  - Scores are computed transposed: S^T[j,i] (j=key pos on partitions,
    i=query pos on free axis). With q scaled by 1/sqrt(HD) and RMSNormed,
    |s| <= sqrt(128) and the ALiBi bias slope*(j-i) <= 0 after causal
    masking, so exp() cannot overflow and NO row-max pass is needed.
    exp bias: +slope*(j-i0) enters via the ACT per-partition bias operand,
    -slope*(i-i0) via one row add (DVE/Pool alternating); the causal mask is
    a precomputed [128,128] additive -1e30 triangle on diagonal blocks.
  - P^T tiles feed the PV matmul as rhs with V as lhsT, accumulating O^T
    [d, i] directly in PSUM (no transposes anywhere). A ones-lhsT matmul
    accumulates the softmax denominators as a row, normalized via
    reciprocal + partition_broadcast.

Scheduling strategy (v2):
  - xt streams in 8 chunks per 512-slice ([P, 4o, 512]); the startup DMA
    order interleaves wq pieces with xt chunks (first loads split in half)
    so the PE starts ~3us in. wk/wv are host-packed to the SBUF layout so
    their DMA descriptors are 8KB runs (the [DM, HD] layout would give
    256B descriptors, which the DMA does at half throughput).
  - One PE "filler queue" per attention phase: attention for block ss runs
    with proj(ss+1) and outproj(ss-1) matmul steps pulled into the exp-
    latency bubbles of the jt pipeline (st(jt+depth) is emitted depth=2
    iterations ahead, 3 in the filler-poor last phase; otp/lps trail once
    exp(jt) lands). Q-projection filler runs as head PAIRS iterating
    o-major, so each xt chunk is consumed slower than its DMA delivers it
    (chunk-major sweeps outran the serial DMA engine and stalled). Each
    head's first score matmuls are emitted during the previous head's last
    iteration (attn_prologue chaining), and phase(ss+1)'s head-0 prologue
    is emitted under the last 8 filler steps of phase ss.
  - RMSNorm is split: chunk epilogues bank the sum-of-squares row and a
    raw bf16 copy (freeing PSUM); the ACT-table-switching Sqrt runs in one
    batched pocket per phase (2 LoadActFuncSet round trips per phase
    instead of ~10 -- Sqrt and Exp live in different ACT table sets).
  - Per-head ALiBi bias rows/columns are precomputed once (masters), so a
    head costs no setup.
  - Output stores: 4 PSUM->SBUF cast copies (alternating ACT/DVE; GPSIMD
    cannot read PSUM) into one [P,4,512] tile, then ONE merged DMA; the
    final m-slices use per-quad DMAs to shorten the kernel tail.
"""

import math
from collections import deque

import numpy as np
import ml_dtypes

import concourse.bass as bass
import concourse.bacc as bacc
import concourse.mybir as mybir
import concourse.tile as tile

F32 = mybir.dt.float32
BF16 = mybir.dt.bfloat16
AF = mybir.ActivationFunctionType
ALU = mybir.AluOpType

B, S, DM = 1, 2048, 4096
H, HKV, HD = 32, 8, 128
NC_CORES = 8
HL = H // NC_CORES          # 4 local q heads per core
EPS = 1e-6
NEG = -1.0e30
P = 128

NBF = ml_dtypes.bfloat16


def _alibi_slopes(n_heads: int) -> np.ndarray:
    start = 2 ** (-(2 ** (-(math.log2(n_heads) - 3))))
    return np.array([start * (start**i) for i in range(n_heads)], dtype=np.float32)


def build_module(s: int = S, repeat: int = 1, phases=('proj', 'attn', 'out')):
    """Build the per-core Bass module. `s` parameterized for small tests."""
    assert s % 512 == 0
    nss = s // 512            # 512-wide s slices / query blocks
    njt = s // P              # 128-wide key tiles
    ndm = DM // P             # 32 contraction tiles
    nxc = ndm // 4            # 8 xt chunks per slice (4 o's each)
    NJ = 4 * nss              # jcol master width
    RB = 4 * (nss - 1)        # ridx bias: ridx = jt - 4*ss + RB in [0, NJ)

    nc = bacc.Bacc(trn_type="TRN2")

    xt_d = nc.dram_tensor("xt", [DM, s], BF16, kind="ExternalInput")
    wq_d = nc.dram_tensor("wq", [DM, HL * HD], BF16, kind="ExternalInput")
    wk_d = nc.dram_tensor("wk", [P, (DM // P) * HD], BF16, kind="ExternalInput")
    wv_d = nc.dram_tensor("wv", [P, (DM // P) * HD], BF16, kind="ExternalInput")
    wo_d = nc.dram_tensor("wo", [HL * HD, DM], BF16, kind="ExternalInput")
    qnw_d = nc.dram_tensor("qnw", [HD, 1], F32, kind="ExternalInput")
    knw_d = nc.dram_tensor("knw", [HD, 1], F32, kind="ExternalInput")
    slp_d = nc.dram_tensor("slp", [P, HL], F32, kind="ExternalInput")
    nslp_d = nc.dram_tensor("nslp", [P, HL], F32, kind="ExternalInput")
    out_d = nc.dram_tensor("out", [s, DM], BF16, kind="ExternalOutput")

    with tile.TileContext(nc) as tc:
        with (
            tc.tile_pool(name="const", bufs=1) as const,
            tc.tile_pool(name="xtc", bufs=10) as xtc_pool,
            tc.tile_pool(name="big", bufs=1) as big,
            tc.tile_pool(name="sq", bufs=2) as sq_pool,
            tc.tile_pool(name="row1", bufs=3) as row1,
            tc.tile_pool(name="msr", bufs=6) as msr_pool,
            tc.tile_pool(name="raw", bufs=6) as raw_pool,
            tc.tile_pool(name="inv", bufs=2) as inv_pool,
            tc.tile_pool(name="tmp", bufs=4) as tmp_pool,
            tc.tile_pool(name="pt", bufs=4) as pt_pool,
            tc.tile_pool(name="fsb", bufs=2) as fsb_pool,
            tc.tile_pool(name="ps", bufs=8, space="PSUM") as ps,
        ):
            # ---------------- small constants (loads deferred into proj) --
            qnw_sb = const.tile([P, 1], F32)
            knw_sb = const.tile([P, 1], F32)
            slp_sb = const.tile([P, HL], F32)
            nslp_sb = const.tile([P, HL], F32)

            def load_small_consts():
                nc.sync.dma_start(qnw_sb, qnw_d[:, :])
                nc.sync.dma_start(knw_sb, knw_d[:, :])
                nc.sync.dma_start(slp_sb, slp_d[:, :])
                nc.sync.dma_start(nslp_sb, nslp_d[:, :])

            # weights tiles (loads interleaved below / deferred)
            wq_sb = const.tile([P, ndm, HL * HD], BF16)
            wq_r = wq_d[:, :].rearrange("(o p) m -> p o m", p=P)
            wk_sb = const.tile([P, ndm, HD], BF16)
            wv_sb = const.tile([P, ndm, HD], BF16)
            wo_sb = const.tile([P, HL, DM], BF16)
            wo_r = wo_d[:, :].rearrange("(o p) m -> p o m", p=P)

            ones_f32 = const.tile([P, 1], F32)
            nc.vector.memset(ones_f32, 1.0)
            ones_sb = const.tile([P, 1], mybir.dt.float32r)
            nc.scalar.copy(ones_sb, ones_f32)
            ones_bf = const.tile([P, 1], BF16)
            # sqrt(1.0) == 1.0; issuing it here pulls the ACT sqrt-table
            # load to t~0 (idle ACT) instead of mid-K-pass where it delays
            # the rmsnorm Squares queued behind it
            nc.scalar.activation(ones_bf, ones_f32, AF.Sqrt)
            eps_sb = const.tile([P, 1], F32)
            nc.vector.memset(eps_sb, EPS)

            # per-head ALiBi masters:
            #   negrowM[p, h, f] = -slope_h * f           (query-col row add)
            #   jcolM[p, h, t]   = slope_h * (128*(t-RB) + p)  (exp bias; at
            #       t = jt - 4*ss + RB it equals slope*(128*jt + p - 512*ss))
            iota_row = const.tile([P, 512], F32)
            nc.gpsimd.iota(iota_row, pattern=[[1, 512]], base=0,
                           channel_multiplier=0,
                           allow_small_or_imprecise_dtypes=True)
            iota_j = const.tile([P, NJ], F32)
            nc.gpsimd.iota(iota_j, pattern=[[P, NJ]], base=-RB * P,
                           channel_multiplier=1,
                           allow_small_or_imprecise_dtypes=True)
            negrowM = const.tile([P, HL, 512], F32)
            jcolM = const.tile([P, HL, NJ], F32)

            def build_masters():
                for h in range(HL):
                    nc.gpsimd.tensor_tensor(
                        negrowM[:, h, :], iota_row,
                        nslp_sb[:, h:h + 1].to_broadcast([P, 512]), ALU.mult)
                    nc.gpsimd.tensor_tensor(
                        jcolM[:, h, :], iota_j,
                        slp_sb[:, h:h + 1].to_broadcast([P, NJ]), ALU.mult)

            # maskneg[p, f] = 0 where p <= f else -1e30  (additive causal
            # mask for diagonal 128x128 blocks of S^T)
            maskneg = const.tile([P, P], F32)
            nc.gpsimd.memset(maskneg, 0.0)
            nc.gpsimd.affine_select(
                out=maskneg, in_=maskneg,
                compare_op=ALU.is_ge, fill=NEG,
                base=0, pattern=[[1, P]], channel_multiplier=-1,
            )

            # ---------------- persistent activations ----------------
            qt_sb = big.tile([P, HL, 2, 512], BF16)  # Q^T ring [d, h, ss%2, i]
            kt_sb = big.tile([P, s], BF16)           # K^T      [d, s]
            v_sb = big.tile([P, njt, HD], BF16)      # V        [s, d]
            ot_sb = big.tile([P, HL, 2, 512], BF16)  # O^T ring [d, h, ib%2, i]

            xt_r = xt_d[:, :].rearrange("(o p) t -> p o t", p=P)

            # xt chunk bookkeeping: chunks[(ss, c)] = sbuf tile [P, 4, 512]
            xt_chunks = {}

            def load_xt_chunk(ss, c):
                t = xtc_pool.tile([P, 4, 512], BF16, tag="xtc",
                                  name=f"xt{ss}_{c}")
                nc.sync.dma_start(
                    t, xt_r[:, 4 * c:4 * c + 4, ss * 512:ss * 512 + 512])
                xt_chunks[(ss, c)] = t

            def xt_o(ss, o):
                return xt_chunks[(ss, o // 4)][:, o % 4, :]

            # ---------------- rmsnorm chain (non-PE parts) ----------------
            def rms_chain_pre(src):
                """Square the PSUM tile; returns sq tile (ACT)."""
                sqt = sq_pool.tile([P, 512], mybir.dt.float32r,
                                   tag="sq", name="sqt")
                nc.scalar.activation(sqt, src, AF.Square)
                return sqt

            def rms_chain_post(src, psq, w_sb, dst):
                """After PE computed psq = ones^T @ sqt: finish the norm."""
                rms = row1.tile([1, 512], F32, tag="row1", name="rms")
                nc.scalar.activation(rms, psq, AF.Sqrt,
                                     bias=eps_sb[:1, :], scale=1.0 / HD)
                rec = row1.tile([1, 512], F32, tag="row1", name="rec")
                nc.vector.reciprocal(rec, rms)
                invb = inv_pool.tile([P, 512], F32, tag="inv", name="invb")
                nc.gpsimd.partition_broadcast(invb, rec)
                nc.vector.scalar_tensor_tensor(
                    out=dst, in0=src, scalar=w_sb, in1=invb,
                    op0=ALU.mult, op1=ALU.mult)

            # ---------------- initial projection (slice 0) ----------------
            def proj_initial():
                """proj(0): DMA-paced. Q pass (o-major over 4 heads), K pass,
                V pass; weight quarters interleave with xt chunks."""
                # Q pass
                pq = [ps.tile([P, 512], F32, tag="ps", name=f"pq{c}")
                      for c in range(HL)]
                for o in range(ndm):
                    if o == 0:          # finest first loads: PE starts ~3us
                        nc.sync.dma_start(wq_sb[:, 0:2, :], wq_r[:, 0:2, :])
                        t0 = xtc_pool.tile([P, 4, 512], BF16, tag="xtc",
                                           name="xt0_0")
                        nc.sync.dma_start(t0[:, 0:2, :], xt_r[:, 0:2, 0:512])
                        nc.sync.dma_start(wq_sb[:, 2:4, :], wq_r[:, 2:4, :])
                        nc.sync.dma_start(t0[:, 2:4, :], xt_r[:, 2:4, 0:512])
                        xt_chunks[(0, 0)] = t0
                    elif o == 2:
                        nc.sync.dma_start(wq_sb[:, 4:8, :], wq_r[:, 4:8, :])
                    elif o % 8 == 0:    # wq quarter q = o//8
                        q = o // 8
                        nc.sync.dma_start(
                            wq_sb[:, 8 * q:8 * q + 8, :],
                            wq_r[:, 8 * q:8 * q + 8, :])
                    if o % 4 == 0 and o > 0:
                        load_xt_chunk(0, o // 4)
                    if o == 1:
                        load_small_consts()
                        build_masters()
                    if o == ndm - 8:
                        nc.sync.dma_start(
                            wk_sb, wk_d[:, :].rearrange(
                                "p (o m) -> p o m", o=ndm))
                        nc.sync.dma_start(
                            wv_sb, wv_d[:, :].rearrange(
                                "p (o m) -> p o m", o=ndm))
                    for c in range(HL):
                        nc.tensor.matmul(
                            pq[c], wq_sb[:, o, c * HD:(c + 1) * HD],
                            xt_o(0, o), start=(o == 0), stop=(o == ndm - 1))
                # K pass (Q rmsnorm chains interleave into it)
                pk = ps.tile([P, 512], F32, tag="ps", name="pk")
                sqs = {}
                for o in range(ndm):
                    nc.tensor.matmul(pk, wk_sb[:, o, :], xt_o(0, o),
                                     start=(o == 0), stop=(o == ndm - 1))
                    if o % 8 == 1 and o // 8 < HL:
                        c = o // 8
                        sqs[c] = rms_chain_pre(pq[c])
                    elif o % 8 == 5 and o // 8 < HL:
                        c = o // 8
                        psq = ps.tile([1, 512], F32, tag="ps", name="psq")
                        nc.tensor.matmul(psq, ones_sb, sqs[c],
                                         start=True, stop=True)
                        rms_chain_post(pq[c], psq, qnw_sb,
                                       qt_sb[:, c, 0, :])
                # V chunks interleave around the K rms chain and the
                # attn(0) prologue: V c=0 covers ACT finishing the Q-chain
                # sqrts before the K sum-of-squares matmul; V c=1,2 cover
                # the K chain -> kt latency before st0; V c=3 covers the
                # first exp warm-up.
                def v_chunk(c):
                    pv = ps.tile([P, HD], F32, tag="ps", name="pv")
                    for o in range(ndm):
                        nc.tensor.matmul(
                            pv, xt_chunks[(0, o // 4)][:, o % 4,
                                                       c * P:(c + 1) * P],
                            wv_sb[:, o, :],
                            start=(o == 0), stop=(o == ndm - 1))
                    nc.scalar.copy(v_sb[:, c, :], pv)

                v_chunk(0)
                sqk = rms_chain_pre(pk)
                psqk = ps.tile([1, 512], F32, tag="ps", name="psqk")
                nc.tensor.matmul(psqk, ones_sb, sqk, start=True, stop=True)
                rms_chain_post(pk, psqk, knw_sb, kt_sb[:, 0:512])
                # throwaway Exp: pulls the exp-table load under the V-pass
                # matmuls instead of into head 0's first softmax chain
                warm = row1.tile([1, 1], F32, tag="row1", name="warm")
                nc.scalar.activation(warm, eps_sb[:1, :1], AF.Exp)
                v_chunk(1)
                v_chunk(2)
                ctx0 = attn_prologue(0, 0) if 'attn' in phases else None
                v_chunk(3)
                return ctx0

            # ---------------- filler step generators ----------------
            # Each filler step is a closure emitting ~1-2 PE matmuls (plus
            # trailing non-PE ops). Steps are pulled into attention bubbles.

            def rms_defer_a(src, pending, w_sb, dst, defer_after=None):
                """Chunk epilogue: sum-of-squares row + raw copy to SBUF so
                the PSUM accumulator frees; the ACT-table-switching sqrt
                runs later in one batched pocket per phase. The psq matmul
                is deferred a few filler steps (via defer_after) so the PE
                does not stall waiting for the ACT Square."""
                sqt = rms_chain_pre(src)

                def part_b():
                    psq = ps.tile([1, 512], F32, tag="ps", name="psq")
                    nc.tensor.matmul(psq, ones_sb, sqt, start=True, stop=True)
                    msrow = msr_pool.tile([1, 512], F32, tag="msr",
                                          name="msr")
                    nc.scalar.copy(msrow, psq)
                    raw = raw_pool.tile([P, 512], BF16, tag="raw", name="raw")
                    cast_copy(raw, src)
                    pending.append((msrow, raw, w_sb, dst))

                if defer_after is None:
                    part_b()
                else:
                    defer_after(3, part_b)

            def rms_defer_b(msrow, raw, w_sb, dst):
                rms = row1.tile([1, 512], F32, tag="row1", name="rms")
                nc.scalar.activation(rms, msrow, AF.Sqrt,
                                     bias=eps_sb[:1, :], scale=1.0 / HD)
                rec = row1.tile([1, 512], F32, tag="row1", name="rec")
                nc.vector.reciprocal(rec, rms)
                invb = inv_pool.tile([P, 512], F32, tag="inv", name="invb")
                nc.gpsimd.partition_broadcast(invb, rec)
                nc.vector.scalar_tensor_tensor(
                    out=dst, in0=raw, scalar=w_sb, in1=invb,
                    op0=ALU.mult, op1=ALU.mult)

            def proj_q_pair_steps(ss, c0, c1, pending, defer_after):
                """Q projection of heads c0,c1 for slice ss, o-major so a
                given xt chunk is consumed at half the single-head rate
                (stays behind the chunk DMA arrival): 64 steps."""
                state = {}

                def step(o, c):
                    def f():
                        if o == 0:
                            state[c] = ps.tile([P, 512], F32, tag="ps",
                                               name=f"pq{ss}_{c}")
                        nc.tensor.matmul(
                            state[c], wq_sb[:, o, c * HD:(c + 1) * HD],
                            xt_o(ss, o), start=(o == 0), stop=(o == ndm - 1))
                        if o == ndm - 1:
                            rms_defer_a(state[c], pending, qnw_sb,
                                        qt_sb[:, c, ss % 2, :], defer_after)
                    return f
                steps = []
                for o in range(ndm):
                    steps.append(step(o, c0))
                    steps.append(step(o, c1))
                return steps

            def proj_k_steps(ss, pending, defer_after):
                state = {}

                def step(o):
                    def f():
                        if o == 0:
                            state['pk'] = ps.tile([P, 512], F32, tag="ps",
                                                  name=f"pk{ss}")
                        nc.tensor.matmul(
                            state['pk'], wk_sb[:, o, :], xt_o(ss, o),
                            start=(o == 0), stop=(o == ndm - 1))
                        if o == ndm - 1:
                            rms_defer_a(state['pk'], pending, knw_sb,
                                        kt_sb[:, ss * 512:ss * 512 + 512],
                                        defer_after)
                    return f
                return [step(o) for o in range(ndm)]

            def proj_v_steps(ss, cs=(0, 1, 2, 3)):
                """V projection: pos-chunks x 4 steps of 8 matmuls."""
                state = {}
                steps = []

                def step(c, g):
                    def f():
                        if g == 0:
                            state[c] = ps.tile([P, HD], F32, tag="ps",
                                               name=f"pv{ss}_{c}")
                        for o in range(8 * g, 8 * g + 8):
                            nc.tensor.matmul(
                                state[c],
                                xt_chunks[(ss, o // 4)][:, o % 4,
                                                        c * P:(c + 1) * P],
                                wv_sb[:, o, :],
                                start=(o == 0), stop=(o == ndm - 1))
                        if g == 3:
                            nc.scalar.copy(v_sb[:, 4 * ss + c, :], state[c])
                    return f
                for c in cs:
                    for g in range(4):
                        steps.append(step(c, g))
                return steps

            _copy_rr = [0]
            _copy_mode = [2]    # 2 = alternate; 3 = 2xACT:1xDVE (DVE-heavy
                                # attention phases)

            def cast_copy(dst, src):
                """PSUM f32 -> SBUF bf16 cast copy over the two PSUM-capable
                non-PE engines (GPSIMD cannot read PSUM)."""
                r = _copy_rr[0] = (_copy_rr[0] + 1) % _copy_mode[0]
                if r != 0:
                    nc.scalar.copy(dst, src)
                else:
                    nc.vector.tensor_copy(dst, src)

            def outproj_steps(ib, mi, split_dma=False):
                """Output projection block ib, m-slice mi: 8 steps of 2
                matmuls; one merged DMA after the 4 quads (or 2 half DMAs
                when split_dma, to shorten the kernel tail)."""
                m0 = mi * 512
                state = {}
                steps = []

                def step(st_i, half):
                    def f():
                        if half == 0:
                            state['fps'] = ps.tile([P, 512], F32, tag="ps",
                                                   name=f"fps{ib}_{mi}")
                            if st_i == 0:
                                state['fsb'] = fsb_pool.tile(
                                    [P, 4, 512], BF16, tag="fsb", name="fsb")
                        for c in (0, 1) if half == 0 else (2, 3):
                            nc.tensor.matmul(
                                state['fps'],
                                ot_sb[:, c, ib % 2, st_i * P:(st_i + 1) * P],
                                wo_sb[:, c, m0:m0 + 512],
                                start=(c == 0), stop=(c == HL - 1))
                        if half == 1:
                            cast_copy(state['fsb'][:, st_i, :], state['fps'])
                            if split_dma:   # per-quad DMA: shortest tail
                                dst = out_d[ib * 512 + st_i * P:
                                            ib * 512 + (st_i + 1) * P,
                                            m0:m0 + 512]
                                nc.sync.dma_start(dst, state['fsb'][:, st_i, :])
                            elif st_i == 3:
                                dst = out_d[ib * 512:ib * 512 + 512,
                                            m0:m0 + 512]
                                nc.sync.dma_start(
                                    dst.rearrange("(st p) m -> p st m", p=P),
                                    state['fsb'])
                    return f
                for st_i in range(4):
                    steps.append(step(st_i, 0))
                    steps.append(step(st_i, 1))
                return steps

            # ---------------- attention ----------------
            def attn_prologue(ss, h):
                """Allocate the head's PSUM accumulators and emit its first
                two score matmuls. Called from the PREVIOUS head's last jt
                iteration so the exp chain of head h warms while the PE is
                still busy, killing the head-boundary bubble."""
                i0 = ss * 512
                jtend = 4 * (ss + 1)
                # last phase has little filler: run a deeper score pipeline
                # (PSUM has room there -- no proj-chunk accumulators live)
                depth = 2 if ss + 1 < nss else 3
                ctx = {'otp': ps.tile([P, 512], F32, tag="ps", name="otp"),
                       'lps': ps.tile([1, 512], F32, tag="ps", name="lps"),
                       'sts': {}, 'pts': {}, 'depth': depth}

                def emit_st(jt):
                    j0 = jt * P
                    c0 = max(0, j0 - i0)
                    stt = ps.tile([P, 512], F32, tag="ps", name="st")
                    nc.tensor.matmul(
                        stt[:, c0:], kt_sb[:, j0:j0 + P],
                        qt_sb[:, h, ss % 2, c0:], start=True, stop=True)
                    ctx['sts'][jt] = (stt, c0)

                def emit_chain(jt):
                    # bias row-add -> (diag mask) -> exp, emitted one full
                    # iteration ahead of the consuming PV matmul so the
                    # chain latency hides under an entire iteration of PE
                    # work instead of ~800ns
                    stt, c0 = ctx['sts'].pop(jt)
                    tmp = tmp_pool.tile([P, 512], F32, tag="tmp", name="tmp")
                    nc.vector.tensor_tensor(
                        tmp[:, c0:], stt[:, c0:], negrowM[:, h, c0:], ALU.add)
                    if jt * P >= i0:
                        nc.gpsimd.tensor_tensor(
                            tmp[:, c0:c0 + P], tmp[:, c0:c0 + P],
                            maskneg, ALU.add)
                    pt = pt_pool.tile([P, 512], BF16, tag="pt", name="pt")
                    ridx = jt - 4 * ss + RB
                    nc.scalar.activation(
                        pt[:, c0:], tmp[:, c0:], AF.Exp,
                        bias=jcolM[:, h, ridx:ridx + 1], scale=1.0)
                    ctx['pts'][jt] = (pt, c0)

                ctx['emit_st'] = emit_st
                ctx['emit_chain'] = emit_chain
                for jt in range(min(depth, jtend)):
                    emit_st(jt)
                emit_chain(0)
                return ctx

            def attn_head(ss, h, pull, ctx, next_prologue=None):
                """Body of one head's attention; `ctx` from attn_prologue.
                `next_prologue` (if set) is invoked during the last jt
                iteration and its result returned."""
                i0 = ss * 512
                jtend = 4 * (ss + 1)
                jlast = jtend - 1
                otp, lps = ctx['otp'], ctx['lps']
                emit_st = ctx['emit_st']
                emit_chain = ctx['emit_chain']
                pts = ctx['pts']
                depth = ctx['depth']
                nctx = None
                for jt in range(jtend):
                    if jt + 1 < jtend:
                        emit_chain(jt + 1)
                    if jt + depth < jtend:
                        emit_st(jt + depth)
                    pull(2 if jt < jlast else 1)
                    if jt == jlast and next_prologue is not None:
                        nctx = next_prologue()
                    pt, c0 = pts.pop(jt)
                    nc.tensor.matmul(
                        otp[:, c0:], v_sb[:, jt, :], pt[:, c0:],
                        start=(jt == 0), stop=(jt == jlast))
                    nc.tensor.matmul(
                        lps[:, c0:], ones_bf, pt[:, c0:],
                        start=(jt == 0), stop=(jt == jlast))
                # normalize
                lrow = row1.tile([1, 512], F32, tag="row1", name="lrow")
                nc.scalar.copy(lrow, lps)
                linv = row1.tile([1, 512], F32, tag="row1", name="linv")
                nc.vector.reciprocal(linv, lrow)
                linvb = inv_pool.tile([P, 512], F32, tag="inv", name="linvb")
                nc.gpsimd.partition_broadcast(linvb, linv)
                nc.vector.tensor_tensor(
                    ot_sb[:, h, ss % 2, :], otp, linvb, ALU.mult)
                return nctx

            # ---------------- phase schedule ----------------
            def phase(ss, ctx):
                """attn(ss) + filler proj(ss+1) + outproj(ss-1). `ctx` is
                head 0's prologue (emitted by the previous phase); returns
                the next phase's head-0 prologue ctx."""
                _copy_mode[0] = 2
                # prefetch DMAs for the next slice; wo afterwards during
                # phase 0 (xt chunks gate proj(1) filler NOW, wo is not
                # read until outproj(0) in phase 1)
                if ss + 1 < nss:
                    for c in range(nxc):
                        load_xt_chunk(ss + 1, c)
                if ss == 0:
                    for q in range(4):
                        nc.sync.dma_start(
                            wo_sb[:, :, q * (DM // 4):(q + 1) * (DM // 4)],
                            wo_r[:, :, q * (DM // 4):(q + 1) * (DM // 4)])

                fill = deque()
                pending = []
                deferred = []

                def defer_after(n, f):
                    deferred.append([n, f])

                def emit_one():
                    for ent in deferred[:]:
                        ent[0] -= 1
                        if ent[0] <= 0:
                            deferred.remove(ent)
                            ent[1]()
                    if fill:
                        fill.popleft()()

                if ss + 1 < nss and 'proj' in phases:
                    qs = [proj_q_pair_steps(ss + 1, 0, 1, pending,
                                            defer_after),
                          proj_q_pair_steps(ss + 1, 2, 3, pending,
                                            defer_after)]
                    ks = proj_k_steps(ss + 1, pending, defer_after)
                    vs = proj_v_steps(ss + 1)
                else:
                    qs, ks, vs = [], None, None
                if ss >= 1 and 'out' in phases:
                    os_ = [outproj_steps(ss - 1, mi) for mi in range(8)]
                else:
                    os_ = []
                # interleave: outproj mi-pairs between proj chunks (paced by
                # xt arrival); K early so the rsqrt pocket (right after q3)
                # finishes well before the next phase needs qt/kt, with the
                # remaining outproj + V steps as padding behind it.
                def chain_step(i):
                    def f():
                        rms_defer_b(*pending[i])
                        if i == HL:   # pocket done: re-warm the exp table
                            warm = row1.tile([1, 1], F32, tag="row1",
                                             name="warm")
                            nc.scalar.activation(warm, eps_sb[:1, :1],
                                                 AF.Exp)
                    return f
                order = []
                proj_units = ([qs[0], ks, qs[1],
                               [chain_step(i) for i in range(HL + 1)], vs]
                              if ks is not None else [])
                ou = list(os_)
                # zip: outproj pair, proj unit, outproj pair, proj unit, ...
                pi = 0
                for i in range(max(len(ou), len(proj_units))):
                    if i < len(ou):
                        order.append(ou[i])
                    if pi < len(proj_units):
                        order.append(proj_units[pi])
                        pi += 1
                for lst in order:
                    fill.extend(lst)

                def pull(n):
                    for _ in range(n):
                        emit_one()

                total = len(fill)
                if ctx is None and 'attn' in phases:
                    ctx = attn_prologue(ss, 0)
                for h in range(HL):
                    if 'attn' in phases:
                        nxt = None
                        if h + 1 < HL:
                            def nxt(hh=h + 1):
                                return attn_prologue(ss, hh)
                        ctx = attn_head(ss, h, pull, ctx, nxt)
                    # drain this head's share of the filler, holding back a
                    # few steps to cover the next phase's exp warm-up
                    hold = 14 if h == HL - 1 else 0
                    target = (total * (h + 1)) // HL - hold
                    while len(fill) > max(0, total - target):
                        emit_one()
                nctx = None
                if ss + 1 < nss and 'attn' in phases:
                    nctx = attn_prologue(ss + 1, 0)
                while fill:
                    emit_one()
                for ent in deferred:   # flush stragglers
                    ent[1]()
                return nctx

            def outproj_final(ib):
                _copy_mode[0] = 2
                for mi in range(8):
                    for st in outproj_steps(ib, mi, split_dma=(mi >= 6)):
                        st()

            for _rep in range(repeat):
                ctx = None
                if 'proj' in phases:
                    ctx = proj_initial()
                for ss in range(nss):
                    ctx = phase(ss, ctx)
                if 'out' in phases and 'attn' in phases:
                    outproj_final(nss - 1)

    nc.finalize()
    return nc


def _prep_kv(w):
    """[HD, DM] weight -> [P, ndm*HD] laid out as [p][o][m] (8KB runs)."""
    ndm = DM // P
    wt = np.ascontiguousarray(w.T)                       # [DM, HD]
    return np.ascontiguousarray(
        wt.reshape(ndm, P, HD).transpose(1, 0, 2).reshape(P, ndm * HD)
    ).astype(NBF)


def shard_inputs(x, Wq, Wk, Wv, Wo, q_norm_w, k_norm_w, s=S):
    """Host-side shard + layout prep. Returns per-core input maps."""
    slopes = _alibi_slopes(H)
    xt = np.ascontiguousarray(x.reshape(s, DM).T).astype(NBF)
    qnw = (np.asarray(q_norm_w, np.float32) / math.sqrt(HD)).reshape(HD, 1)
    knw = np.asarray(k_norm_w, np.float32).reshape(HD, 1).copy()
    in_maps = []
    for g in range(NC_CORES):
        qs = g * HL * HD
        sl = slopes[g * HL:(g + 1) * HL]
        in_maps.append({
            "xt": xt,
            "wq": np.ascontiguousarray(Wq[qs:qs + HL * HD, :].T).astype(NBF),
            "wk": _prep_kv(Wk[g * HD:(g + 1) * HD, :]),
            "wv": _prep_kv(Wv[g * HD:(g + 1) * HD, :]),
            "wo": np.ascontiguousarray(Wo[:, qs:qs + HL * HD].T).astype(NBF),
            "qnw": qnw,
            "knw": knw,
            "slp": np.ascontiguousarray(
                np.broadcast_to(sl, (P, HL))).astype(np.float32),
            "nslp": np.ascontiguousarray(
                np.broadcast_to(-sl, (P, HL))).astype(np.float32),
        })
    return in_maps


_MODULE_CACHE = {}
LAST_RESULT = None


def _get_module(s=S):
    if s not in _MODULE_CACHE:
        _MODULE_CACHE[s] = build_module(s)
    return _MODULE_CACHE[s]


def kernel(x, Wq, Wk, Wv, Wo, q_norm_w, k_norm_w, **run_kwargs):
    global LAST_RESULT
    from concourse.bass_utils import run_bass_kernel_spmd

    x = np.asarray(x)
    in_maps = shard_inputs(np.asarray(x), np.asarray(Wq), np.asarray(Wk),
                           np.asarray(Wv), np.asarray(Wo),
                           np.asarray(q_norm_w), np.asarray(k_norm_w))
    nc = _get_module(S)
    res = run_bass_kernel_spmd(nc, in_maps, core_ids=list(range(NC_CORES)),
                               **run_kwargs)
    LAST_RESULT = res
    acc = np.zeros((S, DM), np.float32)
    for r in res.results:
        acc += r["out"].astype(np.float32)
    return acc.reshape(B, S, DM)


# revision 85
# speedup vs baseline: 1.2370x; 1.0019x over previous
"""Trainium2 Bass kernel: GQA causal self-attention with ALiBi + QK-RMSNorm.

Model: B=1, S=2048, DM=4096, H=32 q-heads, HKV=8 kv-heads, HD=128.
Sharding: tensor-parallel over heads across 8 cores. Core g computes
q-heads 4g..4g+3 with kv-head g, and a row-parallel partial of the output
projection; the host sums the 8 partials (the unshard for row-parallel Wo).

Layout strategy (per core):
  - x is passed transposed (XT [DM,S]) so every projection matmul contracts
    over DM on the partition axis with no on-device transposes.
  - Q,K are produced transposed ([d, s]); V natural ([s, d]).
  - RMSNorm over d (= partition axis) uses a ones-vector matmul for the
    per-position sum of squares, then a GPSIMD partition_broadcast of 1/rms.
  - Scores are computed transposed: S^T[j,i] (j=key pos on partitions,
    i=query pos on free axis). With q scaled by 1/sqrt(HD) and RMSNormed,
    |s| <= sqrt(128) and the ALiBi bias slope*(j-i) <= 0 after causal
    masking, so exp() cannot overflow and NO row-max pass is needed.
    exp bias: +slope*(j-i0) enters via the ACT per-partition bias operand,
    -slope*(i-i0) via one row add (DVE/Pool alternating); the causal mask is
    a precomputed [128,128] additive -1e30 triangle on diagonal blocks.
  - P^T tiles feed the PV matmul as rhs with V as lhsT, accumulating O^T
    [d, i] directly in PSUM (no transposes anywhere). A ones-lhsT matmul
    accumulates the softmax denominators as a row, normalized via
    reciprocal + partition_broadcast.

Scheduling strategy (v2):
  - xt streams in 8 chunks per 512-slice ([P, 4o, 512]); the startup DMA
    order interleaves wq pieces with xt chunks (first loads split in half)
    so the PE starts ~3us in. wk/wv are host-packed to the SBUF layout so
    their DMA descriptors are 8KB runs (the [DM, HD] layout would give
    256B descriptors, which the DMA does at half throughput).
  - One PE "filler queue" per attention phase: attention for block ss runs
    with proj(ss+1) and outproj(ss-1) matmul steps pulled into the exp-
    latency bubbles of the jt pipeline (st(jt+depth) is emitted depth=2
    iterations ahead, 3 in the filler-poor last phase; otp/lps trail once
    exp(jt) lands). Q-projection filler runs as head PAIRS iterating
    o-major, so each xt chunk is consumed slower than its DMA delivers it
    (chunk-major sweeps outran the serial DMA engine and stalled). Each
    head's first score matmuls are emitted during the previous head's last
    iteration (attn_prologue chaining), and phase(ss+1)'s head-0 prologue
    is emitted under the last 8 filler steps of phase ss.
  - RMSNorm is split: chunk epilogues issue the ACT Square, then a
    countdown queue defers the sum-of-squares matmul + copies by ~3 filler
    steps so the PE never stalls on the Square; the ACT-table-switching
    Sqrt runs in one batched pocket per phase (2 LoadActFuncSet round
    trips per phase instead of ~10 -- Sqrt and Exp live in different ACT
    table sets), and dummy Sqrt/Exp ops at idle points pre-warm the
    tables out of the critical chains.
  - Per-head ALiBi bias rows/columns are precomputed once (masters), so a
    head costs no setup.
  - Output stores: 4 PSUM->SBUF cast copies (alternating ACT/DVE; GPSIMD
    cannot read PSUM) into one [P,4,512] tile, then ONE merged DMA; the
    final m-slices use per-quad DMAs to shorten the kernel tail.
"""

import math
from collections import deque

import numpy as np
import ml_dtypes

import concourse.bass as bass
import concourse.bacc as bacc
import concourse.mybir as mybir
import concourse.tile as tile

F32 = mybir.dt.float32
BF16 = mybir.dt.bfloat16
AF = mybir.ActivationFunctionType
ALU = mybir.AluOpType

B, S, DM = 1, 2048, 4096
H, HKV, HD = 32, 8, 128
NC_CORES = 8
HL = H // NC_CORES          # 4 local q heads per core
EPS = 1e-6
NEG = -1.0e30
P = 128

NBF = ml_dtypes.bfloat16


def _alibi_slopes(n_heads: int) -> np.ndarray:
    start = 2 ** (-(2 ** (-(math.log2(n_heads) - 3))))
    return np.array([start * (start**i) for i in range(n_heads)], dtype=np.float32)


def build_module(s: int = S, repeat: int = 1, phases=('proj', 'attn', 'out')):
    """Build the per-core Bass module. `s` parameterized for small tests."""
    assert s % 512 == 0
    nss = s // 512            # 512-wide s slices / query blocks
    njt = s // P              # 128-wide key tiles
    ndm = DM // P             # 32 contraction tiles
    nxc = ndm // 4            # 8 xt chunks per slice (4 o's each)
    NJ = 4 * nss              # jcol master width
    RB = 4 * (nss - 1)        # ridx bias: ridx = jt - 4*ss + RB in [0, NJ)

    nc = bacc.Bacc(trn_type="TRN2")

    xt_d = nc.dram_tensor("xt", [DM, s], BF16, kind="ExternalInput")
    wq_d = nc.dram_tensor("wq", [DM, HL * HD], BF16, kind="ExternalInput")
    wk_d = nc.dram_tensor("wk", [P, (DM // P) * HD], BF16, kind="ExternalInput")
    wv_d = nc.dram_tensor("wv", [P, (DM // P) * HD], BF16, kind="ExternalInput")
    wo_d = nc.dram_tensor("wo", [HL * HD, DM], BF16, kind="ExternalInput")
    qnw_d = nc.dram_tensor("qnw", [HD, 1], F32, kind="ExternalInput")
    knw_d = nc.dram_tensor("knw", [HD, 1], F32, kind="ExternalInput")
    slp_d = nc.dram_tensor("slp", [P, HL], F32, kind="ExternalInput")
    nslp_d = nc.dram_tensor("nslp", [P, HL], F32, kind="ExternalInput")
    out_d = nc.dram_tensor("out", [s, DM], BF16, kind="ExternalOutput")

    with tile.TileContext(nc) as tc:
        with (
            tc.tile_pool(name="const", bufs=1) as const,
            tc.tile_pool(name="xtc", bufs=10) as xtc_pool,
            tc.tile_pool(name="big", bufs=1) as big,
            tc.tile_pool(name="sq", bufs=2) as sq_pool,
            tc.tile_pool(name="row1", bufs=3) as row1,
            tc.tile_pool(name="msr", bufs=6) as msr_pool,
            tc.tile_pool(name="raw", bufs=6) as raw_pool,
            tc.tile_pool(name="inv", bufs=2) as inv_pool,
            tc.tile_pool(name="tmp", bufs=4) as tmp_pool,
            tc.tile_pool(name="pt", bufs=4) as pt_pool,
            tc.tile_pool(name="fsb", bufs=2) as fsb_pool,
            tc.tile_pool(name="ps", bufs=8, space="PSUM") as ps,
        ):
            # ---------------- small constants (loads deferred into proj) --
            qnw_sb = const.tile([P, 1], F32)
            knw_sb = const.tile([P, 1], F32)
            slp_sb = const.tile([P, HL], F32)
            nslp_sb = const.tile([P, HL], F32)

            def load_small_consts():
                nc.sync.dma_start(qnw_sb, qnw_d[:, :])
                nc.sync.dma_start(knw_sb, knw_d[:, :])
                nc.sync.dma_start(slp_sb, slp_d[:, :])
                nc.sync.dma_start(nslp_sb, nslp_d[:, :])

            # weights tiles (loads interleaved below / deferred)
            wq_sb = const.tile([P, ndm, HL * HD], BF16)
            wq_r = wq_d[:, :].rearrange("(o p) m -> p o m", p=P)
            wk_sb = const.tile([P, ndm, HD], BF16)
            wv_sb = const.tile([P, ndm, HD], BF16)
            wo_sb = const.tile([P, HL, DM], BF16)
            wo_r = wo_d[:, :].rearrange("(o p) m -> p o m", p=P)

            ones_f32 = const.tile([P, 1], F32)
            nc.vector.memset(ones_f32, 1.0)
            ones_sb = const.tile([P, 1], mybir.dt.float32r)
            nc.scalar.copy(ones_sb, ones_f32)
            ones_bf = const.tile([P, 1], BF16)
            # sqrt(1.0) == 1.0; issuing it here pulls the ACT sqrt-table
            # load to t~0 (idle ACT) instead of mid-K-pass where it delays
            # the rmsnorm Squares queued behind it
            nc.scalar.activation(ones_bf, ones_f32, AF.Sqrt)
            eps_sb = const.tile([P, 1], F32)
            nc.vector.memset(eps_sb, EPS)

            # per-head ALiBi masters:
            #   negrowM[p, h, f] = -slope_h * f           (query-col row add)
            #   jcolM[p, h, t]   = slope_h * (128*(t-RB) + p)  (exp bias; at
            #       t = jt - 4*ss + RB it equals slope*(128*jt + p - 512*ss))
            iota_row = const.tile([P, 512], F32)
            nc.gpsimd.iota(iota_row, pattern=[[1, 512]], base=0,
                           channel_multiplier=0,
                           allow_small_or_imprecise_dtypes=True)
            iota_j = const.tile([P, NJ], F32)
            nc.gpsimd.iota(iota_j, pattern=[[P, NJ]], base=-RB * P,
                           channel_multiplier=1,
                           allow_small_or_imprecise_dtypes=True)
            negrowM = const.tile([P, HL, 512], F32)
            jcolM = const.tile([P, HL, NJ], F32)

            def build_masters():
                for h in range(HL):
                    nc.gpsimd.tensor_tensor(
                        negrowM[:, h, :], iota_row,
                        nslp_sb[:, h:h + 1].to_broadcast([P, 512]), ALU.mult)
                    nc.gpsimd.tensor_tensor(
                        jcolM[:, h, :], iota_j,
                        slp_sb[:, h:h + 1].to_broadcast([P, NJ]), ALU.mult)

            # maskneg[p, f] = 0 where p <= f else -1e30  (additive causal
            # mask for diagonal 128x128 blocks of S^T)
            maskneg = const.tile([P, P], F32)
            nc.gpsimd.memset(maskneg, 0.0)
            nc.gpsimd.affine_select(
                out=maskneg, in_=maskneg,
                compare_op=ALU.is_ge, fill=NEG,
                base=0, pattern=[[1, P]], channel_multiplier=-1,
            )

            # ---------------- persistent activations ----------------
            qt_sb = big.tile([P, HL, 2, 512], BF16)  # Q^T ring [d, h, ss%2, i]
            kt_sb = big.tile([P, s], BF16)           # K^T      [d, s]
            v_sb = big.tile([P, njt, HD], BF16)      # V        [s, d]
            ot_sb = big.tile([P, HL, 2, 512], BF16)  # O^T ring [d, h, ib%2, i]

            xt_r = xt_d[:, :].rearrange("(o p) t -> p o t", p=P)

            # xt chunk bookkeeping: chunks[(ss, c)] = sbuf tile [P, 4, 512]
            xt_chunks = {}

            def load_xt_chunk(ss, c):
                t = xtc_pool.tile([P, 4, 512], BF16, tag="xtc",
                                  name=f"xt{ss}_{c}")
                nc.sync.dma_start(
                    t, xt_r[:, 4 * c:4 * c + 4, ss * 512:ss * 512 + 512])
                xt_chunks[(ss, c)] = t

            def xt_o(ss, o):
                return xt_chunks[(ss, o // 4)][:, o % 4, :]

            # ---------------- rmsnorm chain (non-PE parts) ----------------
            def rms_chain_pre(src):
                """Square the PSUM tile; returns sq tile (ACT)."""
                sqt = sq_pool.tile([P, 512], mybir.dt.float32r,
                                   tag="sq", name="sqt")
                nc.scalar.activation(sqt, src, AF.Square)
                return sqt

            def rms_chain_post(src, psq, w_sb, dst):
                """After PE computed psq = ones^T @ sqt: finish the norm."""
                rms = row1.tile([1, 512], F32, tag="row1", name="rms")
                nc.scalar.activation(rms, psq, AF.Sqrt,
                                     bias=eps_sb[:1, :], scale=1.0 / HD)
                rec = row1.tile([1, 512], F32, tag="row1", name="rec")
                nc.vector.reciprocal(rec, rms)
                invb = inv_pool.tile([P, 512], F32, tag="inv", name="invb")
                nc.gpsimd.partition_broadcast(invb, rec)
                nc.vector.scalar_tensor_tensor(
                    out=dst, in0=src, scalar=w_sb, in1=invb,
                    op0=ALU.mult, op1=ALU.mult)

            # ---------------- initial projection (slice 0) ----------------
            def proj_initial():
                """proj(0): DMA-paced. Q pass (o-major over 4 heads), K pass,
                V pass; weight quarters interleave with xt chunks."""
                # Q pass
                pq = [ps.tile([P, 512], F32, tag="ps", name=f"pq{c}")
                      for c in range(HL)]
                for o in range(ndm):
                    if o == 0:          # finest first loads: PE starts ~3us
                        nc.sync.dma_start(wq_sb[:, 0:2, :], wq_r[:, 0:2, :])
                        t0 = xtc_pool.tile([P, 4, 512], BF16, tag="xtc",
                                           name="xt0_0")
                        nc.sync.dma_start(t0[:, 0:2, :], xt_r[:, 0:2, 0:512])
                        nc.sync.dma_start(wq_sb[:, 2:4, :], wq_r[:, 2:4, :])
                        nc.sync.dma_start(t0[:, 2:4, :], xt_r[:, 2:4, 0:512])
                        xt_chunks[(0, 0)] = t0
                    elif o == 2:
                        nc.sync.dma_start(wq_sb[:, 4:8, :], wq_r[:, 4:8, :])
                    elif o % 8 == 0:    # wq quarter q = o//8
                        q = o // 8
                        nc.sync.dma_start(
                            wq_sb[:, 8 * q:8 * q + 8, :],
                            wq_r[:, 8 * q:8 * q + 8, :])
                    if o % 4 == 0 and o > 0:
                        load_xt_chunk(0, o // 4)
                    if o == 1:
                        load_small_consts()
                        build_masters()
                    if o == ndm - 8:
                        nc.sync.dma_start(
                            wk_sb, wk_d[:, :].rearrange(
                                "p (o m) -> p o m", o=ndm))
                        nc.sync.dma_start(
                            wv_sb, wv_d[:, :].rearrange(
                                "p (o m) -> p o m", o=ndm))
                    for c in range(HL):
                        nc.tensor.matmul(
                            pq[c], wq_sb[:, o, c * HD:(c + 1) * HD],
                            xt_o(0, o), start=(o == 0), stop=(o == ndm - 1))
                # K pass (Q rmsnorm chains interleave into it)
                pk = ps.tile([P, 512], F32, tag="ps", name="pk")
                sqs = {}
                for o in range(ndm):
                    nc.tensor.matmul(pk, wk_sb[:, o, :], xt_o(0, o),
                                     start=(o == 0), stop=(o == ndm - 1))
                    if o % 8 == 1 and o // 8 < HL:
                        c = o // 8
                        sqs[c] = rms_chain_pre(pq[c])
                    elif o % 8 == 5 and o // 8 < HL:
                        c = o // 8
                        psq = ps.tile([1, 512], F32, tag="ps", name="psq")
                        nc.tensor.matmul(psq, ones_sb, sqs[c],
                                         start=True, stop=True)
                        rms_chain_post(pq[c], psq, qnw_sb,
                                       qt_sb[:, c, 0, :])
                # V chunks interleave around the K rms chain and the
                # attn(0) prologue: V c=0 covers ACT finishing the Q-chain
                # sqrts before the K sum-of-squares matmul; V c=1,2 cover
                # the K chain -> kt latency before st0; V c=3 covers the
                # first exp warm-up.
                def v_chunk(c):
                    pv = ps.tile([P, HD], F32, tag="ps", name="pv")
                    for o in range(ndm):
                        nc.tensor.matmul(
                            pv, xt_chunks[(0, o // 4)][:, o % 4,
                                                       c * P:(c + 1) * P],
                            wv_sb[:, o, :],
                            start=(o == 0), stop=(o == ndm - 1))
                    nc.scalar.copy(v_sb[:, c, :], pv)

                v_chunk(0)
                sqk = rms_chain_pre(pk)
                psqk = ps.tile([1, 512], F32, tag="ps", name="psqk")
                nc.tensor.matmul(psqk, ones_sb, sqk, start=True, stop=True)
                rms_chain_post(pk, psqk, knw_sb, kt_sb[:, 0:512])
                # throwaway Exp: pulls the exp-table load under the V-pass
                # matmuls instead of into head 0's first softmax chain
                warm = row1.tile([1, 1], F32, tag="row1", name="warm")
                nc.scalar.activation(warm, eps_sb[:1, :1], AF.Exp)
                v_chunk(1)
                v_chunk(2)
                ctx0 = attn_prologue(0, 0) if 'attn' in phases else None
                v_chunk(3)
                return ctx0

            # ---------------- filler step generators ----------------
            # Each filler step is a closure emitting ~1-2 PE matmuls (plus
            # trailing non-PE ops). Steps are pulled into attention bubbles.

            def rms_defer_a(src, pending, w_sb, dst, defer_after=None):
                """Chunk epilogue: sum-of-squares row + raw copy to SBUF so
                the PSUM accumulator frees; the ACT-table-switching sqrt
                runs later in one batched pocket per phase. The psq matmul
                is deferred a few filler steps (via defer_after) so the PE
                does not stall waiting for the ACT Square."""
                sqt = rms_chain_pre(src)

                def part_b():
                    psq = ps.tile([1, 512], F32, tag="ps", name="psq")
                    nc.tensor.matmul(psq, ones_sb, sqt, start=True, stop=True)
                    msrow = msr_pool.tile([1, 512], F32, tag="msr",
                                          name="msr")
                    nc.scalar.copy(msrow, psq)
                    raw = raw_pool.tile([P, 512], BF16, tag="raw", name="raw")
                    cast_copy(raw, src)
                    pending.append((msrow, raw, w_sb, dst))

                if defer_after is None:
                    part_b()
                else:
                    defer_after(5, part_b)

            def rms_defer_b(msrow, raw, w_sb, dst):
                rms = row1.tile([1, 512], F32, tag="row1", name="rms")
                nc.scalar.activation(rms, msrow, AF.Sqrt,
                                     bias=eps_sb[:1, :], scale=1.0 / HD)
                rec = row1.tile([1, 512], F32, tag="row1", name="rec")
                nc.vector.reciprocal(rec, rms)
                invb = inv_pool.tile([P, 512], F32, tag="inv", name="invb")
                nc.gpsimd.partition_broadcast(invb, rec)
                nc.vector.scalar_tensor_tensor(
                    out=dst, in0=raw, scalar=w_sb, in1=invb,
                    op0=ALU.mult, op1=ALU.mult)

            def proj_q_pair_steps(ss, c0, c1, pending, defer_after):
                """Q projection of heads c0,c1 for slice ss, o-major so a
                given xt chunk is consumed at half the single-head rate
                (stays behind the chunk DMA arrival): 64 steps."""
                state = {}

                def step(o, c):
                    def f():
                        if o == 0:
                            state[c] = ps.tile([P, 512], F32, tag="ps",
                                               name=f"pq{ss}_{c}")
                        nc.tensor.matmul(
                            state[c], wq_sb[:, o, c * HD:(c + 1) * HD],
                            xt_o(ss, o), start=(o == 0), stop=(o == ndm - 1))
                        if o == ndm - 1:
                            rms_defer_a(state[c], pending, qnw_sb,
                                        qt_sb[:, c, ss % 2, :], defer_after)
                    return f
                steps = []
                for o in range(ndm):
                    steps.append(step(o, c0))
                    steps.append(step(o, c1))
                return steps

            def proj_k_steps(ss, pending, defer_after):
                state = {}

                def step(o):
                    def f():
                        if o == 0:
                            state['pk'] = ps.tile([P, 512], F32, tag="ps",
                                                  name=f"pk{ss}")
                        nc.tensor.matmul(
                            state['pk'], wk_sb[:, o, :], xt_o(ss, o),
                            start=(o == 0), stop=(o == ndm - 1))
                        if o == ndm - 1:
                            rms_defer_a(state['pk'], pending, knw_sb,
                                        kt_sb[:, ss * 512:ss * 512 + 512],
                                        defer_after)
                    return f
                return [step(o) for o in range(ndm)]

            def proj_v_steps(ss, cs=(0, 1, 2, 3)):
                """V projection: pos-chunks x 4 steps of 8 matmuls."""
                state = {}
                steps = []

                def step(c, g):
                    def f():
                        if g == 0:
                            state[c] = ps.tile([P, HD], F32, tag="ps",
                                               name=f"pv{ss}_{c}")
                        for o in range(8 * g, 8 * g + 8):
                            nc.tensor.matmul(
                                state[c],
                                xt_chunks[(ss, o // 4)][:, o % 4,
                                                        c * P:(c + 1) * P],
                                wv_sb[:, o, :],
                                start=(o == 0), stop=(o == ndm - 1))
                        if g == 3:
                            nc.scalar.copy(v_sb[:, 4 * ss + c, :], state[c])
                    return f
                for c in cs:
                    for g in range(4):
                        steps.append(step(c, g))
                return steps

            _copy_rr = [0]
            _copy_mode = [2]    # 2 = alternate; 3 = 2xACT:1xDVE (DVE-heavy
                                # attention phases)

            def cast_copy(dst, src):
                """PSUM f32 -> SBUF bf16 cast copy over the two PSUM-capable
                non-PE engines (GPSIMD cannot read PSUM)."""
                r = _copy_rr[0] = (_copy_rr[0] + 1) % _copy_mode[0]
                if r != 0:
                    nc.scalar.copy(dst, src)
                else:
                    nc.vector.tensor_copy(dst, src)

            def outproj_steps(ib, mi, split_dma=False):
                """Output projection block ib, m-slice mi: 8 steps of 2
                matmuls; one merged DMA after the 4 quads (or 2 half DMAs
                when split_dma, to shorten the kernel tail)."""
                m0 = mi * 512
                state = {}
                steps = []

                def step(st_i, half):
                    def f():
                        if half == 0:
                            state['fps'] = ps.tile([P, 512], F32, tag="ps",
                                                   name=f"fps{ib}_{mi}")
                            if st_i == 0:
                                state['fsb'] = fsb_pool.tile(
                                    [P, 4, 512], BF16, tag="fsb", name="fsb")
                        for c in (0, 1) if half == 0 else (2, 3):
                            nc.tensor.matmul(
                                state['fps'],
                                ot_sb[:, c, ib % 2, st_i * P:(st_i + 1) * P],
                                wo_sb[:, c, m0:m0 + 512],
                                start=(c == 0), stop=(c == HL - 1))
                        if half == 1:
                            cast_copy(state['fsb'][:, st_i, :], state['fps'])
                            if split_dma:   # per-quad DMA: shortest tail
                                dst = out_d[ib * 512 + st_i * P:
                                            ib * 512 + (st_i + 1) * P,
                                            m0:m0 + 512]
                                nc.sync.dma_start(dst, state['fsb'][:, st_i, :])
                            elif st_i == 3:
                                dst = out_d[ib * 512:ib * 512 + 512,
                                            m0:m0 + 512]
                                nc.sync.dma_start(
                                    dst.rearrange("(st p) m -> p st m", p=P),
                                    state['fsb'])
                    return f
                for st_i in range(4):
                    steps.append(step(st_i, 0))
                    steps.append(step(st_i, 1))
                return steps

            # ---------------- attention ----------------
            def attn_prologue(ss, h):
                """Allocate the head's PSUM accumulators and emit its first
                two score matmuls. Called from the PREVIOUS head's last jt
                iteration so the exp chain of head h warms while the PE is
                still busy, killing the head-boundary bubble."""
                i0 = ss * 512
                jtend = 4 * (ss + 1)
                # last phase has little filler: run a deeper score pipeline
                # (PSUM has room there -- no proj-chunk accumulators live)
                depth = 2 if ss + 1 < nss else 3
                ctx = {'otp': ps.tile([P, 512], F32, tag="ps", name="otp"),
                       'lps': ps.tile([1, 512], F32, tag="ps", name="lps"),
                       'sts': {}, 'pts': {}, 'depth': depth}

                def emit_st(jt):
                    j0 = jt * P
                    c0 = max(0, j0 - i0)
                    stt = ps.tile([P, 512], F32, tag="ps", name="st")
                    nc.tensor.matmul(
                        stt[:, c0:], kt_sb[:, j0:j0 + P],
                        qt_sb[:, h, ss % 2, c0:], start=True, stop=True)
                    ctx['sts'][jt] = (stt, c0)

                def emit_chain(jt):
                    # bias row-add -> (diag mask) -> exp, emitted one full
                    # iteration ahead of the consuming PV matmul so the
                    # chain latency hides under an entire iteration of PE
                    # work instead of ~800ns
                    stt, c0 = ctx['sts'].pop(jt)
                    tmp = tmp_pool.tile([P, 512], F32, tag="tmp", name="tmp")
                    nc.vector.tensor_tensor(
                        tmp[:, c0:], stt[:, c0:], negrowM[:, h, c0:], ALU.add)
                    if jt * P >= i0:
                        nc.gpsimd.tensor_tensor(
                            tmp[:, c0:c0 + P], tmp[:, c0:c0 + P],
                            maskneg, ALU.add)
                    pt = pt_pool.tile([P, 512], BF16, tag="pt", name="pt")
                    ridx = jt - 4 * ss + RB
                    nc.scalar.activation(
                        pt[:, c0:], tmp[:, c0:], AF.Exp,
                        bias=jcolM[:, h, ridx:ridx + 1], scale=1.0)
                    ctx['pts'][jt] = (pt, c0)

                ctx['emit_st'] = emit_st
                ctx['emit_chain'] = emit_chain
                for jt in range(min(depth, jtend)):
                    emit_st(jt)
                emit_chain(0)
                return ctx

            def attn_head(ss, h, pull, ctx, next_prologue=None):
                """Body of one head's attention; `ctx` from attn_prologue.
                `next_prologue` (if set) is invoked during the last jt
                iteration and its result returned."""
                i0 = ss * 512
                jtend = 4 * (ss + 1)
                jlast = jtend - 1
                otp, lps = ctx['otp'], ctx['lps']
                emit_st = ctx['emit_st']
                emit_chain = ctx['emit_chain']
                pts = ctx['pts']
                depth = ctx['depth']
                nctx = None
                for jt in range(jtend):
                    if jt + 1 < jtend:
                        emit_chain(jt + 1)
                    if jt + depth < jtend:
                        emit_st(jt + depth)
                    pull(2 if jt < jlast else 1)
                    if jt == jlast and next_prologue is not None:
                        nctx = next_prologue()
                    pt, c0 = pts.pop(jt)
                    nc.tensor.matmul(
                        otp[:, c0:], v_sb[:, jt, :], pt[:, c0:],
                        start=(jt == 0), stop=(jt == jlast))
                    nc.tensor.matmul(
                        lps[:, c0:], ones_bf, pt[:, c0:],
                        start=(jt == 0), stop=(jt == jlast))
                # normalize
                lrow = row1.tile([1, 512], F32, tag="row1", name="lrow")
                nc.scalar.copy(lrow, lps)
                linv = row1.tile([1, 512], F32, tag="row1", name="linv")
                nc.vector.reciprocal(linv, lrow)
                linvb = inv_pool.tile([P, 512], F32, tag="inv", name="linvb")
                nc.gpsimd.partition_broadcast(linvb, linv)
                nc.vector.tensor_tensor(
                    ot_sb[:, h, ss % 2, :], otp, linvb, ALU.mult)
                return nctx

            # ---------------- phase schedule ----------------
            def phase(ss, ctx):
                """attn(ss) + filler proj(ss+1) + outproj(ss-1). `ctx` is
                head 0's prologue (emitted by the previous phase); returns
                the next phase's head-0 prologue ctx."""
                _copy_mode[0] = 2
                # prefetch DMAs for the next slice; wo afterwards during
                # phase 0 (xt chunks gate proj(1) filler NOW, wo is not
                # read until outproj(0) in phase 1)
                if ss + 1 < nss:
                    for c in range(nxc):
                        load_xt_chunk(ss + 1, c)
                if ss == 0:
                    for q in range(4):
                        nc.sync.dma_start(
                            wo_sb[:, :, q * (DM // 4):(q + 1) * (DM // 4)],
                            wo_r[:, :, q * (DM // 4):(q + 1) * (DM // 4)])

                fill = deque()
                pending = []
                deferred = []

                def defer_after(n, f):
                    deferred.append([n, f])

                def emit_one():
                    for ent in deferred[:]:
                        ent[0] -= 1
                        if ent[0] <= 0:
                            deferred.remove(ent)
                            ent[1]()
                    if fill:
                        fill.popleft()()

                if ss + 1 < nss and 'proj' in phases:
                    qs = [proj_q_pair_steps(ss + 1, 0, 1, pending,
                                            defer_after),
                          proj_q_pair_steps(ss + 1, 2, 3, pending,
                                            defer_after)]
                    ks = proj_k_steps(ss + 1, pending, defer_after)
                    vs = proj_v_steps(ss + 1)
                else:
                    qs, ks, vs = [], None, None
                if ss >= 1 and 'out' in phases:
                    os_ = [outproj_steps(ss - 1, mi) for mi in range(8)]
                else:
                    os_ = []
                # interleave: outproj mi-pairs between proj chunks (paced by
                # xt arrival); K early so the rsqrt pocket (right after q3)
                # finishes well before the next phase needs qt/kt, with the
                # remaining outproj + V steps as padding behind it.
                def chain_step(i):
                    def f():
                        rms_defer_b(*pending[i])
                        if i == HL:   # pocket done: re-warm the exp table
                            warm = row1.tile([1, 1], F32, tag="row1",
                                             name="warm")
                            nc.scalar.activation(warm, eps_sb[:1, :1],
                                                 AF.Exp)
                    return f
                order = []
                proj_units = ([qs[0], ks, qs[1],
                               [chain_step(i) for i in range(HL + 1)], vs]
                              if ks is not None else [])
                ou = list(os_)
                # zip: outproj pair, proj unit, outproj pair, proj unit, ...
                pi = 0
                for i in range(max(len(ou), len(proj_units))):
                    if i < len(ou):
                        order.append(ou[i])
                    if pi < len(proj_units):
                        order.append(proj_units[pi])
                        pi += 1
                for lst in order:
                    fill.extend(lst)

                def pull(n):
                    for _ in range(n):
                        emit_one()

                total = len(fill)
                if ctx is None and 'attn' in phases:
                    ctx = attn_prologue(ss, 0)
                for h in range(HL):
                    if 'attn' in phases:
                        nxt = None
                        if h + 1 < HL:
                            def nxt(hh=h + 1):
                                return attn_prologue(ss, hh)
                        ctx = attn_head(ss, h, pull, ctx, nxt)
                    # drain this head's share of the filler, holding back a
                    # few steps to cover the next phase's exp warm-up
                    hold = 14 if h == HL - 1 else 0
                    target = (total * (h + 1)) // HL - hold
                    while len(fill) > max(0, total - target):
                        emit_one()
                nctx = None
                if ss + 1 < nss and 'attn' in phases:
                    nctx = attn_prologue(ss + 1, 0)
                while fill:
                    emit_one()
                for ent in deferred:   # flush stragglers
                    ent[1]()
                return nctx

            def outproj_final(ib):
                _copy_mode[0] = 2
                for mi in range(8):
                    for st in outproj_steps(ib, mi, split_dma=(mi >= 6)):
                        st()

            for _rep in range(repeat):
                ctx = None
                if 'proj' in phases:
                    ctx = proj_initial()
                for ss in range(nss):
                    ctx = phase(ss, ctx)
                if 'out' in phases and 'attn' in phases:
                    outproj_final(nss - 1)

    nc.finalize()
    return nc


def _prep_kv(w):
    """[HD, DM] weight -> [P, ndm*HD] laid out as [p][o][m] (8KB runs)."""
    ndm = DM // P
    wt = np.ascontiguousarray(w.T)                       # [DM, HD]
    return np.ascontiguousarray(
        wt.reshape(ndm, P, HD).transpose(1, 0, 2).reshape(P, ndm * HD)
    ).astype(NBF)


def shard_inputs(x, Wq, Wk, Wv, Wo, q_norm_w, k_norm_w, s=S):
    """Host-side shard + layout prep. Returns per-core input maps."""
    slopes = _alibi_slopes(H)
    xt = np.ascontiguousarray(x.reshape(s, DM).T).astype(NBF)
    qnw = (np.asarray(q_norm_w, np.float32) / math.sqrt(HD)).reshape(HD, 1)
    knw = np.asarray(k_norm_w, np.float32).reshape(HD, 1).copy()
    in_maps = []
    for g in range(NC_CORES):
        qs = g * HL * HD
        sl = slopes[g * HL:(g + 1) * HL]
        in_maps.append({
            "xt": xt,
            "wq": np.ascontiguousarray(Wq[qs:qs + HL * HD, :].T).astype(NBF),
            "wk": _prep_kv(Wk[g * HD:(g + 1) * HD, :]),
            "wv": _prep_kv(Wv[g * HD:(g + 1) * HD, :]),
            "wo": np.ascontiguousarray(Wo[:, qs:qs + HL * HD].T).astype(NBF),
            "qnw": qnw,
            "knw": knw,
            "slp": np.ascontiguousarray(
                np.broadcast_to(sl, (P, HL))).astype(np.float32),
            "nslp": np.ascontiguousarray(
                np.broadcast_to(-sl, (P, HL))).astype(np.float32),
        })
    return in_maps


_MODULE_CACHE = {}
LAST_RESULT = None


def _get_module(s=S):
    if s not in _MODULE_CACHE:
        _MODULE_CACHE[s] = build_module(s)
    return _MODULE_CACHE[s]


def kernel(x, Wq, Wk, Wv, Wo, q_norm_w, k_norm_w, **run_kwargs):
    global LAST_RESULT
    from concourse.bass_utils import run_bass_kernel_spmd

    x = np.asarray(x)
    in_maps = shard_inputs(np.asarray(x), np.asarray(Wq), np.asarray(Wk),
                           np.asarray(Wv), np.asarray(Wo),
                           np.asarray(q_norm_w), np.asarray(k_norm_w))
    nc = _get_module(S)
    res = run_bass_kernel_spmd(nc, in_maps, core_ids=list(range(NC_CORES)),
                               **run_kwargs)
    LAST_RESULT = res
    acc = np.zeros((S, DM), np.float32)
    for r in res.results:
        acc += r["out"].astype(np.float32)
    return acc.reshape(B, S, DM)


# revision 89
# speedup vs baseline: 1.2427x; 1.0046x over previous
"""Trainium2 Bass kernel: GQA causal self-attention with ALiBi + QK-RMSNorm.

Model: B=1, S=2048, DM=4096, H=32 q-heads, HKV=8 kv-heads, HD=128.
Sharding: tensor-parallel over heads across 8 cores. Core g computes
q-heads 4g..4g+3 with kv-head g, and a row-parallel partial of the output
projection; the host sums the 8 partials (the unshard for row-parallel Wo).

Layout strategy (per core):
  - x is passed transposed (XT [DM,S]) so every projection matmul contracts
    over DM on the partition axis with no on-device transposes.
  - Q,K are produced transposed ([d, s]); V natural ([s, d]).
  - RMSNorm over d (= partition axis) uses a ones-vector matmul for the
    per-position sum of squares, then a GPSIMD partition_broadcast of 1/rms.
  - Scores are computed transposed: S^T[j,i] (j=key pos on partitions,
    i=query pos on free axis). With q scaled by 1/sqrt(HD) and RMSNormed,
    |s| <= sqrt(128) and the ALiBi bias slope*(j-i) <= 0 after causal
    masking, so exp() cannot overflow and NO row-max pass is needed.
    exp bias: +slope*(j-i0) enters via the ACT per-partition bias operand,
    -slope*(i-i0) via one row add (DVE/Pool alternating); the causal mask is
    a precomputed [128,128] additive -1e30 triangle on diagonal blocks.
  - P^T tiles feed the PV matmul as rhs with V as lhsT, accumulating O^T
    [d, i] directly in PSUM (no transposes anywhere). A ones-lhsT matmul
    accumulates the softmax denominators as a row, normalized via
    reciprocal + partition_broadcast.

Scheduling strategy (v2):
  - xt streams in 8 chunks per 512-slice ([P, 4o, 512]); the startup DMA
    order interleaves wq pieces with xt chunks (first loads split in half)
    so the PE starts ~3us in. wk/wv are host-packed to the SBUF layout so
    their DMA descriptors are 8KB runs (the [DM, HD] layout would give
    256B descriptors, which the DMA does at half throughput).
  - One PE "filler queue" per attention phase: attention for block ss runs
    with proj(ss+1) and outproj(ss-1) matmul steps pulled into the exp-
    latency bubbles of the jt pipeline (st(jt+depth) is emitted depth=2
    iterations ahead, 3 in the filler-poor last phase; otp/lps trail once
    exp(jt) lands). Q-projection filler runs as head PAIRS iterating
    o-major, so each xt chunk is consumed slower than its DMA delivers it
    (chunk-major sweeps outran the serial DMA engine and stalled). Each
    head's first score matmuls are emitted during the previous head's last
    iteration (attn_prologue chaining), and phase(ss+1)'s head-0 prologue
    is emitted under the last 8 filler steps of phase ss.
  - RMSNorm is split: chunk epilogues issue the ACT Square, then a
    countdown queue defers the sum-of-squares matmul + copies by ~5 filler
    steps so the PE never stalls on the Square; the ACT-table-switching
    Sqrt runs in one batched pocket per phase (2 LoadActFuncSet round
    trips per phase instead of ~10 -- Sqrt and Exp live in different ACT
    table sets), and dummy Sqrt/Exp ops at idle points pre-warm the
    tables out of the critical chains.
  - Per-head ALiBi bias rows/columns are precomputed once (masters), so a
    head costs no setup.
  - Output stores: 4 PSUM->SBUF cast copies (alternating ACT/DVE; GPSIMD
    cannot read PSUM) into one [P,4,512] tile, then ONE merged DMA; the
    final m-slices use per-quad DMAs to shorten the kernel tail.
"""

import math
from collections import deque

import numpy as np
import ml_dtypes

import concourse.bass as bass
import concourse.bacc as bacc
import concourse.mybir as mybir
import concourse.tile as tile

F32 = mybir.dt.float32
BF16 = mybir.dt.bfloat16
AF = mybir.ActivationFunctionType
ALU = mybir.AluOpType

B, S, DM = 1, 2048, 4096
H, HKV, HD = 32, 8, 128
NC_CORES = 8
HL = H // NC_CORES          # 4 local q heads per core
EPS = 1e-6
NEG = -1.0e30
P = 128

NBF = ml_dtypes.bfloat16


def _alibi_slopes(n_heads: int) -> np.ndarray:
    start = 2 ** (-(2 ** (-(math.log2(n_heads) - 3))))
    return np.array([start * (start**i) for i in range(n_heads)], dtype=np.float32)


def build_module(s: int = S, repeat: int = 1, phases=('proj', 'attn', 'out')):
    """Build the per-core Bass module. `s` parameterized for small tests."""
    assert s % 512 == 0
    nss = s // 512            # 512-wide s slices / query blocks
    njt = s // P              # 128-wide key tiles
    ndm = DM // P             # 32 contraction tiles
    nxc = ndm // 4            # 8 xt chunks per slice (4 o's each)
    NJ = 4 * nss              # jcol master width
    RB = 4 * (nss - 1)        # ridx bias: ridx = jt - 4*ss + RB in [0, NJ)

    nc = bacc.Bacc(trn_type="TRN2")

    xt_d = nc.dram_tensor("xt", [DM, s], BF16, kind="ExternalInput")
    wq_d = nc.dram_tensor("wq", [DM, HL * HD], BF16, kind="ExternalInput")
    wk_d = nc.dram_tensor("wk", [P, (DM // P) * HD], BF16, kind="ExternalInput")
    wv_d = nc.dram_tensor("wv", [P, (DM // P) * HD], BF16, kind="ExternalInput")
    wo_d = nc.dram_tensor("wo", [HL * HD, DM], BF16, kind="ExternalInput")
    qnw_d = nc.dram_tensor("qnw", [HD, 1], F32, kind="ExternalInput")
    knw_d = nc.dram_tensor("knw", [HD, 1], F32, kind="ExternalInput")
    slp_d = nc.dram_tensor("slp", [P, HL], F32, kind="ExternalInput")
    nslp_d = nc.dram_tensor("nslp", [P, HL], F32, kind="ExternalInput")
    out_d = nc.dram_tensor("out", [s, DM], BF16, kind="ExternalOutput")

    with tile.TileContext(nc) as tc:
        with (
            tc.tile_pool(name="const", bufs=1) as const,
            tc.tile_pool(name="xtc", bufs=10) as xtc_pool,
            tc.tile_pool(name="big", bufs=1) as big,
            tc.tile_pool(name="sq", bufs=2) as sq_pool,
            tc.tile_pool(name="row1", bufs=3) as row1,
            tc.tile_pool(name="msr", bufs=6) as msr_pool,
            tc.tile_pool(name="raw", bufs=6) as raw_pool,
            tc.tile_pool(name="inv", bufs=2) as inv_pool,
            tc.tile_pool(name="tmp", bufs=4) as tmp_pool,
            tc.tile_pool(name="pt", bufs=4) as pt_pool,
            tc.tile_pool(name="fsb", bufs=2) as fsb_pool,
            tc.tile_pool(name="ps", bufs=8, space="PSUM") as ps,
        ):
            # ---------------- small constants (loads deferred into proj) --
            qnw_sb = const.tile([P, 1], F32)
            knw_sb = const.tile([P, 1], F32)
            slp_sb = const.tile([P, HL], F32)
            nslp_sb = const.tile([P, HL], F32)

            def load_small_consts():
                nc.sync.dma_start(qnw_sb, qnw_d[:, :])
                nc.sync.dma_start(knw_sb, knw_d[:, :])
                nc.sync.dma_start(slp_sb, slp_d[:, :])
                nc.sync.dma_start(nslp_sb, nslp_d[:, :])

            # weights tiles (loads interleaved below / deferred)
            wq_sb = const.tile([P, ndm, HL * HD], BF16)
            wq_r = wq_d[:, :].rearrange("(o p) m -> p o m", p=P)
            wk_sb = const.tile([P, ndm, HD], BF16)
            wv_sb = const.tile([P, ndm, HD], BF16)
            wo_sb = const.tile([P, HL, DM], BF16)
            wo_r = wo_d[:, :].rearrange("(o p) m -> p o m", p=P)

            ones_f32 = const.tile([P, 1], F32)
            nc.vector.memset(ones_f32, 1.0)
            ones_sb = const.tile([P, 1], mybir.dt.float32r)
            nc.scalar.copy(ones_sb, ones_f32)
            ones_bf = const.tile([P, 1], BF16)
            # sqrt(1.0) == 1.0; issuing it here pulls the ACT sqrt-table
            # load to t~0 (idle ACT) instead of mid-K-pass where it delays
            # the rmsnorm Squares queued behind it
            nc.scalar.activation(ones_bf, ones_f32, AF.Sqrt)
            eps_sb = const.tile([P, 1], F32)
            nc.vector.memset(eps_sb, EPS)

            # per-head ALiBi masters:
            #   negrowM[p, h, f] = -slope_h * f           (query-col row add)
            #   jcolM[p, h, t]   = slope_h * (128*(t-RB) + p)  (exp bias; at
            #       t = jt - 4*ss + RB it equals slope*(128*jt + p - 512*ss))
            iota_row = const.tile([P, 512], F32)
            nc.gpsimd.iota(iota_row, pattern=[[1, 512]], base=0,
                           channel_multiplier=0,
                           allow_small_or_imprecise_dtypes=True)
            iota_j = const.tile([P, NJ], F32)
            nc.gpsimd.iota(iota_j, pattern=[[P, NJ]], base=-RB * P,
                           channel_multiplier=1,
                           allow_small_or_imprecise_dtypes=True)
            negrowM = const.tile([P, HL, 512], F32)
            jcolM = const.tile([P, HL, NJ], F32)

            def build_masters():
                for h in range(HL):
                    nc.gpsimd.tensor_tensor(
                        negrowM[:, h, :], iota_row,
                        nslp_sb[:, h:h + 1].to_broadcast([P, 512]), ALU.mult)
                    nc.gpsimd.tensor_tensor(
                        jcolM[:, h, :], iota_j,
                        slp_sb[:, h:h + 1].to_broadcast([P, NJ]), ALU.mult)

            # maskneg[p, f] = 0 where p <= f else -1e30  (additive causal
            # mask for diagonal 128x128 blocks of S^T)
            maskneg = const.tile([P, P], F32)
            nc.gpsimd.memset(maskneg, 0.0)
            nc.gpsimd.affine_select(
                out=maskneg, in_=maskneg,
                compare_op=ALU.is_ge, fill=NEG,
                base=0, pattern=[[1, P]], channel_multiplier=-1,
            )

            # ---------------- persistent activations ----------------
            qt_sb = big.tile([P, HL, 2, 512], BF16)  # Q^T ring [d, h, ss%2, i]
            kt_sb = big.tile([P, s], BF16)           # K^T      [d, s]
            v_sb = big.tile([P, njt, HD], BF16)      # V        [s, d]
            ot_sb = big.tile([P, HL, 2, 512], BF16)  # O^T ring [d, h, ib%2, i]

            xt_r = xt_d[:, :].rearrange("(o p) t -> p o t", p=P)

            # xt chunk bookkeeping: chunks[(ss, c)] = sbuf tile [P, 4, 512]
            xt_chunks = {}

            def load_xt_chunk(ss, c):
                t = xtc_pool.tile([P, 4, 512], BF16, tag="xtc",
                                  name=f"xt{ss}_{c}")
                nc.sync.dma_start(
                    t, xt_r[:, 4 * c:4 * c + 4, ss * 512:ss * 512 + 512])
                xt_chunks[(ss, c)] = t

            def xt_o(ss, o):
                return xt_chunks[(ss, o // 4)][:, o % 4, :]

            # ---------------- rmsnorm chain (non-PE parts) ----------------
            def rms_chain_pre(src):
                """Square the PSUM tile; returns sq tile (ACT)."""
                sqt = sq_pool.tile([P, 512], mybir.dt.float32r,
                                   tag="sq", name="sqt")
                nc.scalar.activation(sqt, src, AF.Square)
                return sqt

            def rms_chain_post(src, psq, w_sb, dst):
                """After PE computed psq = ones^T @ sqt: finish the norm."""
                rms = row1.tile([1, 512], F32, tag="row1", name="rms")
                nc.scalar.activation(rms, psq, AF.Sqrt,
                                     bias=eps_sb[:1, :], scale=1.0 / HD)
                rec = row1.tile([1, 512], F32, tag="row1", name="rec")
                nc.vector.reciprocal(rec, rms)
                invb = inv_pool.tile([P, 512], F32, tag="inv", name="invb")
                nc.gpsimd.partition_broadcast(invb, rec)
                nc.vector.scalar_tensor_tensor(
                    out=dst, in0=src, scalar=w_sb, in1=invb,
                    op0=ALU.mult, op1=ALU.mult)

            # ---------------- initial projection (slice 0) ----------------
            def proj_initial():
                """proj(0): DMA-paced. Q pass (o-major over 4 heads), K pass,
                V pass; weight quarters interleave with xt chunks."""
                # Q pass
                pq = [ps.tile([P, 512], F32, tag="ps", name=f"pq{c}")
                      for c in range(HL)]
                for o in range(ndm):
                    if o == 0:          # finest first loads: PE starts ~3us
                        nc.sync.dma_start(wq_sb[:, 0:2, :], wq_r[:, 0:2, :])
                        t0 = xtc_pool.tile([P, 4, 512], BF16, tag="xtc",
                                           name="xt0_0")
                        nc.sync.dma_start(t0[:, 0:2, :], xt_r[:, 0:2, 0:512])
                        nc.sync.dma_start(wq_sb[:, 2:4, :], wq_r[:, 2:4, :])
                        nc.sync.dma_start(t0[:, 2:4, :], xt_r[:, 2:4, 0:512])
                        xt_chunks[(0, 0)] = t0
                    elif o == 2:
                        nc.sync.dma_start(wq_sb[:, 4:8, :], wq_r[:, 4:8, :])
                    elif o % 8 == 0:    # wq quarter q = o//8
                        q = o // 8
                        nc.sync.dma_start(
                            wq_sb[:, 8 * q:8 * q + 8, :],
                            wq_r[:, 8 * q:8 * q + 8, :])
                    if o % 4 == 0 and o > 0:
                        load_xt_chunk(0, o // 4)
                    if o == 1:
                        load_small_consts()
                        build_masters()
                    if o == ndm - 8:
                        nc.sync.dma_start(
                            wk_sb, wk_d[:, :].rearrange(
                                "p (o m) -> p o m", o=ndm))
                        nc.sync.dma_start(
                            wv_sb, wv_d[:, :].rearrange(
                                "p (o m) -> p o m", o=ndm))
                    for c in range(HL):
                        nc.tensor.matmul(
                            pq[c], wq_sb[:, o, c * HD:(c + 1) * HD],
                            xt_o(0, o), start=(o == 0), stop=(o == ndm - 1))
                # K pass (Q rmsnorm chains interleave into it)
                pk = ps.tile([P, 512], F32, tag="ps", name="pk")
                sqs = {}
                for o in range(ndm):
                    nc.tensor.matmul(pk, wk_sb[:, o, :], xt_o(0, o),
                                     start=(o == 0), stop=(o == ndm - 1))
                    if o % 8 == 1 and o // 8 < HL:
                        c = o // 8
                        sqs[c] = rms_chain_pre(pq[c])
                    elif o % 8 == 5 and o // 8 < HL:
                        c = o // 8
                        psq = ps.tile([1, 512], F32, tag="ps", name="psq")
                        nc.tensor.matmul(psq, ones_sb, sqs[c],
                                         start=True, stop=True)
                        rms_chain_post(pq[c], psq, qnw_sb,
                                       qt_sb[:, c, 0, :])
                # V chunks interleave around the K rms chain and the
                # attn(0) prologue: V c=0 covers ACT finishing the Q-chain
                # sqrts before the K sum-of-squares matmul; V c=1,2 cover
                # the K chain -> kt latency before st0; V c=3 covers the
                # first exp warm-up.
                def v_chunk(c):
                    pv = ps.tile([P, HD], F32, tag="ps", name="pv")
                    for o in range(ndm):
                        nc.tensor.matmul(
                            pv, xt_chunks[(0, o // 4)][:, o % 4,
                                                       c * P:(c + 1) * P],
                            wv_sb[:, o, :],
                            start=(o == 0), stop=(o == ndm - 1))
                    nc.scalar.copy(v_sb[:, c, :], pv)

                v_chunk(0)
                sqk = rms_chain_pre(pk)
                psqk = ps.tile([1, 512], F32, tag="ps", name="psqk")
                nc.tensor.matmul(psqk, ones_sb, sqk, start=True, stop=True)
                rms_chain_post(pk, psqk, knw_sb, kt_sb[:, 0:512])
                # throwaway Exp: pulls the exp-table load under the V-pass
                # matmuls instead of into head 0's first softmax chain
                warm = row1.tile([1, 1], F32, tag="row1", name="warm")
                nc.scalar.activation(warm, eps_sb[:1, :1], AF.Exp)
                v_chunk(1)
                v_chunk(2)
                ctx0 = attn_prologue(0, 0) if 'attn' in phases else None
                v_chunk(3)
                return ctx0

            # ---------------- filler step generators ----------------
            # Each filler step is a closure emitting ~1-2 PE matmuls (plus
            # trailing non-PE ops). Steps are pulled into attention bubbles.

            def rms_defer_a(src, pending, w_sb, dst, defer_after=None):
                """Chunk epilogue: sum-of-squares row + raw copy to SBUF so
                the PSUM accumulator frees; the ACT-table-switching sqrt
                runs later in one batched pocket per phase. The psq matmul
                is deferred a few filler steps (via defer_after) so the PE
                does not stall waiting for the ACT Square."""
                sqt = rms_chain_pre(src)

                def part_b():
                    psq = ps.tile([1, 512], F32, tag="ps", name="psq")
                    nc.tensor.matmul(psq, ones_sb, sqt, start=True, stop=True)
                    msrow = msr_pool.tile([1, 512], F32, tag="msr",
                                          name="msr")
                    nc.scalar.copy(msrow, psq)
                    raw = raw_pool.tile([P, 512], BF16, tag="raw", name="raw")
                    cast_copy(raw, src)
                    pending.append((msrow, raw, w_sb, dst))

                if defer_after is None:
                    part_b()
                else:
                    defer_after(5, part_b)

            def rms_defer_b(msrow, raw, w_sb, dst):
                rms = row1.tile([1, 512], F32, tag="row1", name="rms")
                nc.scalar.activation(rms, msrow, AF.Sqrt,
                                     bias=eps_sb[:1, :], scale=1.0 / HD)
                rec = row1.tile([1, 512], F32, tag="row1", name="rec")
                nc.vector.reciprocal(rec, rms)
                invb = inv_pool.tile([P, 512], F32, tag="inv", name="invb")
                nc.gpsimd.partition_broadcast(invb, rec)
                nc.vector.scalar_tensor_tensor(
                    out=dst, in0=raw, scalar=w_sb, in1=invb,
                    op0=ALU.mult, op1=ALU.mult)

            def proj_q_pair_steps(ss, c0, c1, pending, defer_after):
                """Q projection of heads c0,c1 for slice ss, o-major so a
                given xt chunk is consumed at half the single-head rate
                (stays behind the chunk DMA arrival): 64 steps."""
                state = {}

                def step(o, c):
                    def f():
                        if o == 0:
                            state[c] = ps.tile([P, 512], F32, tag="ps",
                                               name=f"pq{ss}_{c}")
                        nc.tensor.matmul(
                            state[c], wq_sb[:, o, c * HD:(c + 1) * HD],
                            xt_o(ss, o), start=(o == 0), stop=(o == ndm - 1))
                        if o == ndm - 1:
                            rms_defer_a(state[c], pending, qnw_sb,
                                        qt_sb[:, c, ss % 2, :], defer_after)
                    return f
                steps = []
                for o in range(ndm):
                    steps.append(step(o, c0))
                    steps.append(step(o, c1))
                return steps

            def proj_k_steps(ss, pending, defer_after):
                state = {}

                def step(o):
                    def f():
                        if o == 0:
                            state['pk'] = ps.tile([P, 512], F32, tag="ps",
                                                  name=f"pk{ss}")
                        nc.tensor.matmul(
                            state['pk'], wk_sb[:, o, :], xt_o(ss, o),
                            start=(o == 0), stop=(o == ndm - 1))
                        if o == ndm - 1:
                            rms_defer_a(state['pk'], pending, knw_sb,
                                        kt_sb[:, ss * 512:ss * 512 + 512],
                                        defer_after)
                    return f
                return [step(o) for o in range(ndm)]

            def proj_v_steps(ss, cs=(0, 1, 2, 3)):
                """V projection: pos-chunks x 4 steps of 8 matmuls."""
                state = {}
                steps = []

                def step(c, g):
                    def f():
                        if g == 0:
                            state[c] = ps.tile([P, HD], F32, tag="ps",
                                               name=f"pv{ss}_{c}")
                        for o in range(8 * g, 8 * g + 8):
                            nc.tensor.matmul(
                                state[c],
                                xt_chunks[(ss, o // 4)][:, o % 4,
                                                        c * P:(c + 1) * P],
                                wv_sb[:, o, :],
                                start=(o == 0), stop=(o == ndm - 1))
                        if g == 3:
                            nc.scalar.copy(v_sb[:, 4 * ss + c, :], state[c])
                    return f
                for c in cs:
                    for g in range(4):
                        steps.append(step(c, g))
                return steps

            _copy_rr = [0]
            _copy_mode = [2]    # 2 = alternate; 3 = 2xACT:1xDVE (DVE-heavy
                                # attention phases)

            def cast_copy(dst, src):
                """PSUM f32 -> SBUF bf16 cast copy over the two PSUM-capable
                non-PE engines (GPSIMD cannot read PSUM)."""
                r = _copy_rr[0] = (_copy_rr[0] + 1) % _copy_mode[0]
                if r != 0:
                    nc.scalar.copy(dst, src)
                else:
                    nc.vector.tensor_copy(dst, src)

            def outproj_steps(ib, mi, split_dma=False):
                """Output projection block ib, m-slice mi: 8 steps of 2
                matmuls; one merged DMA after the 4 quads (or 2 half DMAs
                when split_dma, to shorten the kernel tail)."""
                m0 = mi * 512
                state = {}
                steps = []

                def step(st_i, half):
                    def f():
                        if half == 0:
                            state['fps'] = ps.tile([P, 512], F32, tag="ps",
                                                   name=f"fps{ib}_{mi}")
                            if st_i == 0:
                                state['fsb'] = fsb_pool.tile(
                                    [P, 4, 512], BF16, tag="fsb", name="fsb")
                        for c in (0, 1) if half == 0 else (2, 3):
                            nc.tensor.matmul(
                                state['fps'],
                                ot_sb[:, c, ib % 2, st_i * P:(st_i + 1) * P],
                                wo_sb[:, c, m0:m0 + 512],
                                start=(c == 0), stop=(c == HL - 1))
                        if half == 1:
                            cast_copy(state['fsb'][:, st_i, :], state['fps'])
                            if split_dma:   # per-quad DMA: shortest tail
                                dst = out_d[ib * 512 + st_i * P:
                                            ib * 512 + (st_i + 1) * P,
                                            m0:m0 + 512]
                                nc.sync.dma_start(dst, state['fsb'][:, st_i, :])
                            elif st_i == 3:
                                dst = out_d[ib * 512:ib * 512 + 512,
                                            m0:m0 + 512]
                                nc.sync.dma_start(
                                    dst.rearrange("(st p) m -> p st m", p=P),
                                    state['fsb'])
                    return f
                for st_i in range(4):
                    steps.append(step(st_i, 0))
                    steps.append(step(st_i, 1))
                return steps

            # ---------------- attention ----------------
            def attn_prologue(ss, h):
                """Allocate the head's PSUM accumulators and emit its first
                two score matmuls. Called from the PREVIOUS head's last jt
                iteration so the exp chain of head h warms while the PE is
                still busy, killing the head-boundary bubble."""
                i0 = ss * 512
                jtend = 4 * (ss + 1)
                # last phase has little filler: run a deeper score pipeline
                # (PSUM has room there -- no proj-chunk accumulators live)
                depth = 2 if ss + 1 < nss else 3
                ctx = {'otp': ps.tile([P, 512], F32, tag="ps", name="otp"),
                       'lps': ps.tile([1, 512], F32, tag="ps", name="lps"),
                       'sts': {}, 'pts': {}, 'depth': depth}

                def emit_st(jt):
                    j0 = jt * P
                    c0 = max(0, j0 - i0)
                    stt = ps.tile([P, 512], F32, tag="ps", name="st")
                    nc.tensor.matmul(
                        stt[:, c0:], kt_sb[:, j0:j0 + P],
                        qt_sb[:, h, ss % 2, c0:], start=True, stop=True)
                    ctx['sts'][jt] = (stt, c0)

                def emit_chain(jt):
                    # bias row-add -> (diag mask) -> exp, emitted one full
                    # iteration ahead of the consuming PV matmul so the
                    # chain latency hides under an entire iteration of PE
                    # work instead of ~800ns
                    stt, c0 = ctx['sts'].pop(jt)
                    tmp = tmp_pool.tile([P, 512], F32, tag="tmp", name="tmp")
                    nc.vector.tensor_tensor(
                        tmp[:, c0:], stt[:, c0:], negrowM[:, h, c0:], ALU.add)
                    if jt * P >= i0:
                        nc.gpsimd.tensor_tensor(
                            tmp[:, c0:c0 + P], tmp[:, c0:c0 + P],
                            maskneg, ALU.add)
                    pt = pt_pool.tile([P, 512], BF16, tag="pt", name="pt")
                    ridx = jt - 4 * ss + RB
                    nc.scalar.activation(
                        pt[:, c0:], tmp[:, c0:], AF.Exp,
                        bias=jcolM[:, h, ridx:ridx + 1], scale=1.0)
                    ctx['pts'][jt] = (pt, c0)

                ctx['emit_st'] = emit_st
                ctx['emit_chain'] = emit_chain
                for jt in range(min(depth, jtend)):
                    emit_st(jt)
                emit_chain(0)
                return ctx

            def attn_head(ss, h, pull, ctx, next_prologue=None):
                """Body of one head's attention; `ctx` from attn_prologue.
                `next_prologue` (if set) is invoked during the last jt
                iteration and its result returned."""
                i0 = ss * 512
                jtend = 4 * (ss + 1)
                jlast = jtend - 1
                otp, lps = ctx['otp'], ctx['lps']
                emit_st = ctx['emit_st']
                emit_chain = ctx['emit_chain']
                pts = ctx['pts']
                depth = ctx['depth']
                nctx = None
                for jt in range(jtend):
                    if jt + 1 < jtend:
                        emit_chain(jt + 1)
                    if jt + depth < jtend:
                        emit_st(jt + depth)
                    pull((2 if ss + 1 < nss else 1)
                         if jt < jlast else 1)
                    if jt == jlast and next_prologue is not None:
                        nctx = next_prologue()
                    pt, c0 = pts.pop(jt)
                    nc.tensor.matmul(
                        otp[:, c0:], v_sb[:, jt, :], pt[:, c0:],
                        start=(jt == 0), stop=(jt == jlast))
                    nc.tensor.matmul(
                        lps[:, c0:], ones_bf, pt[:, c0:],
                        start=(jt == 0), stop=(jt == jlast))
                # normalize
                lrow = row1.tile([1, 512], F32, tag="row1", name="lrow")
                nc.scalar.copy(lrow, lps)
                linv = row1.tile([1, 512], F32, tag="row1", name="linv")
                nc.vector.reciprocal(linv, lrow)
                linvb = inv_pool.tile([P, 512], F32, tag="inv", name="linvb")
                nc.gpsimd.partition_broadcast(linvb, linv)
                nc.vector.tensor_tensor(
                    ot_sb[:, h, ss % 2, :], otp, linvb, ALU.mult)
                return nctx

            # ---------------- phase schedule ----------------
            def phase(ss, ctx):
                """attn(ss) + filler proj(ss+1) + outproj(ss-1). `ctx` is
                head 0's prologue (emitted by the previous phase); returns
                the next phase's head-0 prologue ctx."""
                _copy_mode[0] = 2
                # prefetch DMAs for the next slice; wo afterwards during
                # phase 0 (xt chunks gate proj(1) filler NOW, wo is not
                # read until outproj(0) in phase 1)
                if ss + 1 < nss:
                    for c in range(nxc):
                        load_xt_chunk(ss + 1, c)
                if ss == 0:
                    for q in range(4):
                        nc.sync.dma_start(
                            wo_sb[:, :, q * (DM // 4):(q + 1) * (DM // 4)],
                            wo_r[:, :, q * (DM // 4):(q + 1) * (DM // 4)])

                fill = deque()
                pending = []
                deferred = []

                def defer_after(n, f):
                    deferred.append([n, f])

                def emit_one():
                    for ent in deferred[:]:
                        ent[0] -= 1
                        if ent[0] <= 0:
                            deferred.remove(ent)
                            ent[1]()
                    if fill:
                        fill.popleft()()

                if ss + 1 < nss and 'proj' in phases:
                    qs = [proj_q_pair_steps(ss + 1, 0, 1, pending,
                                            defer_after),
                          proj_q_pair_steps(ss + 1, 2, 3, pending,
                                            defer_after)]
                    ks = proj_k_steps(ss + 1, pending, defer_after)
                    vs = proj_v_steps(ss + 1)
                else:
                    qs, ks, vs = [], None, None
                if ss >= 1 and 'out' in phases:
                    os_ = [outproj_steps(ss - 1, mi) for mi in range(8)]
                else:
                    os_ = []
                # interleave: outproj mi-pairs between proj chunks (paced by
                # xt arrival); K early so the rsqrt pocket (right after q3)
                # finishes well before the next phase needs qt/kt, with the
                # remaining outproj + V steps as padding behind it.
                def chain_step(i):
                    def f():
                        rms_defer_b(*pending[i])
                        if i == HL:   # pocket done: re-warm the exp table
                            warm = row1.tile([1, 1], F32, tag="row1",
                                             name="warm")
                            nc.scalar.activation(warm, eps_sb[:1, :1],
                                                 AF.Exp)
                    return f
                order = []
                proj_units = ([qs[0], ks, qs[1],
                               [chain_step(i) for i in range(HL + 1)], vs]
                              if ks is not None else [])
                ou = list(os_)
                # zip: outproj pair, proj unit, outproj pair, proj unit, ...
                pi = 0
                for i in range(max(len(ou), len(proj_units))):
                    if i < len(ou):
                        order.append(ou[i])
                    if pi < len(proj_units):
                        order.append(proj_units[pi])
                        pi += 1
                for lst in order:
                    fill.extend(lst)

                def pull(n):
                    for _ in range(n):
                        emit_one()

                total = len(fill)
                if ctx is None and 'attn' in phases:
                    ctx = attn_prologue(ss, 0)
                for h in range(HL):
                    if 'attn' in phases:
                        nxt = None
                        if h + 1 < HL:
                            def nxt(hh=h + 1):
                                return attn_prologue(ss, hh)
                        ctx = attn_head(ss, h, pull, ctx, nxt)
                    # drain this head's share of the filler, holding back a
                    # few steps to cover the next phase's exp warm-up
                    hold = 14 if h == HL - 1 else 0
                    target = (total * (h + 1)) // HL - hold
                    while len(fill) > max(0, total - target):
                        emit_one()
                nctx = None
                if ss + 1 < nss and 'attn' in phases:
                    nctx = attn_prologue(ss + 1, 0)
                while fill:
                    emit_one()
                for ent in deferred:   # flush stragglers
                    ent[1]()
                return nctx

            def outproj_final(ib):
                _copy_mode[0] = 2
                for mi in range(8):
                    for st in outproj_steps(ib, mi, split_dma=(mi >= 6)):
                        st()

            for _rep in range(repeat):
                ctx = None
                if 'proj' in phases:
                    ctx = proj_initial()
                for ss in range(nss):
                    ctx = phase(ss, ctx)
                if 'out' in phases and 'attn' in phases:
                    outproj_final(nss - 1)

    nc.finalize()
    return nc


def _prep_kv(w):
    """[HD, DM] weight -> [P, ndm*HD] laid out as [p][o][m] (8KB runs)."""
    ndm = DM // P
    wt = np.ascontiguousarray(w.T)                       # [DM, HD]
    return np.ascontiguousarray(
        wt.reshape(ndm, P, HD).transpose(1, 0, 2).reshape(P, ndm * HD)
    ).astype(NBF)


def shard_inputs(x, Wq, Wk, Wv, Wo, q_norm_w, k_norm_w, s=S):
    """Host-side shard + layout prep. Returns per-core input maps."""
    slopes = _alibi_slopes(H)
    xt = np.ascontiguousarray(x.reshape(s, DM).T).astype(NBF)
    qnw = (np.asarray(q_norm_w, np.float32) / math.sqrt(HD)).reshape(HD, 1)
    knw = np.asarray(k_norm_w, np.float32).reshape(HD, 1).copy()
    in_maps = []
    for g in range(NC_CORES):
        qs = g * HL * HD
        sl = slopes[g * HL:(g + 1) * HL]
        in_maps.append({
            "xt": xt,
            "wq": np.ascontiguousarray(Wq[qs:qs + HL * HD, :].T).astype(NBF),
            "wk": _prep_kv(Wk[g * HD:(g + 1) * HD, :]),
            "wv": _prep_kv(Wv[g * HD:(g + 1) * HD, :]),
            "wo": np.ascontiguousarray(Wo[:, qs:qs + HL * HD].T).astype(NBF),
            "qnw": qnw,
            "knw": knw,
            "slp": np.ascontiguousarray(
                np.broadcast_to(sl, (P, HL))).astype(np.float32),
            "nslp": np.ascontiguousarray(
                np.broadcast_to(-sl, (P, HL))).astype(np.float32),
        })
    return in_maps


_MODULE_CACHE = {}
LAST_RESULT = None


def _get_module(s=S):
    if s not in _MODULE_CACHE:
        _MODULE_CACHE[s] = build_module(s)
    return _MODULE_CACHE[s]


def kernel(x, Wq, Wk, Wv, Wo, q_norm_w, k_norm_w, **run_kwargs):
    global LAST_RESULT
    from concourse.bass_utils import run_bass_kernel_spmd

    x = np.asarray(x)
    in_maps = shard_inputs(np.asarray(x), np.asarray(Wq), np.asarray(Wk),
                           np.asarray(Wv), np.asarray(Wo),
                           np.asarray(q_norm_w), np.asarray(k_norm_w))
    nc = _get_module(S)
    res = run_bass_kernel_spmd(nc, in_maps, core_ids=list(range(NC_CORES)),
                               **run_kwargs)
    LAST_RESULT = res
    acc = np.zeros((S, DM), np.float32)
    for r in res.results:
        acc += r["out"].astype(np.float32)
    return acc.reshape(B, S, DM)
